# revision 1
# baseline (speedup 1.0000x reference)
"""MFGCGRU (graph-conv GRU cell) Trainium2 kernel.

Strategy: data-parallel over batch B=32 across 8 NeuronCores (4 batches
per core). All NxN supports replicated per core. Host pre-transposes
everything so the device never transposes:

  - adjacency matrices are passed as S^T [m, n] (bf16) and used as the
    *moving* matmul operand,
  - activations live feature-major: x_cat^T [66, N] with rows 0:64 = h,
    rows 64:66 = inputs (kernel rows permuted to match),
  - the diffusion conv is computed kernel-first:
        S_m @ (X @ k_m)  ==  (S_m X) k_m
    with Y_m = X @ k_m packed [node, 128] = [k_r | k_u] for the r/u pass
    (and batch-pairs for the c pass) so the PE always runs 128 wide,
  - the attention support is built unnormalized as e^T = exp(K Q^T / 8);
    its row-normalizer 1/d is applied to the e-contribution after the
    node contraction via a second PSUM accumulator.
"""

import contextlib
import os

import numpy as np
import ml_dtypes

import concourse.bass as bass
import concourse.bacc as bacc
import concourse.tile as tile
from concourse import mybir
from concourse.bass_utils import run_bass_kernel_spmd

F32 = mybir.dt.float32
BF16 = mybir.dt.bfloat16
AF = mybir.ActivationFunctionType

B, N, DIN, U, FD, SD = 32, 2048, 2, 64, 32, 64
NCORES = 8
BL = B // NCORES          # batches per core
NTW = 512                 # n-tile width
NT = N // NTW             # 4 n-tiles
NBW = 128                 # node-block width
NB = N // NBW             # 16 node blocks
FROWS = DIN + U           # 66


def _build_program():
    nc = bacc.Bacc("TRN2", debug=False, num_devices=NCORES)

    d = {}

    def din(name, shape, dt):
        d[name] = nc.dram_tensor(name, shape, dt, kind="ExternalInput").ap()

    din("xT", [BL, FROWS, N], BF16)
    din("hT", [BL, U, N], F32)
    din("a1T", [N, N], BF16)
    din("a2T", [N, N], BF16)
    din("fsT", [FD + SD, N], F32)
    din("wq", [FD, U], F32)
    din("wk", [FD, U], F32)
    din("ws1", [FD + SD, U], F32)
    din("bs1v", [U, 1], F32)
    din("ws2", [U, 1], F32)
    din("bs2v", [1, 1], F32)
    din("kkall", [FROWS, 3 * 2 * U], BF16)
    din("kk0", [FROWS, 2 * U], BF16)
    din("kcall", [FROWS, 3 * U], BF16)
    din("kc0", [FROWS, U], BF16)
    din("bru", [2 * U, 1], F32)
    din("bc2", [2 * U, 1], F32)
    out_h = nc.dram_tensor("out", [BL, U, N], F32, kind="ExternalOutput").ap()
    uscr = nc.dram_tensor("uscr", [BL, U, N], F32).ap()

    with tile.TileContext(nc) as tc:
        _emit(tc, d, out_h, uscr)
    nc.compile()
    return nc


def _emit(tc, d, out_h, uscr):
    nc = tc.nc
    ctx = contextlib.ExitStack()
    const = ctx.enter_context(tc.tile_pool(name="const", bufs=1))
    persist = ctx.enter_context(tc.tile_pool(name="persist", bufs=1))
    adjp = ctx.enter_context(tc.tile_pool(name="adjp", bufs=4))
    etp = ctx.enter_context(tc.tile_pool(name="etp", bufs=1))
    ypool = ctx.enter_context(tc.tile_pool(name="ypool", bufs=1))
    stage = ctx.enter_context(tc.tile_pool(name="stage", bufs=2))
    p3p = ctx.enter_context(tc.tile_pool(name="p3p", bufs=2))
    psacc = ctx.enter_context(tc.tile_pool(name="psacc", bufs=5, space="PSUM"))
    psscr = ctx.enter_context(tc.tile_pool(name="psscr", bufs=3, space="PSUM"))

    # ---- constants / weights in SBUF ----
    def cload(name, shape=None, dt=None):
        ap = d[name]
        t = const.tile(list(ap.shape) if shape is None else shape,
                       ap.dtype if dt is None else dt, name=f"c_{name}")
        nc.sync.dma_start(out=t, in_=ap)
        return t

    fsT = const.tile([FD + SD, N], F32, name="c_fsT")
    nc.sync.dma_start(out=fsT[0:FD, :], in_=d["fsT"][0:FD, :])
    wq = cload("wq")
    wk = cload("wk")
    nc.sync.dma_start(out=fsT[FD:, :], in_=d["fsT"][FD:, :])
    ws1 = cload("ws1")
    bs1v = cload("bs1v")
    ws2 = cload("ws2")
    bs2v = cload("bs2v")

    kkall = cload("kkall")
    kcall = cload("kcall")
    kk0 = cload("kk0")
    kc0 = cload("kc0")
    bru = cload("bru")
    bc2 = cload("bc2")

    # ---- persistent activations ----
    xT = [persist.tile([FROWS, N], BF16, name=f"xT{b}", tag=f"xT{b}")
          for b in range(BL)]
    for b in range(BL):
        nc.sync.dma_start(out=xT[b], in_=d["xT"][b])

    ones_col = const.tile([128, 1], BF16, name="ones_col")
    nc.vector.memset(ones_col, 1.0)
    ones_row = const.tile([1, 128], F32, name="ones_row")
    nc.vector.memset(ones_row, 1.0)

    QT = persist.tile([U, N], BF16, name="QT", tag="QT")
    KT = persist.tile([U, N], BF16, name="KT", tag="KT")
    s_row = persist.tile([1, N], F32, name="s_row", tag="s_row")
    rdbc = [persist.tile([128, NTW], BF16, name=f"rdbc{t}", tag=f"rdbc{t}")
            for t in range(NT)]

    # ---- prelude: Q^T, K^T, s ----
    for t in range(NT):
        sl = slice(t * NTW, (t + 1) * NTW)
        pq = psscr.tile([U, NTW], F32, name="pq", tag="scr")
        nc.tensor.matmul(pq, wq, fsT[0:FD, sl], start=True, stop=True)
        nc.scalar.activation(QT[:, sl], pq, AF.Relu)
        pk = psscr.tile([U, NTW], F32, name="pk", tag="scr")
        nc.tensor.matmul(pk, wk, fsT[0:FD, sl], start=True, stop=True)
        nc.scalar.activation(KT[:, sl], pk, AF.Relu)
        ps1 = psscr.tile([U, NTW], F32, name="ps1", tag="scr")
        nc.tensor.matmul(ps1, ws1, fsT[:, sl], start=True, stop=True)
        s1t = stage.tile([U, NTW], F32, name="s1t", tag="s1t")
        nc.scalar.activation(s1t, ps1, AF.Relu, bias=bs1v)
        ps2 = psscr.tile([1, NTW], F32, name="ps2", tag="scr")
        nc.tensor.matmul(ps2, ws2, s1t, start=True, stop=True)
        nc.scalar.activation(s_row[:, sl], ps2, AF.Relu, bias=bs2v)

    # ---- phase-1 Y tiles: Y[m,b] = X_b @ [k_r[m]|k_u[m]], all m in one
    # MM: stored as [128, NB, 3, 128]: [node%128, node//128, m, u']
    y = [ypool.tile([NBW, NB, 3, 2 * U], BF16, name=f"y_{b}", tag=f"y{b}")
         for b in range(BL)]
    for b in range(BL):
        for j in range(NB):
            nsl = slice(j * NBW, (j + 1) * NBW)
            py = psscr.tile([NBW, 3 * 2 * U], F32, name="py", tag="scr")
            nc.tensor.matmul(py, xT[b][:, nsl], kkall, start=True, stop=True)
            nc.vector.tensor_copy(
                y[b][:, j, :, :],
                py.rearrange("p (m u) -> p m u", m=3))

    def e_thunks(t, et):
        """Thunks generating e^T[:, t] = exp(K Q^T / 8) into et, one
        node-block per call — interleaved into adjacency groups so the
        ACT exp evacuations hide under PE matmul streaming."""
        sl = slice(t * NTW, (t + 1) * NTW)

        def mk(j):
            def f():
                pe = psscr.tile([NBW, NTW], F32, name="pe", tag="scr")
                nc.tensor.matmul(pe, KT[:, j * NBW:(j + 1) * NBW], QT[:, sl],
                                 start=True, stop=True)
                nc.scalar.activation(et[:, j, :], pe, AF.Exp, scale=0.125)
            return f
        return [mk(j) for j in range(NB)]

    def d_thunks(et, pd):
        def mk(j):
            def f():
                nc.tensor.matmul(pd, ones_col, et[:, j, :],
                                 start=(j == 0), stop=(j == NB - 1))
            return f
        return [mk(j) for j in range(NB)]

    def interleave(main, extra, ratio=2):
        """Emit `ratio` thunks from main per one from extra."""
        mi = ei = 0
        while mi < len(main) or ei < len(extra):
            for _ in range(ratio):
                if mi < len(main):
                    main[mi](); mi += 1
            if ei < len(extra):
                extra[ei](); ei += 1

    def adjslice(name, t):
        sl = d[name][:, t * NTW:(t + 1) * NTW]
        a = adjp.tile([NBW, NB, NTW], BF16, name=f"sl_{name}", tag="adj")
        nc.sync.dma_start(out=a, in_=sl.rearrange("(j p) w -> p j w", p=NBW))
        return a

    # =================== phase 1: r & u gates ===================
    def a_thunks1(b, sl, a1, a2, pa):
        th = [lambda: nc.tensor.matmul(pa, kk0, xT[b][:, sl],
                                       start=True, stop=False)]
        for m, asl in ((0, a1), (1, a2)):
            for j in range(NB):
                def f(m=m, asl=asl, j=j):
                    nc.tensor.matmul(pa, y[b][:, j, m, :], asl[:, j, :],
                                     start=False,
                                     stop=(m == 1 and j == NB - 1))
                th.append(f)
        return th

    def agroup1(b, sl, a1, a2, extra=(), ratio=2):
        pa = psacc.tile([128, NTW], F32, name="pa", tag="acc")
        interleave(a_thunks1(b, sl, a1, a2, pa), list(extra), ratio=ratio)
        return pa

    def bgroup1(b, t, sl, et, pa):
        pb = psacc.tile([128, NTW], F32, name="pb", tag="acc")
        for j in range(NB):
            nc.tensor.matmul(pb, y[b][:, j, 2, :], et[:, j, :],
                             start=(j == 0), stop=(j == NB - 1))
        tmp = stage.tile([128, NTW], F32, name="tmp", tag="tmp")
        nc.vector.tensor_mul(tmp, pb, rdbc[t])
        ssum = stage.tile([128, NTW], F32, name="ssum", tag="ssum")
        nc.vector.tensor_add(ssum, pa, tmp)
        sig = stage.tile([128, NTW], F32, name="sig", tag="sig")
        nc.scalar.activation(sig, ssum, AF.Sigmoid, scale=0.25, bias=bru)
        # rh -> x_cat_c rows 0:64 in place; u -> DRAM scratch
        nc.vector.tensor_mul(xT[b][0:U, sl], sig[0:U, :], xT[b][0:U, sl])
        nc.sync.dma_start(out=uscr[b][:, sl], in_=sig[U:128, :])

    preload = (adjslice("a1T", 0), adjslice("a2T", 0))

    def ycgen_thunks(yct, p):
        th = []
        for half in range(2):
            b = 2 * p + half
            usl = slice(half * U, (half + 1) * U)
            for j in range(NB):
                def f(b=b, usl=usl, j=j, yct=yct):
                    nsl = slice(j * NBW, (j + 1) * NBW)
                    pyc = psscr.tile([NBW, 3 * U], F32, name="pyc", tag="scr")
                    nc.tensor.matmul(pyc, xT[b][:, nsl], kcall,
                                     start=True, stop=True)
                    nc.vector.tensor_copy(
                        yct[:, j, :, usl],
                        pyc.rearrange("p (m u) -> p m u", m=3))
                th.append(f)
        return th

    yc = [None, None]

    for t in range(NT):
        sl = slice(t * NTW, (t + 1) * NTW)
        if t == 0:
            a1, a2 = preload
        else:
            a1 = adjslice("a1T", t)
            a2 = adjslice("a2T", t)
        et = etp.tile([NBW, NB, NTW], BF16, name="et", tag="et")
        pd = psscr.tile([1, NTW], F32, name="pd", tag="scr")
        pa0 = agroup1(0, sl, a1, a2)
        pa1 = agroup1(1, sl, a1, a2, e_thunks(t, et))
        pa2 = agroup1(2, sl, a1, a2, d_thunks(et, pd))

        # d[n] = s[n] + colsum(e^T)[n]; rdbc[t][p, n] = 1 / d[n]
        dsb = stage.tile([1, NTW], F32, name="dsb", tag="dsb")
        nc.vector.tensor_add(dsb, pd, s_row[:, sl])
        rds = stage.tile([1, NTW], F32, name="rds", tag="dsb")
        nc.vector.reciprocal(rds, dsb)
        pr = psscr.tile([128, NTW], F32, name="pr", tag="scr")
        nc.tensor.matmul(pr, ones_row, rds, start=True, stop=True)
        nc.scalar.activation(rdbc[t], pr, AF.Copy)

        bgroup1(0, t, sl, et, pa0)
        bgroup1(1, t, sl, et, pa1)
        if t == NT - 1:
            yc[0] = ypool.tile([NBW, NB, 3, 2 * U], BF16, name="yc_0",
                               tag="y0")
            pa3 = agroup1(3, sl, a1, a2, ycgen_thunks(yc[0], 0), ratio=1)
        else:
            pa3 = agroup1(3, sl, a1, a2)
        bgroup1(2, t, sl, et, pa2)
        bgroup1(3, t, sl, et, pa3)

    # =================== phase 2+3: c gate & h_new ===================
    # Yc[pair] = [Xc_b0 @ kc[m] | Xc_b1 @ kc[m]] packed [128, NB, 3, 128]
    # (yc[0] was already generated inside phase-1's final A-group)
    yc[1] = ypool.tile([NBW, NB, 3, 2 * U], BF16, name="yc_1", tag="y1")
    for f in ycgen_thunks(yc[1], 1):
        f()

    for t in range(NT):
        sl = slice(t * NTW, (t + 1) * NTW)
        a1 = adjslice("a1T", t)
        a2 = adjslice("a2T", t)
        et = etp.tile([NBW, NB, NTW], BF16, name="et2", tag="et")
        pas = []
        for p in range(BL // 2):
            b0, b1 = 2 * p, 2 * p + 1
            pa = psacc.tile([128, NTW], F32, name="pa2", tag="acc")
            th = [lambda pa=pa, p=p: nc.tensor.matmul(
                      pa, yc[p][:, 0, 0, :], a1[:, 0, :], start=True, stop=False),
                  lambda pa=pa, b0=b0: nc.tensor.matmul(
                      pa[0:U, :], kc0, xT[b0][:, sl], start=False, stop=False),
                  lambda pa=pa, b1=b1: nc.tensor.matmul(
                      pa[U:128, :], kc0, xT[b1][:, sl], start=False, stop=False)]
            for m, asl in ((0, a1), (1, a2)):
                for j in range(NB):
                    if m == 0 and j == 0:
                        continue
                    def f(pa=pa, p=p, m=m, asl=asl, j=j):
                        nc.tensor.matmul(pa, yc[p][:, j, m, :], asl[:, j, :],
                                         start=False,
                                         stop=(m == 1 and j == NB - 1))
                    th.append(f)
            interleave(th, e_thunks(t, et) if p == 0 else [])
            pas.append(pa)
        for p in range(BL // 2):
            b0, b1 = 2 * p, 2 * p + 1
            pa = pas[p]
            # prefetch h and u for the tail chain
            hp = p3p.tile([128, NTW], F32, name="hp", tag="hp")
            up = p3p.tile([128, NTW], F32, name="up", tag="up")
            for half, b in ((0, b0), (1, b1)):
                psl = slice(half * U, (half + 1) * U)
                nc.sync.dma_start(out=hp[psl, :], in_=d["hT"][b][:, sl])
                nc.sync.dma_start(out=up[psl, :], in_=uscr[b][:, sl])
            pb = psacc.tile([128, NTW], F32, name="pb2", tag="acc")
            for j in range(NB):
                nc.tensor.matmul(pb, yc[p][:, j, 2, :], et[:, j, :],
                                 start=(j == 0), stop=(j == NB - 1))
            tmp = stage.tile([128, NTW], F32, name="tmp2", tag="tmp")
            ct = stage.tile([128, NTW], F32, name="ct", tag="sig")
            t1 = p3p.tile([128, NTW], F32, name="t1", tag="t1")
            # run the gate + elementwise chain in column halves so the
            # DVE / ACT / DMA stages pipeline instead of serializing
            for c0 in range(0, NTW, NTW // 2):
                cs = slice(c0, c0 + NTW // 2)
                nc.vector.tensor_mul(tmp[:, cs], pb[:, cs], rdbc[t][:, cs])
                nc.vector.tensor_add(tmp[:, cs], pa[:, cs], tmp[:, cs])
                nc.scalar.activation(ct[:, cs], tmp[:, cs], AF.Tanh,
                                     scale=0.25, bias=bc2)
                nc.vector.tensor_sub(t1[:, cs], hp[:, cs], ct[:, cs])
                nc.vector.tensor_mul(t1[:, cs], up[:, cs], t1[:, cs])
                nc.vector.tensor_add(t1[:, cs], t1[:, cs], ct[:, cs])
                for half, b in ((0, b0), (1, b1)):
                    psl = slice(half * U, (half + 1) * U)
                    nc.sync.dma_start(
                        out=out_h[b][:, t * NTW + c0:t * NTW + c0 + NTW // 2],
                        in_=t1[psl, cs])

    ctx.close()


_CACHE = {}


def _get_program():
    if "nc" not in _CACHE:
        _CACHE["nc"] = _build_program()
    return _CACHE["nc"]


def _prep_inputs(inputs, h_prev, adj1, adj2, feat, SE, Wq, Wk, Ws1, bs1, Ws2,
                 bs2, r_kernel, r_bias, u_kernel, u_bias, c_kernel, c_bias):
    bf = ml_dtypes.bfloat16
    f32 = np.float32
    perm = list(range(DIN, FROWS)) + list(range(DIN))  # [h(64); inputs(2)]

    h3 = np.asarray(h_prev, f32).reshape(B, N, U)
    hT = np.ascontiguousarray(h3.transpose(0, 2, 1))            # [B, U, N]
    inT = np.asarray(inputs, f32).transpose(0, 2, 1)            # [B, DIN, N]
    xT = np.concatenate([hT, inT], axis=1).astype(bf)           # [B, 66, N]

    rk = np.asarray(r_kernel, f32)[:, perm, :]
    uk = np.asarray(u_kernel, f32)[:, perm, :]
    ck = np.asarray(c_kernel, f32)[:, perm, :]
    kkall = np.concatenate(
        [np.concatenate([rk[m], uk[m]], axis=1) for m in (1, 2, 3)],
        axis=1).astype(bf)                                      # [66, 384]
    kk0 = np.concatenate([rk[0], uk[0]], axis=1).astype(bf)     # [66, 128]
    kcall = np.concatenate([ck[1], ck[2], ck[3]], axis=1).astype(bf)
    kc0 = ck[0].astype(bf)

    shared = {
        "a1T": np.ascontiguousarray(np.asarray(adj1, f32).T).astype(bf),
        "a2T": np.ascontiguousarray(np.asarray(adj2, f32).T).astype(bf),
        "fsT": np.ascontiguousarray(
            np.concatenate([np.asarray(feat, f32).T, np.asarray(SE, f32).T],
                           axis=0)),
        "wq": np.asarray(Wq, f32),
        "wk": np.asarray(Wk, f32),
        "ws1": np.asarray(Ws1, f32),
        "bs1v": np.asarray(bs1, f32).reshape(U, 1),
        "ws2": np.asarray(Ws2, f32).reshape(U, 1),
        "bs2v": np.asarray(bs2, f32).reshape(1, 1),
        "kkall": kkall,
        "kk0": kk0,
        "kcall": kcall,
        "kc0": kc0,
        "bru": np.concatenate([np.asarray(r_bias, f32).mean(0),
                               np.asarray(u_bias, f32).mean(0)]).reshape(-1, 1),
        "bc2": np.tile(np.asarray(c_bias, f32).mean(0), 2).reshape(-1, 1),
    }
    in_maps = []
    for c in range(NCORES):
        bsl = slice(c * BL, (c + 1) * BL)
        m = dict(shared)
        m["xT"] = np.ascontiguousarray(xT[bsl])
        m["hT"] = np.ascontiguousarray(hT[bsl])
        in_maps.append(m)
    return in_maps


def kernel(**inputs):
    os.environ.setdefault("NEURON_RT_RESET_CORES", "1")
    nc = _get_program()
    in_maps = _prep_inputs(**inputs)
    res = None
    err = None
    for _ in range(2):
        try:
            res = run_bass_kernel_spmd(nc, in_maps, list(range(NCORES)))
            break
        except Exception as e:  # e.g. a wedged device; retry once
            err = e
    if res is None:
        raise err
    outs = []
    for c in range(NCORES):
        o = res.results[c]["out"]                     # [BL, U, N] f32
        outs.append(o.transpose(0, 2, 1).reshape(BL, N * U))
    return np.concatenate(outs, axis=0).astype(np.float32)



# revision 4
# speedup vs baseline: 1.6720x; 1.6720x over previous
"""MFGCGRU (graph-conv GRU cell) Trainium2 kernel.

Strategy: data-parallel over batch B=32 across 8 NeuronCores (4 batches
per core). All NxN supports replicated per core and resident in SBUF.

The dominant work — applying the three supports (adj1, adj2, e-attn) to
the per-gate projections Y_m = X @ k_m — runs as fp8(e4m3) matmuls in
DoubleRow perf mode: operands are packed [128, 2, F] so each PE pass
contracts 256 source nodes at half a cycle per output column.

Scaling bookkeeping (so fp8 operands sit in e4m3's sweet spot):
  - adjacencies are sent as 64*S^T            (C_ADJ = 64)
  - support kernels are folded with x16       (C_Y = 16, Y' = 16*X@k)
  - identity kernels are folded with x1024    (G = C_ADJ*C_Y)
  - the e-term normalizer is rdbc = 64/d, applied to the (x16-scaled)
    e-contribution PSUM, matching the x1024 of the adj/identity terms
  - gate activations then use ACT scale = 0.25/1024 (the /M fold)

The attention support is built unnormalized as e^T = exp(K Q^T / 8)
from fp8 Q/K packed [32, 2, N] (u = plane*32 + p), stored fp8 and kept
resident for both the r/u and the c passes; its row-normalizer is
applied to the e-contribution via a second PSUM accumulator.
"""

import contextlib
import os

import numpy as np
import ml_dtypes

import concourse.bass as bass
import concourse.bacc as bacc
import concourse.tile as tile
from concourse import mybir
from concourse.bass_utils import run_bass_kernel_spmd

F32 = mybir.dt.float32
BF16 = mybir.dt.bfloat16
F8 = mybir.dt.float8e4
AF = mybir.ActivationFunctionType
DR = mybir.MatmulPerfMode.DoubleRow

B, N, DIN, U, FD, SD = 32, 2048, 2, 64, 32, 64
NCORES = 8
BL = B // NCORES          # batches per core
NTW = 512                 # n-tile width
NT = N // NTW             # 4 n-tiles
NBW = 128                 # node-block width
NB = N // NBW             # 16 node blocks
NSB = NB // 2             # 8 node super-blocks (256 nodes, DoubleRow)
FROWS = DIN + U           # 66

C_ADJ = 64.0              # host scale on S^T before e4m3 cast
C_Y = 16.0                # host scale folded into support kernels
G = C_ADJ * C_Y           # net scale of the PSUM accumulators
SCL = 0.25 / G            # ACT scale for the gate activations (incl /M)


def _build_program():
    nc = bacc.Bacc("TRN2", debug=False, num_devices=NCORES)

    d = {}

    def din(name, shape, dt):
        d[name] = nc.dram_tensor(name, shape, dt, kind="ExternalInput").ap()

    din("xT", [BL, FROWS, N], BF16)
    din("hT", [BL, U, N], F32)
    din("a1T", [N, N], F8)
    din("a2T", [N, N], F8)
    din("fsT", [FD + SD, N], BF16)
    din("wq", [FD, U], BF16)
    din("wk", [FD, U], BF16)
    din("ws1", [FD + SD, U], BF16)
    din("bs1v", [U, 1], F32)
    din("ws2", [U, 1], BF16)
    din("bs2v", [1, 1], F32)
    din("kkall", [FROWS, 3 * 2 * U], BF16)
    din("kk0", [FROWS, 2 * U], BF16)
    din("kcall", [FROWS, 3 * U], BF16)
    din("kc0", [FROWS, U], BF16)
    din("bru", [2 * U, 1], F32)
    din("bc2", [2 * U, 1], F32)
    out_h = nc.dram_tensor("out", [BL, U, N], F32, kind="ExternalOutput").ap()
    uscr = nc.dram_tensor("uscr", [BL, U, N], F32).ap()

    with tile.TileContext(nc) as tc:
        _emit(tc, d, out_h, uscr)
    nc.compile()
    return nc


def _emit(tc, d, out_h, uscr):
    nc = tc.nc
    ctx = contextlib.ExitStack()
    const = ctx.enter_context(tc.tile_pool(name="const", bufs=1))
    persist = ctx.enter_context(tc.tile_pool(name="persist", bufs=1))
    ypool = ctx.enter_context(tc.tile_pool(name="ypool", bufs=1))
    stage = ctx.enter_context(tc.tile_pool(name="stage", bufs=2))
    p3p = ctx.enter_context(tc.tile_pool(name="p3p", bufs=2))
    psacc = ctx.enter_context(tc.tile_pool(name="psacc", bufs=5, space="PSUM"))
    psscr = ctx.enter_context(tc.tile_pool(name="psscr", bufs=3, space="PSUM"))

    # ---- constants / weights in SBUF ----
    def cload(name, shape=None, dt=None):
        ap = d[name]
        t = const.tile(list(ap.shape) if shape is None else shape,
                       ap.dtype if dt is None else dt, name=f"c_{name}")
        nc.sync.dma_start(out=t, in_=ap)
        return t

    fsT = const.tile([FD + SD, N], BF16, name="c_fsT")
    nc.sync.dma_start(out=fsT[0:FD, :], in_=d["fsT"][0:FD, :])
    wq = cload("wq")
    wk = cload("wk")
    nc.sync.dma_start(out=fsT[FD:, :], in_=d["fsT"][FD:, :])
    ws1 = cload("ws1")
    bs1v = cload("bs1v")
    ws2 = cload("ws2")
    bs2v = cload("bs2v")

    kkall = cload("kkall")
    kcall = cload("kcall")
    kk0 = cload("kk0")
    kc0 = cload("kc0")
    bru = cload("bru")
    bc2 = cload("bc2")

    # ---- resident adjacency slices (fp8, reused by both phases) ----
    a1t, a2t = [], []
    for t in range(NT):
        sl = slice(t * NTW, (t + 1) * NTW)
        for name, lst in (("a1T", a1t), ("a2T", a2t)):
            a = persist.tile([NBW, NB, NTW], F8, name=f"{name}_{t}",
                             tag=f"{name}_{t}")
            nc.sync.dma_start(
                out=a, in_=d[name][:, sl].rearrange("(j p) w -> p j w", p=NBW))
            lst.append(a)

    # ---- persistent activations ----
    xT = [persist.tile([FROWS, N], BF16, name=f"xT{b}", tag=f"xT{b}")
          for b in range(BL)]
    for b in range(BL):
        nc.sync.dma_start(out=xT[b], in_=d["xT"][b])

    # k-plane stride of a DoubleRow LDWEIGHTS AP must be 16-byte aligned,
    # so pad the ones column out to 16 bytes per plane
    ones8f = const.tile([NBW, 2, 16], F8, name="ones8")
    nc.vector.memset(ones8f, 1.0)
    ones8 = ones8f[:, :, 0:1]
    ones_row = const.tile([1, NBW], BF16, name="ones_row")
    nc.vector.memset(ones_row, C_ADJ)

    QT = persist.tile([U // 2, 2, N], F8, name="QT", tag="QT")
    KT = persist.tile([U // 2, 2, N], F8, name="KT", tag="KT")
    s_row = persist.tile([1, N], BF16, name="s_row", tag="s_row")
    rdbc = [persist.tile([NBW, NTW], BF16, name=f"rdbc{t}", tag=f"rdbc{t}")
            for t in range(NT)]
    ets = [persist.tile([NBW, NB, NTW], F8, name=f"et{t}", tag=f"et{t}")
           for t in range(NT)]

    # ---- prelude: Q^T, K^T (fp8 split-u packing), s ----
    for t in range(NT):
        sl = slice(t * NTW, (t + 1) * NTW)
        for w, qt in ((wq, QT), (wk, KT)):
            for half in range(2):
                pq = psscr.tile([U // 2, NTW], F32, name="pq", tag="scr")
                nc.tensor.matmul(pq, w[:, half * 32:(half + 1) * 32],
                                 fsT[0:FD, sl], start=True, stop=True)
                nc.scalar.activation(qt[:, half, sl], pq, AF.Relu)
        ps1 = psscr.tile([U, NTW], F32, name="ps1", tag="scr")
        nc.tensor.matmul(ps1, ws1, fsT[:, sl], start=True, stop=True)
        s1t = stage.tile([U, NTW], BF16, name="s1t", tag="s1t")
        nc.scalar.activation(s1t, ps1, AF.Relu, bias=bs1v)
        ps2 = psscr.tile([1, NTW], F32, name="ps2", tag="scr")
        nc.tensor.matmul(ps2, ws2, s1t, start=True, stop=True)
        nc.scalar.activation(s_row[:, sl], ps2, AF.Relu, bias=bs2v)

    # ---- Y tiles: Y[m,b] = C_Y * X_b @ [k_r[m]|k_u[m]], stored fp8
    # [128, NB, 3, 128]: [node%128, node//128, m, u'] ----
    y = [ypool.tile([NBW, NB, 3, 2 * U], F8, name=f"y_{b}", tag=f"y{b}")
         for b in range(BL)]

    _eng = [0]

    def evac(out_ap, in_ap):
        """PSUM evacuation, alternating DVE / ACT to balance load."""
        _eng[0] ^= 1
        if _eng[0]:
            nc.vector.tensor_copy(out_ap, in_ap)
        else:
            nc.scalar.activation(out_ap, in_ap, AF.Copy)

    def ygen_thunks(b):
        def mk(j):
            def f():
                nsl = slice(j * NBW, (j + 1) * NBW)
                py = psscr.tile([NBW, 3 * 2 * U], F32, name="py", tag="scr")
                nc.tensor.matmul(py, xT[b][:, nsl], kkall, start=True,
                                 stop=True)
                evac(y[b][:, j, :, :], py.rearrange("p (m u) -> p m u", m=3))
            return f
        return [mk(j) for j in range(NB)]

    for b in range(2):
        for f in ygen_thunks(b):
            f()

    def e_thunks(t, et):
        """e^T[:, t] = exp(K Q^T / 8) into et (fp8), one node-block per
        call."""
        sl = slice(t * NTW, (t + 1) * NTW)

        def mk(j):
            def f():
                pe = psscr.tile([NBW, NTW], F32, name="pe", tag="scr")
                nc.tensor.matmul(pe, KT[:, :, j * NBW:(j + 1) * NBW],
                                 QT[:, :, sl], start=True, stop=True,
                                 perf_mode=DR)
                nc.scalar.activation(et[:, j, :], pe, AF.Exp, scale=0.125)
            return f
        return [mk(j) for j in range(NB)]

    def emit_dsum(et, pd):
        for j in range(NSB):
            nc.tensor.matmul(pd, ones8, et[:, 2 * j:2 * j + 2, :],
                             start=(j == 0), stop=(j == NSB - 1),
                             perf_mode=DR)

    def interleave(main, extra, ratio=2):
        """Emit `ratio` thunks from main per one from extra."""
        mi = ei = 0
        while mi < len(main) or ei < len(extra):
            for _ in range(ratio):
                if mi < len(main):
                    main[mi](); mi += 1
            if ei < len(extra):
                extra[ei](); ei += 1

    # =================== phase 1: r & u gates ===================
    def a_thunks1(b, t, sl, pa):
        th = [lambda: nc.tensor.matmul(pa, kk0, xT[b][:, sl],
                                       start=True, stop=False)]
        for m, asl in ((0, a1t[t]), (1, a2t[t])):
            for j in range(NSB):
                def f(m=m, asl=asl, j=j):
                    nc.tensor.matmul(pa, y[b][:, 2 * j:2 * j + 2, m, :],
                                     asl[:, 2 * j:2 * j + 2, :],
                                     start=False,
                                     stop=(m == 1 and j == NSB - 1),
                                     perf_mode=DR)
                th.append(f)
        return th

    def agroup1(b, t, sl, extra=(), ratio=2):
        pa = psacc.tile([128, NTW], F32, name="pa", tag="acc")
        interleave(a_thunks1(b, t, sl, pa), list(extra), ratio=ratio)
        return pa

    def bgroup1(b, t, sl, et, pa):
        pb = psacc.tile([128, NTW], F32, name="pb", tag="acc")
        for j in range(NSB):
            nc.tensor.matmul(pb, y[b][:, 2 * j:2 * j + 2, 2, :],
                             et[:, 2 * j:2 * j + 2, :],
                             start=(j == 0), stop=(j == NSB - 1),
                             perf_mode=DR)
        tmp = stage.tile([128, NTW], F32, name="tmp", tag="tmp")
        nc.vector.tensor_mul(tmp, pb, rdbc[t])
        ssum = stage.tile([128, NTW], F32, name="ssum", tag="ssum")
        nc.vector.tensor_add(ssum, pa, tmp)
        sig = stage.tile([128, NTW], F32, name="sig", tag="sig")
        nc.scalar.activation(sig, ssum, AF.Sigmoid, scale=SCL, bias=bru)
        # rh -> x_cat_c rows 0:64 in place; u -> DRAM scratch
        nc.gpsimd.tensor_mul(xT[b][0:U, sl], sig[0:U, :], xT[b][0:U, sl])
        nc.scalar.dma_start(out=uscr[b][:, sl], in_=sig[U:128, :])

    def ycgen_thunks(yct, p):
        th = []
        for half in range(2):
            b = 2 * p + half
            usl = slice(half * U, (half + 1) * U)
            for j in range(NB):
                def f(b=b, usl=usl, j=j, yct=yct):
                    nsl = slice(j * NBW, (j + 1) * NBW)
                    pyc = psscr.tile([NBW, 3 * U], F32, name="pyc", tag="scr")
                    nc.tensor.matmul(pyc, xT[b][:, nsl], kcall,
                                     start=True, stop=True)
                    evac(yct[:, j, :, usl],
                         pyc.rearrange("p (m u) -> p m u", m=3))
                th.append(f)
        return th

    yc = [None, None]

    for t in range(NT):
        sl = slice(t * NTW, (t + 1) * NTW)
        et = ets[t]
        eth = e_thunks(t, et)
        if t == 0:
            y2g, y3g = ygen_thunks(2), ygen_thunks(3)
            pa0 = agroup1(0, t, sl, eth[0:8], ratio=1)
            pa1 = agroup1(1, t, sl, eth[8:16] + y2g, ratio=1)
            pa2 = agroup1(2, t, sl, y3g, ratio=1)
        else:
            pa0 = agroup1(0, t, sl, eth[0:8], ratio=2)
            pa1 = agroup1(1, t, sl, eth[8:16], ratio=2)
            pa2 = agroup1(2, t, sl)
        pd = psscr.tile([1, NTW], F32, name="pd", tag="scr")
        emit_dsum(et, pd)

        # d[n] = s[n] + colsum(e^T)[n]; rdbc[t][p, n] = C_ADJ / d[n]
        dsb = stage.tile([1, NTW], F32, name="dsb", tag="dsb")
        nc.vector.tensor_add(dsb, pd, s_row[:, sl])
        rds = stage.tile([1, NTW], BF16, name="rds", tag="dsb")
        with nc.allow_low_precision(reason="1/d feeds an fp8-noise-dominated "
                                    "term; bf16 is plenty"):
            nc.vector.reciprocal(rds, dsb)
        pr = psscr.tile([128, NTW], F32, name="pr", tag="scr")
        nc.tensor.matmul(pr, ones_row, rds, start=True, stop=True)
        nc.scalar.activation(rdbc[t], pr, AF.Copy)

        bgroup1(0, t, sl, et, pa0)
        bgroup1(1, t, sl, et, pa1)
        if t == NT - 1:
            yc[0] = ypool.tile([NBW, NB, 3, 2 * U], F8, name="yc_0", tag="y0")
            pa3 = agroup1(3, t, sl, ycgen_thunks(yc[0], 0), ratio=1)
        else:
            pa3 = agroup1(3, t, sl)
        bgroup1(2, t, sl, et, pa2)
        bgroup1(3, t, sl, et, pa3)

    # =================== phase 2+3: c gate & h_new ===================
    # Yc[pair] = C_Y * [Xc_b0 @ kc[m] | Xc_b1 @ kc[m]], fp8
    # (yc[0] was already generated inside phase-1's final A-group)
    yc[1] = ypool.tile([NBW, NB, 3, 2 * U], F8, name="yc_1", tag="y1")
    for f in ycgen_thunks(yc[1], 1):
        f()

    for t in range(NT):
        sl = slice(t * NTW, (t + 1) * NTW)
        et = ets[t]
        a1, a2 = a1t[t], a2t[t]
        pas = []
        for p in range(BL // 2):
            b0, b1 = 2 * p, 2 * p + 1
            pa = psacc.tile([128, NTW], F32, name="pa2", tag="acc")
            th = [lambda pa=pa, p=p: nc.tensor.matmul(
                      pa, yc[p][:, 0:2, 0, :], a1[:, 0:2, :],
                      start=True, stop=False, perf_mode=DR),
                  lambda pa=pa, b0=b0: nc.tensor.matmul(
                      pa[0:U, :], kc0, xT[b0][:, sl], start=False, stop=False),
                  lambda pa=pa, b1=b1: nc.tensor.matmul(
                      pa[U:128, :], kc0, xT[b1][:, sl], start=False,
                      stop=False)]
            for m, asl in ((0, a1), (1, a2)):
                for j in range(NSB):
                    if m == 0 and j == 0:
                        continue
                    def f(pa=pa, p=p, m=m, asl=asl, j=j):
                        nc.tensor.matmul(pa, yc[p][:, 2 * j:2 * j + 2, m, :],
                                         asl[:, 2 * j:2 * j + 2, :],
                                         start=False,
                                         stop=(m == 1 and j == NSB - 1),
                                         perf_mode=DR)
                    th.append(f)
            for f in th:
                f()
            pas.append(pa)
        for p in range(BL // 2):
            b0, b1 = 2 * p, 2 * p + 1
            pa = pas[p]
            # prefetch h and u for the tail chain
            hp = p3p.tile([128, NTW], F32, name="hp", tag="hp")
            up = p3p.tile([128, NTW], F32, name="up", tag="up")
            for half, b in ((0, b0), (1, b1)):
                psl = slice(half * U, (half + 1) * U)
                nc.sync.dma_start(out=hp[psl, :], in_=d["hT"][b][:, sl])
                nc.sync.dma_start(out=up[psl, :], in_=uscr[b][:, sl])
            pb = psacc.tile([128, NTW], F32, name="pb2", tag="acc")
            for j in range(NSB):
                nc.tensor.matmul(pb, yc[p][:, 2 * j:2 * j + 2, 2, :],
                                 et[:, 2 * j:2 * j + 2, :],
                                 start=(j == 0), stop=(j == NSB - 1),
                                 perf_mode=DR)
            tmp = stage.tile([128, NTW], F32, name="tmp2", tag="tmp")
            nc.vector.tensor_mul(tmp, pb, rdbc[t])
            ssum = stage.tile([128, NTW], F32, name="ssum2", tag="ssum")
            nc.vector.tensor_add(ssum, pa, tmp)
            ct = stage.tile([128, NTW], F32, name="ct", tag="sig")
            nc.scalar.activation(ct, ssum, AF.Tanh, scale=SCL, bias=bc2)
            # h_new = c + u*(h - c), in place on hp (GpSimd; SBUF-only)
            nc.gpsimd.tensor_sub(hp, hp, ct)
            nc.gpsimd.tensor_mul(hp, up, hp)
            nc.gpsimd.tensor_add(hp, hp, ct)
            for half, b in ((0, b0), (1, b1)):
                psl = slice(half * U, (half + 1) * U)
                nc.scalar.dma_start(out=out_h[b][:, sl], in_=hp[psl, :])

    ctx.close()


_CACHE = {}


def _get_program():
    if "nc" not in _CACHE:
        _CACHE["nc"] = _build_program()
    return _CACHE["nc"]


def _prep_inputs(inputs, h_prev, adj1, adj2, feat, SE, Wq, Wk, Ws1, bs1, Ws2,
                 bs2, r_kernel, r_bias, u_kernel, u_bias, c_kernel, c_bias):
    bf = ml_dtypes.bfloat16
    f8 = ml_dtypes.float8_e4m3
    f32 = np.float32
    perm = list(range(DIN, FROWS)) + list(range(DIN))  # [h(64); inputs(2)]

    h3 = np.asarray(h_prev, f32).reshape(B, N, U)
    hT = np.ascontiguousarray(h3.transpose(0, 2, 1))            # [B, U, N]
    inT = np.asarray(inputs, f32).transpose(0, 2, 1)            # [B, DIN, N]
    xT = np.concatenate([hT, inT], axis=1).astype(bf)           # [B, 66, N]

    rk = np.asarray(r_kernel, f32)[:, perm, :]
    uk = np.asarray(u_kernel, f32)[:, perm, :]
    ck = np.asarray(c_kernel, f32)[:, perm, :]
    kkall = (np.concatenate(
        [np.concatenate([rk[m], uk[m]], axis=1) for m in (1, 2, 3)],
        axis=1) * C_Y).astype(bf)                               # [66, 384]
    kk0 = (np.concatenate([rk[0], uk[0]], axis=1) * G).astype(bf)
    kcall = (np.concatenate([ck[1], ck[2], ck[3]], axis=1) * C_Y).astype(bf)
    kc0 = (ck[0] * G).astype(bf)

    shared = {
        "a1T": np.ascontiguousarray(
            np.asarray(adj1, f32).T * C_ADJ).astype(f8),
        "a2T": np.ascontiguousarray(
            np.asarray(adj2, f32).T * C_ADJ).astype(f8),
        "fsT": np.ascontiguousarray(
            np.concatenate([np.asarray(feat, f32).T, np.asarray(SE, f32).T],
                           axis=0)).astype(bf),
        "wq": np.asarray(Wq, f32).astype(bf),
        "wk": np.asarray(Wk, f32).astype(bf),
        "ws1": np.asarray(Ws1, f32).astype(bf),
        "bs1v": np.asarray(bs1, f32).reshape(U, 1),
        "ws2": np.asarray(Ws2, f32).reshape(U, 1).astype(bf),
        "bs2v": np.asarray(bs2, f32).reshape(1, 1),
        "kkall": kkall,
        "kk0": kk0,
        "kcall": kcall,
        "kc0": kc0,
        "bru": np.concatenate([np.asarray(r_bias, f32).mean(0),
                               np.asarray(u_bias, f32).mean(0)]).reshape(-1, 1),
        "bc2": np.tile(np.asarray(c_bias, f32).mean(0), 2).reshape(-1, 1),
    }
    in_maps = []
    for c in range(NCORES):
        bsl = slice(c * BL, (c + 1) * BL)
        m = dict(shared)
        m["xT"] = np.ascontiguousarray(xT[bsl])
        m["hT"] = np.ascontiguousarray(hT[bsl])
        in_maps.append(m)
    return in_maps


def kernel(**inputs):
    os.environ.setdefault("NEURON_RT_RESET_CORES", "1")
    nc = _get_program()
    in_maps = _prep_inputs(**inputs)
    res = None
    err = None
    for _ in range(2):
        try:
            res = run_bass_kernel_spmd(nc, in_maps, list(range(NCORES)))
            break
        except Exception as e:  # e.g. a wedged device; retry once
            err = e
    if res is None:
        raise err
    outs = []
    for c in range(NCORES):
        o = res.results[c]["out"]                     # [BL, U, N] f32
        outs.append(o.transpose(0, 2, 1).reshape(BL, N * U))
    return np.concatenate(outs, axis=0).astype(np.float32)


# revision 67
# speedup vs baseline: 2.0941x; 1.2525x over previous
"""MFGCGRU (graph-conv GRU cell) Trainium2 kernel.

Strategy: data-parallel over batch B=32 across 8 NeuronCores (4 batches
per core). All NxN supports replicated per core and resident in SBUF.

The dominant work — applying the three supports (adj1, adj2, e-attn) to
the per-gate projections Y_m = X @ k_m — runs as fp8(e4m3) matmuls in
DoubleRow perf mode: operands are packed [128, 2, F] so each PE pass
contracts 256 source nodes at half a cycle per output column.

Scaling bookkeeping (so fp8 operands sit in e4m3's sweet spot):
  - adjacencies are sent as 64*S^T            (C_ADJ = 64)
  - support kernels are folded with x16       (C_Y = 16, Y' = 16*X@k)
  - identity kernels are folded with x1024    (G = C_ADJ*C_Y)
  - the e-term normalizer is rdbc = 64/d, applied to the (x16-scaled)
    e-contribution PSUM, matching the x1024 of the adj/identity terms
  - gate activations then use ACT scale = 0.25/1024 (the /M fold)

The attention support is built unnormalized as e^T = exp(K Q^T / 8)
from fp8 Q/K packed [32, 2, N] (u = plane*32 + p), stored fp8 and kept
resident for both the r/u and the c passes; its row-normalizer is
applied to the e-contribution via a second PSUM accumulator.
"""

import contextlib
import os

import numpy as np
import ml_dtypes

import concourse.bass as bass
import concourse.bacc as bacc
import concourse.tile as tile
from concourse import mybir
from concourse.bass_utils import run_bass_kernel_spmd

F32 = mybir.dt.float32
BF16 = mybir.dt.bfloat16
F8 = mybir.dt.float8e4
AF = mybir.ActivationFunctionType
DR = mybir.MatmulPerfMode.DoubleRow

B, N, DIN, U, FD, SD = 32, 2048, 2, 64, 32, 64
NCORES = 8
BL = B // NCORES          # batches per core
NTW = 512                 # n-tile width
NT = N // NTW             # 4 n-tiles
NBW = 128                 # node-block width
NB = N // NBW             # 16 node blocks
NSB = NB // 2             # 8 node super-blocks (256 nodes, DoubleRow)
FROWS = DIN + U           # 66

C_ADJ = 64.0              # host scale on S^T before e4m3 cast
C_Y = 16.0                # host scale folded into support kernels
G = C_ADJ * C_Y           # net scale of the PSUM accumulators
SCL = 0.25 / G            # ACT scale for the gate activations (incl /M)


def _build_program():
    nc = bacc.Bacc("TRN2", debug=False, num_devices=NCORES)

    d = {}

    def din(name, shape, dt):
        d[name] = nc.dram_tensor(name, shape, dt, kind="ExternalInput").ap()

    din("xT", [BL, FROWS, N], BF16)
    din("hT", [BL, U, N], F32)
    din("a1T", [N, N], F8)
    din("a2T", [N, N], F8)
    din("fsT", [FD + SD, N], BF16)
    # kernels are packed into one tensor: each dma_start holds the
    # (single, serialized) HWDGE device ~630ns regardless of size
    din("wqk", [FD, 2 * U], BF16)            # [wq | wk]
    din("ws1", [FD + SD, U], BF16)
    din("ws2", [U, 1], BF16)
    din("kblob", [FROWS, 768], BF16)         # [kkall|kk0|kcall|kc0]
    din("bs1v", [U, 1], F32)
    din("bruh", [2 * U, 1], F32)
    din("bc2", [2 * U, 1], F32)
    din("bs2v", [1, 1], F32)
    out_h = nc.dram_tensor("out", [BL, U, N], F32, kind="ExternalOutput").ap()
    uscr = nc.dram_tensor("uscr", [BL, U, N], F32).ap()

    with tile.TileContext(nc) as tc:
        _emit(tc, d, out_h, uscr)
    nc.compile()
    return nc


def _emit(tc, d, out_h, uscr):
    nc = tc.nc
    ctx = contextlib.ExitStack()
    const = ctx.enter_context(tc.tile_pool(name="const", bufs=1))
    persist = ctx.enter_context(tc.tile_pool(name="persist", bufs=1))
    ypool = ctx.enter_context(tc.tile_pool(name="ypool", bufs=1))
    stage = ctx.enter_context(tc.tile_pool(name="stage", bufs=3))
    # phase-2/3 pipeline tiles: 4 pair-tile tails in flight, so the h/u
    # prefetch and the tanh of tile t+1 never wait on tile t's drain
    p3p = ctx.enter_context(tc.tile_pool(name="p3p", bufs=4))
    ctp = ctx.enter_context(tc.tile_pool(name="ctp", bufs=4))
    dsp = ctx.enter_context(tc.tile_pool(name="dsp", bufs=1))
    psacc = ctx.enter_context(tc.tile_pool(name="psacc", bufs=5, space="PSUM"))
    psscr = ctx.enter_context(tc.tile_pool(name="psscr", bufs=3, space="PSUM"))

    # ---- constants / weights in SBUF ----
    def cload(name, shape=None, dt=None):
        ap = d[name]
        t = const.tile(list(ap.shape) if shape is None else shape,
                       ap.dtype if dt is None else dt, name=f"c_{name}")
        nc.sync.dma_start(out=t, in_=ap)
        return t

    # DMA order matters: the startup critical path is
    #   fsT/wqk (prelude) and xT/kblob (y-gen), then adj slice t=0.
    fsT = cload("fsT")
    wqk = cload("wqk")
    wq, wk = wqk[:, 0:U], wqk[:, U:2 * U]
    kblob = cload("kblob")
    kkall = kblob[:, 0:384]
    kk0 = kblob[:, 384:512]
    kcall = kblob[:, 512:704]
    kc0 = kblob[:, 704:768]

    xTall = persist.tile([FROWS, BL, N], BF16, name="xTall", tag="xTall")
    # two DMAs so y-gen for b=0/1 can start before b=2/3 lands
    nc.sync.dma_start(out=xTall[:, 0:2, :],
                      in_=d["xT"][0:2].rearrange("b f n -> f b n"))
    nc.sync.dma_start(out=xTall[:, 2:4, :],
                      in_=d["xT"][2:4].rearrange("b f n -> f b n"))
    xT = [xTall[:, b, :] for b in range(BL)]

    ws1 = cload("ws1")
    ws2 = cload("ws2")
    bs1v = cload("bs1v")
    bruh = cload("bruh")        # pre-halved host-side for the tanh trick
    bc2 = cload("bc2")
    bs2v = cload("bs2v")

    # ---- resident adjacency slices (fp8, reused by both phases) ----
    a1t = [persist.tile([NBW, NB, NTW], F8, name=f"a1T_{t}", tag=f"a1T_{t}")
           for t in range(NT)]
    a2t = [persist.tile([NBW, NB, NTW], F8, name=f"a2T_{t}", tag=f"a2T_{t}")
           for t in range(NT)]
    for t in range(NT):
        sl = slice(t * NTW, (t + 1) * NTW)
        for name, lst in (("a1T", a1t), ("a2T", a2t)):
            nc.sync.dma_start(
                out=lst[t],
                in_=d[name][:, sl].rearrange("(j p) w -> p j w", p=NBW))

    # k-plane stride of a DoubleRow LDWEIGHTS AP must be 16-byte aligned,
    # so pad the ones column out to 16 bytes per plane
    ones8f = const.tile([NBW, 2, 16], F8, name="ones8")
    nc.vector.memset(ones8f, 1.0)
    ones8 = ones8f[:, :, 0:1]
    ones_row = const.tile([1, NBW], BF16, name="ones_row")
    nc.vector.memset(ones_row, C_ADJ)

    QT = persist.tile([U // 2, 2, N], F8, name="QT", tag="QT")
    KT = persist.tile([U // 2, 2, N], F8, name="KT", tag="KT")
    s_row = persist.tile([1, N], BF16, name="s_row", tag="s_row")
    rdbc = [persist.tile([NBW, NTW], BF16, name=f"rdbc{t}", tag=f"rdbc{t}")
            for t in range(NT)]
    ets = [persist.tile([NBW, NB, NTW], F8, name=f"et{t}", tag=f"et{t}")
           for t in range(NT)]

    # ---- prelude: K^T first (e-gen of t=0 reads all of K), then Q^T
    # (fp8 split-u packing), then s (not needed until the first dchain) ----
    for w, qt in ((wk, KT), (wq, QT)):
        for t in range(NT):
            sl = slice(t * NTW, (t + 1) * NTW)
            for half in range(2):
                pq = psscr.tile([U // 2, NTW], F32, name="pq", tag="scr")
                nc.tensor.matmul(pq, w[:, half * 32:(half + 1) * 32],
                                 fsT[0:FD, sl], start=True, stop=True)
                nc.scalar.activation(qt[:, half, sl], pq, AF.Relu)
    for t in range(NT):
        sl = slice(t * NTW, (t + 1) * NTW)
        ps1 = psscr.tile([U, NTW], F32, name="ps1", tag="scr")
        nc.tensor.matmul(ps1, ws1, fsT[:, sl], start=True, stop=True)
        s1t = stage.tile([U, NTW], BF16, name="s1t", tag="s1t")
        nc.scalar.activation(s1t, ps1, AF.Relu, bias=bs1v)
        ps2 = psscr.tile([1, NTW], F32, name="ps2", tag="scr")
        nc.tensor.matmul(ps2, ws2, s1t, start=True, stop=True)
        nc.scalar.activation(s_row[:, sl], ps2, AF.Relu, bias=bs2v)

    # ---- Y tiles: Y[m,b] = C_Y * X_b @ [k_r[m]|k_u[m]], stored fp8
    # [128, NB, 3, 128]: [node%128, node//128, m, u'] ----
    y = [ypool.tile([NBW, NB, 3, 2 * U], F8, name=f"y_{b}", tag=f"y{b}")
         for b in range(BL)]

    _eng = [0]

    def evac(out_ap, in_ap, dve_share=1):
        """PSUM evacuation, rotating DVE / ACT to balance load
        (`dve_share` DVE copies per ACT copy; -1 = DVE only, 0 = ACT
        only)."""
        if dve_share < 0:
            nc.vector.tensor_copy(out_ap, in_ap)
            return
        _eng[0] = (_eng[0] + 1) % (dve_share + 1)
        if _eng[0]:
            nc.vector.tensor_copy(out_ap, in_ap)
        else:
            nc.scalar.activation(out_ap, in_ap, AF.Copy)

    def ygen_thunks(b, dve_share=1):
        def mk(j):
            def f():
                nsl = slice(j * NBW, (j + 1) * NBW)
                py = psscr.tile([NBW, 3 * 2 * U], F32, name="py", tag="scr")
                nc.tensor.matmul(py, xT[b][:, nsl], kkall, start=True,
                                 stop=True)
                evac(y[b][:, j, :, :], py.rearrange("p (m u) -> p m u", m=3),
                     dve_share=dve_share)
            return f
        return [mk(j) for j in range(NB)]

    # y-gen for b=0/1, with tile-0's e^T generation interleaved so et[0]
    # is ready before the t-loop needs it (every later tile's e^T is
    # generated one tile ahead inside agroups, keeping the ACT exp burst
    # off the critical path)
    def e_thunks(t, et):
        """e^T[:, t] = exp(K Q^T / 8) into et (fp8), one node-block per
        call."""
        sl = slice(t * NTW, (t + 1) * NTW)

        def mk(j):
            def f():
                pe = psscr.tile([NBW, NTW], F32, name="pe", tag="scr")
                nc.tensor.matmul(pe, KT[:, :, j * NBW:(j + 1) * NBW],
                                 QT[:, :, sl], start=True, stop=True,
                                 perf_mode=DR)
                nc.scalar.activation(et[:, j, :], pe, AF.Exp, scale=0.125)
            return f
        return [mk(j) for j in range(NB)]

    def emit_dsum(et, pd):
        for j in range(NSB):
            nc.tensor.matmul(pd, ones8, et[:, 2 * j:2 * j + 2, :],
                             start=(j == 0), stop=(j == NSB - 1),
                             perf_mode=DR)

    def interleave(main, extra, ratio=2):
        """Emit `ratio` thunks from main per one from extra."""
        mi = ei = 0
        while mi < len(main) or ei < len(extra):
            for _ in range(ratio):
                if mi < len(main):
                    main[mi](); mi += 1
            if ei < len(extra):
                extra[ei](); ei += 1

    # y-gen for b=0/1, with tile-0's e^T generation interleaved so et[0]
    # is ready before the t-loop needs it (every later tile's e^T is
    # generated one tile ahead inside agroups, keeping the ACT exp burst
    # off the critical path)
    interleave(ygen_thunks(0) + ygen_thunks(1), e_thunks(0, ets[0]),
               ratio=2)

    # =================== phase 1: r & u gates ===================
    def a_thunks1(b, t, sl, pa):
        th = [lambda: nc.tensor.matmul(pa, kk0, xT[b][:, sl],
                                       start=True, stop=False)]
        for m, asl in ((0, a1t[t]), (1, a2t[t])):
            for j in range(NSB):
                def f(m=m, asl=asl, j=j):
                    nc.tensor.matmul(pa, y[b][:, 2 * j:2 * j + 2, m, :],
                                     asl[:, 2 * j:2 * j + 2, :],
                                     start=False,
                                     stop=(m == 1 and j == NSB - 1),
                                     perf_mode=DR)
                th.append(f)
        return th

    def agroup1(b, t, sl, extra=(), ratio=2):
        pa = psacc.tile([128, NTW], F32, name="pa", tag="acc")
        interleave(a_thunks1(b, t, sl, pa), list(extra), ratio=ratio)
        return pa

    def pbgroup(b, t, et, ytile=None, m=2):
        """The e-support contraction (PE only, so it can be emitted away
        from the DVE/ACT finish chain)."""
        yt = y[b] if ytile is None else ytile
        pb = psacc.tile([128, NTW], F32, name="pb", tag="acc")
        for j in range(NSB):
            nc.tensor.matmul(pb, yt[:, 2 * j:2 * j + 2, m, :],
                             et[:, 2 * j:2 * j + 2, :],
                             start=(j == 0), stop=(j == NSB - 1),
                             perf_mode=DR)
        return pb

    def bfinish1(b, t, sl, pa, pb):
        tmp = stage.tile([128, NTW], F32, name="tmp", tag="tmp")
        nc.vector.tensor_mul(tmp, pb, rdbc[t])
        ssum = stage.tile([128, NTW], F32, name="ssum", tag="ssum")
        nc.vector.tensor_add(ssum, pa, tmp)
        # sigmoid(z) = 0.5*(1 + tanh(z/2)): Tanh lives in the same ACT
        # function table as Exp/Copy, so phase 1 never reloads the table
        # (Sigmoid does not; a reload is 1283ns). sig holds th = 2r-1 /
        # 2u-1; the affine fixups are folded downstream.
        sig = stage.tile([128, NTW], F32, name="sig", tag="sig")
        nc.scalar.activation(sig, ssum, AF.Tanh, scale=SCL / 2, bias=bruh)
        # rh' = (1+th)*h = 2*(r*h) -> rows 0:64 in place as th*h + h
        # (the 0.5 is folded into the state rows of kc0/kcall host-side);
        # two plain tensor-tensor ops on the otherwise idle GpSimd.
        # thu -> DRAM scratch.
        rhs = stage.tile([U, NTW], BF16, name="rhs", tag="s1t")
        nc.gpsimd.tensor_mul(rhs, sig[0:U, :], xT[b][0:U, sl])
        nc.gpsimd.tensor_add(xT[b][0:U, sl], rhs, xT[b][0:U, sl])
        nc.scalar.dma_start(out=uscr[b][:, sl], in_=sig[U:128, :])

    def ycgen_thunks(yct, p, dve_share=1):
        th = []
        for half in range(2):
            b = 2 * p + half
            usl = slice(half * U, (half + 1) * U)
            for j in range(NB):
                def f(b=b, usl=usl, j=j, yct=yct):
                    nsl = slice(j * NBW, (j + 1) * NBW)
                    pyc = psscr.tile([NBW, 3 * U], F32, name="pyc", tag="scr")
                    nc.tensor.matmul(pyc, xT[b][:, nsl], kcall,
                                     start=True, stop=True)
                    evac(yct[:, j, :, usl],
                         pyc.rearrange("p (m u) -> p m u", m=3),
                         dve_share=dve_share)
                th.append(f)
        return th

    yc = [None, None]

    # Per-tile emission order keeps PE fed continuously:
    #   ag0(+e) ag1(+e) ag2 pb0 d8 [dchain] ag3 fin0 pb1 fin1 pb2 fin2
    #   pb3 fin3
    # The PSUM "acc" live set never exceeds 5 (pa0-3 + one pb).
    for t in range(NT):
        sl = slice(t * NTW, (t + 1) * NTW)
        et = ets[t]
        # generate NEXT tile's e^T inside this tile's agroups (this
        # tile's was generated one tile ago)
        eth = e_thunks(t + 1, ets[t + 1]) if t + 1 < NT else []
        if t == 0:
            y2g, y3g = ygen_thunks(2), ygen_thunks(3)
            pa0 = agroup1(0, t, sl, eth[0:8], ratio=1)
            pa1 = agroup1(1, t, sl, eth[8:16] + y2g, ratio=1)
            pa2 = agroup1(2, t, sl, y3g, ratio=1)
        else:
            pa0 = agroup1(0, t, sl, eth[0:8], ratio=2)
            pa1 = agroup1(1, t, sl, eth[8:16], ratio=2)
            pa2 = agroup1(2, t, sl)
        pb0 = pbgroup(0, t, et)
        pd = psscr.tile([1, NTW], F32, name="pd", tag="scr")
        emit_dsum(et, pd)

        # d[n] = s[n] + colsum(e^T)[n]; rdbc[t][p, n] = C_ADJ / d[n]
        dsb = dsp.tile([1, NTW], F32, name="dsb", tag="dsb")
        nc.vector.tensor_add(dsb, pd, s_row[:, sl])
        rds = dsp.tile([1, NTW], BF16, name="rds", tag="rds")
        with nc.allow_low_precision(reason="1/d feeds an fp8-noise-dominated "
                                    "term; bf16 is plenty"):
            nc.vector.reciprocal(rds, dsb)
        pr = psscr.tile([128, NTW], F32, name="pr", tag="scr")
        nc.tensor.matmul(pr, ones_row, rds, start=True, stop=True)
        nc.vector.tensor_copy(rdbc[t], pr)

        pa3 = agroup1(3, t, sl)
        bfinish1(0, t, sl, pa0, pb0)
        if t == NT - 1:
            yc[0] = ypool.tile([NBW, NB, 3, 2 * U], F8, name="yc_0", tag="y0")
            pb1 = pbgroup(1, t, et)
            bfinish1(1, t, sl, pa1, pb1)
            # yc[0] gen needs the rh updates of b0/b1 (fin0/fin1) above;
            # interleave it with the remaining e-support contractions
            yc0g = ycgen_thunks(yc[0], 0)
            pb23 = []
            th23 = [lambda: pb23.append(pbgroup(2, t, et)),
                    lambda: pb23.append(pbgroup(3, t, et))]
            interleave(th23, yc0g, ratio=1)
            bfinish1(2, t, sl, pa2, pb23[0])
            bfinish1(3, t, sl, pa3, pb23[1])
        else:
            for b, pa in ((1, pa1), (2, pa2), (3, pa3)):
                pb = pbgroup(b, t, et)
                bfinish1(b, t, sl, pa, pb)

    # =================== phase 2+3: c gate & h_new ===================
    # Yc[pair] = C_Y * [Xc_b0 @ kc[m] | Xc_b1 @ kc[m]], fp8
    # (yc[0] was already generated inside phase-1's final A-group; yc[1]
    # is interleaved into phase-2 t=0's first A-group below, on the
    # mostly idle ACT)
    yc[1] = ypool.tile([NBW, NB, 3, 2 * U], F8, name="yc_1", tag="y1")
    yc1g = ycgen_thunks(yc[1], 1)

    for t in range(NT):
        sl = slice(t * NTW, (t + 1) * NTW)
        et = ets[t]
        a1, a2 = a1t[t], a2t[t]
        # prefetch h and u for the tail chains, one pair-merged DMA each
        hps, ups = [], []
        for p in range(BL // 2):
            hp = p3p.tile([128, NTW], F32, name="hp", tag="hp")
            up = p3p.tile([128, NTW], F32, name="up", tag="up")
            psl = slice(2 * p, 2 * p + 2)
            # the SBUF side stays a plain [128, 512] AP: its (partition,
            # col) iteration order already matches the DRAM side's
            # (b, u, w) — splitting the SBUF partition dim via rearrange
            # produces corrupt flat-offset strides
            nc.sync.dma_start(out=hp, in_=d["hT"][psl, :, sl])
            nc.sync.dma_start(out=up, in_=uscr[psl, :, sl])
            # uscr holds thu = 2u-1; recover u = 0.5*thu + 0.5 in place,
            # off the tail's critical path (the chain below then uses
            # only plain tensor-tensor ops)
            nc.scalar.activation(up, up, AF.Copy, bias=0.5, scale=0.5)
            hps.append(hp)
            ups.append(up)
        for p in range(BL // 2):
            b0, b1 = 2 * p, 2 * p + 1
            pa = psacc.tile([128, NTW], F32, name="pa2", tag="acc")
            th = [lambda pa=pa, p=p: nc.tensor.matmul(
                      pa, yc[p][:, 0:2, 0, :], a1[:, 0:2, :],
                      start=True, stop=False, perf_mode=DR),
                  lambda pa=pa, b0=b0: nc.tensor.matmul(
                      pa[0:U, :], kc0, xT[b0][:, sl], start=False, stop=False),
                  lambda pa=pa, b1=b1: nc.tensor.matmul(
                      pa[U:128, :], kc0, xT[b1][:, sl], start=False,
                      stop=False)]
            for m, asl in ((0, a1), (1, a2)):
                for j in range(NSB):
                    if m == 0 and j == 0:
                        continue
                    def f(pa=pa, p=p, m=m, asl=asl, j=j):
                        nc.tensor.matmul(pa, yc[p][:, 2 * j:2 * j + 2, m, :],
                                         asl[:, 2 * j:2 * j + 2, :],
                                         start=False,
                                         stop=(m == 1 and j == NSB - 1),
                                         perf_mode=DR)
                    th.append(f)
            if t == 0 and p == 0:
                interleave(th, yc1g, ratio=1)
            else:
                for f in th:
                    f()
            hp, up = hps[p], ups[p]
            pb = pbgroup(p, t, et, ytile=yc[p])
            tmp = stage.tile([128, NTW], F32, name="tmp2", tag="tmp")
            ct = ctp.tile([128, NTW], F32, name="ct", tag="ct")
            ssum = stage.tile([128, NTW], F32, name="ssum2", tag="ssum")
            # run the gate + h_new chain in column halves so the DVE / ACT
            # / GpSimd / DMA stages pipeline; the elementwise h_new chain
            # (h_new = c + u*(h - c), in place on hp) runs on the otherwise
            # idle GpSimd except on the final tile, where its latency is
            # exposed and DVE (2x f32 SBUF mode) drains faster.
            # the h_new chain stays OFF the DVE queue (in-order: a chain op
            # waiting on tanh would block the next pair's tmp/ssum) except
            # on the final tile, where DVE has nothing left queued and
            # drains faster than GpSimd
            veng = nc.vector if t == NT - 1 else nc.gpsimd
            for c0 in range(0, NTW, NTW // 2):
                cs = slice(c0, c0 + NTW // 2)
                nc.vector.tensor_mul(tmp[:, cs], pb[:, cs], rdbc[t][:, cs])
                nc.vector.tensor_add(ssum[:, cs], pa[:, cs], tmp[:, cs])
                nc.scalar.activation(ct[:, cs], ssum[:, cs], AF.Tanh,
                                     scale=SCL, bias=bc2)
                # h_new = c + u*(h - c), in place on hp
                veng.tensor_sub(hp[:, cs], hp[:, cs], ct[:, cs])
                veng.tensor_mul(hp[:, cs], up[:, cs], hp[:, cs])
                veng.tensor_add(hp[:, cs], hp[:, cs], ct[:, cs])
            # one pair-merged output DMA per (pair, tile)
            nc.scalar.dma_start(out=out_h[b0:b1 + 1, :, sl], in_=hp)

    ctx.close()


_CACHE = {}


def _get_program():
    if "nc" not in _CACHE:
        _CACHE["nc"] = _build_program()
    return _CACHE["nc"]


def _prep_inputs(inputs, h_prev, adj1, adj2, feat, SE, Wq, Wk, Ws1, bs1, Ws2,
                 bs2, r_kernel, r_bias, u_kernel, u_bias, c_kernel, c_bias):
    bf = ml_dtypes.bfloat16
    f8 = ml_dtypes.float8_e4m3
    f32 = np.float32
    perm = list(range(DIN, FROWS)) + list(range(DIN))  # [h(64); inputs(2)]

    h3 = np.asarray(h_prev, f32).reshape(B, N, U)
    hT = np.ascontiguousarray(h3.transpose(0, 2, 1))            # [B, U, N]
    inT = np.asarray(inputs, f32).transpose(0, 2, 1)            # [B, DIN, N]
    xT = np.concatenate([hT, inT], axis=1).astype(bf)           # [B, 66, N]

    rk = np.asarray(r_kernel, f32)[:, perm, :]
    uk = np.asarray(u_kernel, f32)[:, perm, :]
    ck = np.asarray(c_kernel, f32)[:, perm, :]
    # the device keeps rh' = 2*(r*h) in the state rows of x_cat_c (tanh
    # trick), so halve the state rows of the c kernels to compensate
    ck = ck.copy()
    ck[:, 0:U, :] *= 0.5
    kkall = (np.concatenate(
        [np.concatenate([rk[m], uk[m]], axis=1) for m in (1, 2, 3)],
        axis=1) * C_Y).astype(bf)                               # [66, 384]
    kk0 = (np.concatenate([rk[0], uk[0]], axis=1) * G).astype(bf)
    kcall = (np.concatenate([ck[1], ck[2], ck[3]], axis=1) * C_Y).astype(bf)
    kc0 = (ck[0] * G).astype(bf)

    shared = {
        "a1T": np.ascontiguousarray(
            np.asarray(adj1, f32).T * C_ADJ).astype(f8),
        "a2T": np.ascontiguousarray(
            np.asarray(adj2, f32).T * C_ADJ).astype(f8),
        "fsT": np.ascontiguousarray(
            np.concatenate([np.asarray(feat, f32).T, np.asarray(SE, f32).T],
                           axis=0)).astype(bf),
        "wqk": np.concatenate([np.asarray(Wq, f32), np.asarray(Wk, f32)],
                              axis=1).astype(bf),
        "ws1": np.asarray(Ws1, f32).astype(bf),
        "ws2": np.asarray(Ws2, f32).reshape(U, 1).astype(bf),
        "kblob": np.concatenate([kkall, kk0, kcall, kc0], axis=1),
        "bs1v": np.asarray(bs1, f32).reshape(U, 1),
        "bruh": 0.5 * np.concatenate(
            [np.asarray(r_bias, f32).mean(0),
             np.asarray(u_bias, f32).mean(0)]).reshape(-1, 1),
        "bc2": np.tile(np.asarray(c_bias, f32).mean(0), 2).reshape(-1, 1),
        "bs2v": np.asarray(bs2, f32).reshape(1, 1),
    }
    in_maps = []
    for c in range(NCORES):
        bsl = slice(c * BL, (c + 1) * BL)
        m = dict(shared)
        m["xT"] = np.ascontiguousarray(xT[bsl])
        m["hT"] = np.ascontiguousarray(hT[bsl])
        in_maps.append(m)
    return in_maps


def kernel(**inputs):
    os.environ.setdefault("NEURON_RT_RESET_CORES", "1")
    nc = _get_program()
    in_maps = _prep_inputs(**inputs)
    res = None
    err = None
    for _ in range(2):
        try:
            res = run_bass_kernel_spmd(nc, in_maps, list(range(NCORES)))
            break
        except Exception as e:  # e.g. a wedged device; retry once
            err = e
    if res is None:
        raise err
    outs = []
    for c in range(NCORES):
        o = res.results[c]["out"]                     # [BL, U, N] f32
        outs.append(o.transpose(0, 2, 1).reshape(BL, N * U))
    return np.concatenate(outs, axis=0).astype(np.float32)


# revision 74
# speedup vs baseline: 2.2794x; 1.0885x over previous
"""MFGCGRU (graph-conv GRU cell) Trainium2 kernel.

Strategy: data-parallel over batch B=32 across 8 NeuronCores (4 batches
per core). All NxN supports replicated per core and resident in SBUF.

The dominant work — applying the three supports (adj1, adj2, e-attn) to
the per-gate projections Y_m = X @ k_m — runs as fp8(e4m3) matmuls in
DoubleRow perf mode: operands are packed [128, 2, F] so each PE pass
contracts 256 source nodes at half a cycle per output column.

Scaling bookkeeping (so fp8 operands sit in e4m3's sweet spot):
  - adjacencies are sent as 64*S^T            (C_ADJ = 64)
  - support kernels are folded with x16       (C_Y = 16, Y' = 16*X@k)
  - identity kernels are folded with x1024    (G = C_ADJ*C_Y)
  - the e-term normalizer is rdbc = 64/d, applied to the (x16-scaled)
    e-contribution PSUM, matching the x1024 of the adj/identity terms
  - gate activations then use ACT scale = 0.25/1024 (the /M fold)

The attention support is built unnormalized as e^T = exp(K Q^T / 8)
from fp8 Q/K packed [32, 2, N] (u = plane*32 + p), stored fp8 and kept
resident for both the r/u and the c passes; its row-normalizer is
applied to the e-contribution via a second PSUM accumulator.
"""

import contextlib
import os

import numpy as np
import ml_dtypes

import concourse.bass as bass
import concourse.bacc as bacc
import concourse.tile as tile
from concourse import mybir
from concourse.bass_utils import run_bass_kernel_spmd

F32 = mybir.dt.float32
BF16 = mybir.dt.bfloat16
F8 = mybir.dt.float8e4
AF = mybir.ActivationFunctionType
DR = mybir.MatmulPerfMode.DoubleRow

B, N, DIN, U, FD, SD = 32, 2048, 2, 64, 32, 64
NCORES = 8
BL = B // NCORES          # batches per core
NTW = 512                 # n-tile width
NT = N // NTW             # 4 n-tiles
NBW = 128                 # node-block width
NB = N // NBW             # 16 node blocks
NSB = NB // 2             # 8 node super-blocks (256 nodes, DoubleRow)
FROWS = DIN + U           # 66

C_ADJ = 64.0              # host scale on S^T before e4m3 cast
C_Y = 16.0                # host scale folded into support kernels
G = C_ADJ * C_Y           # net scale of the PSUM accumulators
SCL = 0.25 / G            # ACT scale for the gate activations (incl /M)


def _build_program():
    nc = bacc.Bacc("TRN2", debug=False, num_devices=NCORES)

    d = {}

    def din(name, shape, dt):
        d[name] = nc.dram_tensor(name, shape, dt, kind="ExternalInput").ap()

    din("xT", [BL, FROWS, N], BF16)
    din("hT", [BL, U, N], F32)
    din("a1T", [N, N], F8)
    din("a2T", [N, N], F8)
    din("fsT", [FD + SD, N], BF16)
    # kernels are packed into one tensor: each dma_start holds the
    # (single, serialized) HWDGE device ~630ns regardless of size
    din("wqk", [FD, 2 * U], BF16)            # [wq | wk]
    din("ws1", [FD + SD, U], BF16)
    din("ws2", [U, 1], BF16)
    din("kblob", [FROWS, 768], BF16)         # [kkall|kk0|kcall|kc0]
    din("bs1v", [U, 1], F32)
    din("bruh", [2 * U, 1], F32)
    din("bc2", [2 * U, 1], F32)
    din("bs2v", [1, 1], F32)
    out_h = nc.dram_tensor("out", [BL, U, N], F32, kind="ExternalOutput").ap()
    uscr = nc.dram_tensor("uscr", [BL, U, N], F32).ap()

    with tile.TileContext(nc) as tc:
        _emit(tc, d, out_h, uscr)
    nc.compile()
    return nc


def _emit(tc, d, out_h, uscr):
    nc = tc.nc
    ctx = contextlib.ExitStack()
    const = ctx.enter_context(tc.tile_pool(name="const", bufs=1))
    persist = ctx.enter_context(tc.tile_pool(name="persist", bufs=1))
    ypool = ctx.enter_context(tc.tile_pool(name="ypool", bufs=1))
    stage = ctx.enter_context(tc.tile_pool(name="stage", bufs=3))
    # phase-2/3 pipeline tiles: 4 pair-tile tails in flight, so the h/u
    # prefetch and the tanh of tile t+1 never wait on tile t's drain
    p3p = ctx.enter_context(tc.tile_pool(name="p3p", bufs=4))
    ctp = ctx.enter_context(tc.tile_pool(name="ctp", bufs=4))
    dsp = ctx.enter_context(tc.tile_pool(name="dsp", bufs=1))
    psacc = ctx.enter_context(tc.tile_pool(name="psacc", bufs=4, space="PSUM"))
    psscr = ctx.enter_context(tc.tile_pool(name="psscr", bufs=4, space="PSUM"))

    # ---- constants / weights in SBUF ----
    def cload(name, shape=None, dt=None):
        ap = d[name]
        t = const.tile(list(ap.shape) if shape is None else shape,
                       ap.dtype if dt is None else dt, name=f"c_{name}")
        nc.sync.dma_start(out=t, in_=ap)
        return t

    # DMA order matters: the startup critical path is
    #   fsT/wqk (prelude) and xT/kblob (y-gen), then adj slice t=0.
    fsT = cload("fsT")
    wqk = cload("wqk")
    wq, wk = wqk[:, 0:U], wqk[:, U:2 * U]
    kblob = cload("kblob")
    kkall = kblob[:, 0:384]
    kk0 = kblob[:, 384:512]
    kcall = kblob[:, 512:704]
    kc0 = kblob[:, 704:768]

    xTall = persist.tile([FROWS, BL, N], BF16, name="xTall", tag="xTall")
    # two DMAs so y-gen for b=0/1 can start before b=2/3 lands
    nc.sync.dma_start(out=xTall[:, 0:2, :],
                      in_=d["xT"][0:2].rearrange("b f n -> f b n"))
    nc.sync.dma_start(out=xTall[:, 2:4, :],
                      in_=d["xT"][2:4].rearrange("b f n -> f b n"))
    xT = [xTall[:, b, :] for b in range(BL)]

    ws1 = cload("ws1")
    ws2 = cload("ws2")
    bs1v = cload("bs1v")
    bruh = cload("bruh")        # pre-halved host-side for the tanh trick
    bc2 = cload("bc2")
    bs2v = cload("bs2v")

    # ---- resident adjacency slices (fp8, reused by both phases) ----
    a1t = [persist.tile([NBW, NB, NTW], F8, name=f"a1T_{t}", tag=f"a1T_{t}")
           for t in range(NT)]
    a2t = [persist.tile([NBW, NB, NTW], F8, name=f"a2T_{t}", tag=f"a2T_{t}")
           for t in range(NT)]
    for t in range(NT):
        sl = slice(t * NTW, (t + 1) * NTW)
        for name, lst in (("a1T", a1t), ("a2T", a2t)):
            nc.sync.dma_start(
                out=lst[t],
                in_=d[name][:, sl].rearrange("(j p) w -> p j w", p=NBW))

    # k-plane stride of a DoubleRow LDWEIGHTS AP must be 16-byte aligned,
    # so pad the ones column out to 16 bytes per plane
    ones8f = const.tile([NBW, 2, 16], F8, name="ones8")
    nc.vector.memset(ones8f, 1.0)
    ones8 = ones8f[:, :, 0:1]
    ones_row = const.tile([1, NBW], BF16, name="ones_row")
    nc.vector.memset(ones_row, C_ADJ)

    QT = persist.tile([U // 2, 2, N], F8, name="QT", tag="QT")
    KT = persist.tile([U // 2, 2, N], F8, name="KT", tag="KT")
    s_row = persist.tile([1, N], BF16, name="s_row", tag="s_row")
    rdbc = [persist.tile([NBW, NTW], BF16, name=f"rdbc{t}", tag=f"rdbc{t}")
            for t in range(NT)]
    ets = [persist.tile([NBW, NB, NTW], F8, name=f"et{t}", tag=f"et{t}")
           for t in range(NT)]

    # ---- prelude: K^T first (e-gen of t=0 reads all of K), then Q^T
    # (fp8 split-u packing), then s (not needed until the first dchain) ----
    for w, qt in ((wk, KT), (wq, QT)):
        for t in range(NT):
            sl = slice(t * NTW, (t + 1) * NTW)
            for half in range(2):
                pq = psscr.tile([U // 2, NTW], F32, name="pq", tag="scr")
                nc.tensor.matmul(pq, w[:, half * 32:(half + 1) * 32],
                                 fsT[0:FD, sl], start=True, stop=True)
                # alternate engines; both have startup slack
                if half:
                    nc.scalar.activation(qt[:, half, sl], pq, AF.Relu)
                else:
                    nc.vector.tensor_scalar_max(qt[:, half, sl], pq, 0.0)
    for t in range(NT):
        sl = slice(t * NTW, (t + 1) * NTW)
        ps1 = psscr.tile([U, NTW], F32, name="ps1", tag="scr")
        nc.tensor.matmul(ps1, ws1, fsT[:, sl], start=True, stop=True)
        s1t = stage.tile([U, NTW], BF16, name="s1t", tag="s1t")
        nc.scalar.activation(s1t, ps1, AF.Relu, bias=bs1v)
        ps2 = psscr.tile([1, NTW], F32, name="ps2", tag="scr")
        nc.tensor.matmul(ps2, ws2, s1t, start=True, stop=True)
        nc.scalar.activation(s_row[:, sl], ps2, AF.Relu, bias=bs2v)

    # ---- Y tiles: Y[m,b] = C_Y * X_b @ [k_r[m]|k_u[m]], stored fp8
    # [128, NB, 3, 128]: [node%128, node//128, m, u'] ----
    y = [ypool.tile([NBW, NB, 3, 2 * U], F8, name=f"y_{b}", tag=f"y{b}")
         for b in range(BL)]

    _eng = [0]

    def evac(out_ap, in_ap, dve_share=1):
        """PSUM evacuation, rotating DVE / ACT to balance load
        (`dve_share` DVE copies per ACT copy; -1 = DVE only, 0 = ACT
        only)."""
        if dve_share < 0:
            nc.vector.tensor_copy(out_ap, in_ap)
            return
        _eng[0] = (_eng[0] + 1) % (dve_share + 1)
        if _eng[0]:
            nc.vector.tensor_copy(out_ap, in_ap)
        else:
            nc.scalar.activation(out_ap, in_ap, AF.Copy)

    def ygen_thunks(b, dve_share=1):
        def mk(j):
            def f():
                nsl = slice(j * NBW, (j + 1) * NBW)
                py = psscr.tile([NBW, 3 * 2 * U], F32, name="py", tag="scr")
                nc.tensor.matmul(py, xT[b][:, nsl], kkall, start=True,
                                 stop=True)
                evac(y[b][:, j, :, :], py.rearrange("p (m u) -> p m u", m=3),
                     dve_share=dve_share)
            return f
        return [mk(j) for j in range(NB)]

    # y-gen for b=0/1, with tile-0's e^T generation interleaved so et[0]
    # is ready before the t-loop needs it (every later tile's e^T is
    # generated one tile ahead inside agroups, keeping the ACT exp burst
    # off the critical path)
    def e_thunks(t, et):
        """e^T[:, t] = exp(K Q^T / 8) into et (fp8), one node-block per
        call."""
        sl = slice(t * NTW, (t + 1) * NTW)

        def mk(j):
            def f():
                pe = psscr.tile([NBW, NTW], F32, name="pe", tag="scr")
                nc.tensor.matmul(pe, KT[:, :, j * NBW:(j + 1) * NBW],
                                 QT[:, :, sl], start=True, stop=True,
                                 perf_mode=DR)
                nc.scalar.activation(et[:, j, :], pe, AF.Exp, scale=0.125)
            return f
        return [mk(j) for j in range(NB)]

    def emit_dsum(et, pd):
        for j in range(NSB):
            nc.tensor.matmul(pd, ones8, et[:, 2 * j:2 * j + 2, :],
                             start=(j == 0), stop=(j == NSB - 1),
                             perf_mode=DR)

    def interleave(main, extra, ratio=2):
        """Emit `ratio` thunks from main per one from extra."""
        mi = ei = 0
        while mi < len(main) or ei < len(extra):
            for _ in range(ratio):
                if mi < len(main):
                    main[mi](); mi += 1
            if ei < len(extra):
                extra[ei](); ei += 1

    # y-gen for b=0/1, with tile-0's e^T generation interleaved so et[0]
    # is ready before the t-loop needs it (every later tile's e^T is
    # generated one tile ahead inside agroups, keeping the ACT exp burst
    # off the critical path)
    interleave(ygen_thunks(0, dve_share=-1) + ygen_thunks(1, dve_share=-1),
               e_thunks(0, ets[0]), ratio=2)

    # =================== phase 1: r & u gates ===================
    def a_thunks1(b, t, sl, pa):
        th = [lambda: nc.tensor.matmul(pa, kk0, xT[b][:, sl],
                                       start=True, stop=False)]
        for m, asl in ((0, a1t[t]), (1, a2t[t])):
            for j in range(NSB):
                def f(m=m, asl=asl, j=j):
                    nc.tensor.matmul(pa, y[b][:, 2 * j:2 * j + 2, m, :],
                                     asl[:, 2 * j:2 * j + 2, :],
                                     start=False,
                                     stop=(m == 1 and j == NSB - 1),
                                     perf_mode=DR)
                th.append(f)
        return th

    def agroup1(b, t, sl, extra=(), ratio=2):
        pa = psacc.tile([128, NTW], F32, name="pa", tag="acc")
        interleave(a_thunks1(b, t, sl, pa), list(extra), ratio=ratio)
        return pa

    def pbgroup(b, t, et, ytile=None, m=2):
        """The e-support contraction (PE only, so it can be emitted away
        from the DVE/ACT finish chain)."""
        yt = y[b] if ytile is None else ytile
        pb = psacc.tile([128, NTW], F32, name="pb", tag="acc")
        for j in range(NSB):
            nc.tensor.matmul(pb, yt[:, 2 * j:2 * j + 2, m, :],
                             et[:, 2 * j:2 * j + 2, :],
                             start=(j == 0), stop=(j == NSB - 1),
                             perf_mode=DR)
        return pb

    def bfinish1(b, t, sl, pa, pb):
        tmp = stage.tile([128, NTW], F32, name="tmp", tag="tmp")
        nc.vector.tensor_mul(tmp, pb, rdbc[t])
        ssum = stage.tile([128, NTW], F32, name="ssum", tag="ssum")
        nc.vector.tensor_add(ssum, pa, tmp)
        # sigmoid(z) = 0.5*(1 + tanh(z/2)): Tanh lives in the same ACT
        # function table as Exp/Copy, so phase 1 never reloads the table
        # (Sigmoid does not; a reload is 1283ns). sig holds th = 2r-1 /
        # 2u-1; the affine fixups are folded downstream.
        sig = stage.tile([128, NTW], F32, name="sig", tag="sig")
        nc.scalar.activation(sig, ssum, AF.Tanh, scale=SCL / 2, bias=bruh)
        # rh' = (1+th)*h = 2*(r*h) -> rows 0:64 in place as th*h + h
        # (the 0.5 is folded into the state rows of kc0/kcall host-side);
        # two plain tensor-tensor ops on the otherwise idle GpSimd.
        # thu -> DRAM scratch.
        nc.vector.scalar_tensor_tensor(
            xT[b][0:U, sl], sig[0:U, :], 1.0, xT[b][0:U, sl],
            op0=mybir.AluOpType.add, op1=mybir.AluOpType.mult)
        nc.scalar.dma_start(out=uscr[b][:, sl], in_=sig[U:128, :])

    def ycgen_thunks(yct, p, dve_share=1):
        th = []
        for half in range(2):
            b = 2 * p + half
            usl = slice(half * U, (half + 1) * U)
            for j in range(NB):
                def f(b=b, usl=usl, j=j, yct=yct):
                    nsl = slice(j * NBW, (j + 1) * NBW)
                    pyc = psscr.tile([NBW, 3 * U], F32, name="pyc", tag="scr")
                    nc.tensor.matmul(pyc, xT[b][:, nsl], kcall,
                                     start=True, stop=True)
                    evac(yct[:, j, :, usl],
                         pyc.rearrange("p (m u) -> p m u", m=3),
                         dve_share=dve_share)
                th.append(f)
        return th

    yc = [None, None]

    # Per-tile emission order keeps PE fed continuously:
    #   ag0(+e) ag1(+e) ag2 pb0 d8 [dchain] ag3 fin0 pb1 fin1 pb2 fin2
    #   pb3 fin3
    # The PSUM "acc" live set never exceeds 5 (pa0-3 + one pb).
    for t in range(NT):
        sl = slice(t * NTW, (t + 1) * NTW)
        et = ets[t]
        # generate NEXT tile's e^T inside this tile's agroups (this
        # tile's was generated one tile ago)
        eth = e_thunks(t + 1, ets[t + 1]) if t + 1 < NT else []
        if t == 0:
            y2g, y3g = ygen_thunks(2), ygen_thunks(3)
            pa0 = agroup1(0, t, sl, eth[0:8], ratio=1)
        else:
            pa0 = agroup1(0, t, sl, eth[0:8], ratio=2)

        # d/rdbc right after ag0 — et[t] was generated one tile ago, so
        # the serial d8 -> dsb -> 1/d -> broadcast chain hides under the
        # remaining agroups instead of stalling the finishes
        pd = psscr.tile([1, NTW], F32, name="pd", tag="scr")
        emit_dsum(et, pd)
        # d[n] = s[n] + colsum(e^T)[n]; rdbc[t][p, n] = C_ADJ / d[n]
        dsb = dsp.tile([1, NTW], F32, name="dsb", tag="dsb")
        nc.vector.tensor_add(dsb, pd, s_row[:, sl])
        rds = dsp.tile([1, NTW], BF16, name="rds", tag="rds")
        with nc.allow_low_precision(reason="1/d feeds an fp8-noise-dominated "
                                    "term; bf16 is plenty"):
            nc.vector.reciprocal(rds, dsb)
        pr = psscr.tile([128, NTW], F32, name="pr", tag="scr")
        nc.tensor.matmul(pr, ones_row, rds, start=True, stop=True)
        nc.vector.tensor_copy(rdbc[t], pr)

        if t == 0:
            pa1 = agroup1(1, t, sl, eth[8:16] + y2g, ratio=1)
            pa2 = agroup1(2, t, sl, y3g, ratio=1)
        else:
            pa1 = agroup1(1, t, sl, eth[8:16], ratio=2)
            pa2 = agroup1(2, t, sl)
        pb0 = pbgroup(0, t, et)
        bfinish1(0, t, sl, pa0, pb0)
        pa3 = agroup1(3, t, sl)
        if t == NT - 1:
            yc[0] = ypool.tile([NBW, NB, 3, 2 * U], F8, name="yc_0", tag="y0")
            pb1 = pbgroup(1, t, et)
            bfinish1(1, t, sl, pa1, pb1)
            # yc[0] gen needs the rh updates of b0/b1 (fin0/fin1) above;
            # interleave it with the remaining e-support contractions
            yc0g = ycgen_thunks(yc[0], 0)
            pb23 = []
            th23 = [lambda: pb23.append(pbgroup(2, t, et)),
                    lambda: pb23.append(pbgroup(3, t, et))]
            interleave(th23, yc0g, ratio=1)
            bfinish1(2, t, sl, pa2, pb23[0])
            bfinish1(3, t, sl, pa3, pb23[1])
        else:
            for b, pa in ((1, pa1), (2, pa2), (3, pa3)):
                pb = pbgroup(b, t, et)
                bfinish1(b, t, sl, pa, pb)

    # =================== phase 2+3: c gate & h_new ===================
    # Yc[pair] = C_Y * [Xc_b0 @ kc[m] | Xc_b1 @ kc[m]], fp8
    # (yc[0] was already generated inside phase-1's final A-group; yc[1]
    # is interleaved into phase-2 t=0's first A-group below, on the
    # mostly idle ACT)
    yc[1] = ypool.tile([NBW, NB, 3, 2 * U], F8, name="yc_1", tag="y1")
    yc1g = ycgen_thunks(yc[1], 1)

    for t in range(NT):
        sl = slice(t * NTW, (t + 1) * NTW)
        et = ets[t]
        a1, a2 = a1t[t], a2t[t]
        # prefetch h and u for the tail chains, one pair-merged DMA each
        hps, ups = [], []
        for p in range(BL // 2):
            hp = p3p.tile([128, NTW], F32, name="hp", tag="hp")
            up = p3p.tile([128, NTW], F32, name="up", tag="up")
            psl = slice(2 * p, 2 * p + 2)
            # the SBUF side stays a plain [128, 512] AP: its (partition,
            # col) iteration order already matches the DRAM side's
            # (b, u, w) — splitting the SBUF partition dim via rearrange
            # produces corrupt flat-offset strides
            nc.sync.dma_start(out=hp, in_=d["hT"][psl, :, sl])
            nc.sync.dma_start(out=up, in_=uscr[psl, :, sl])
            # uscr holds thu = 2u-1; recover u = 0.5*thu + 0.5 in place,
            # off the tail's critical path (the chain below then uses
            # only plain tensor-tensor ops)
            nc.vector.tensor_scalar(up, up, 1.0, 0.5,
                                    op0=mybir.AluOpType.add,
                                    op1=mybir.AluOpType.mult)
            hps.append(hp)
            ups.append(up)
        for p in range(BL // 2):
            b0, b1 = 2 * p, 2 * p + 1
            pa = psacc.tile([128, NTW], F32, name="pa2", tag="acc")
            th = [lambda pa=pa, p=p: nc.tensor.matmul(
                      pa, yc[p][:, 0:2, 0, :], a1[:, 0:2, :],
                      start=True, stop=False, perf_mode=DR),
                  lambda pa=pa, b0=b0: nc.tensor.matmul(
                      pa[0:U, :], kc0, xT[b0][:, sl], start=False, stop=False),
                  lambda pa=pa, b1=b1: nc.tensor.matmul(
                      pa[U:128, :], kc0, xT[b1][:, sl], start=False,
                      stop=False)]
            for m, asl in ((0, a1), (1, a2)):
                for j in range(NSB):
                    if m == 0 and j == 0:
                        continue
                    def f(pa=pa, p=p, m=m, asl=asl, j=j):
                        nc.tensor.matmul(pa, yc[p][:, 2 * j:2 * j + 2, m, :],
                                         asl[:, 2 * j:2 * j + 2, :],
                                         start=False,
                                         stop=(m == 1 and j == NSB - 1),
                                         perf_mode=DR)
                    th.append(f)
            if t == 0 and p == 0:
                interleave(th, yc1g, ratio=1)
            else:
                for f in th:
                    f()
            hp, up = hps[p], ups[p]
            pb = pbgroup(p, t, et, ytile=yc[p])
            tmp = stage.tile([128, NTW], F32, name="tmp2", tag="tmp")
            ct = ctp.tile([128, NTW], F32, name="ct", tag="ct")
            ssum = stage.tile([128, NTW], F32, name="ssum2", tag="ssum")
            # run the gate + h_new chain in column halves so the DVE / ACT
            # / GpSimd / DMA stages pipeline; the elementwise h_new chain
            # (h_new = c + u*(h - c), in place on hp) runs on the otherwise
            # idle GpSimd except on the final tile, where its latency is
            # exposed and DVE (2x f32 SBUF mode) drains faster.
            # the h_new chain stays OFF the DVE queue (in-order: a chain op
            # waiting on tanh would block the next pair's tmp/ssum) except
            # on the final tile, where DVE has nothing left queued and
            # drains faster than GpSimd
            veng = nc.vector if t == NT - 1 else nc.gpsimd
            for c0 in range(0, NTW, NTW // 2):
                cs = slice(c0, c0 + NTW // 2)
                nc.vector.tensor_mul(tmp[:, cs], pb[:, cs], rdbc[t][:, cs])
                nc.vector.tensor_add(ssum[:, cs], pa[:, cs], tmp[:, cs])
                nc.scalar.activation(ct[:, cs], ssum[:, cs], AF.Tanh,
                                     scale=SCL, bias=bc2)
                # h_new = c + u*(h - c), in place on hp
                veng.tensor_sub(hp[:, cs], hp[:, cs], ct[:, cs])
                veng.tensor_mul(hp[:, cs], up[:, cs], hp[:, cs])
                veng.tensor_add(hp[:, cs], hp[:, cs], ct[:, cs])
            # one pair-merged output DMA per (pair, tile)
            nc.scalar.dma_start(out=out_h[b0:b1 + 1, :, sl], in_=hp)

    ctx.close()


_CACHE = {}


def _get_program():
    if "nc" not in _CACHE:
        _CACHE["nc"] = _build_program()
    return _CACHE["nc"]


def _prep_inputs(inputs, h_prev, adj1, adj2, feat, SE, Wq, Wk, Ws1, bs1, Ws2,
                 bs2, r_kernel, r_bias, u_kernel, u_bias, c_kernel, c_bias):
    bf = ml_dtypes.bfloat16
    f8 = ml_dtypes.float8_e4m3
    f32 = np.float32
    perm = list(range(DIN, FROWS)) + list(range(DIN))  # [h(64); inputs(2)]

    h3 = np.asarray(h_prev, f32).reshape(B, N, U)
    hT = np.ascontiguousarray(h3.transpose(0, 2, 1))            # [B, U, N]
    inT = np.asarray(inputs, f32).transpose(0, 2, 1)            # [B, DIN, N]
    xT = np.concatenate([hT, inT], axis=1).astype(bf)           # [B, 66, N]

    rk = np.asarray(r_kernel, f32)[:, perm, :]
    uk = np.asarray(u_kernel, f32)[:, perm, :]
    ck = np.asarray(c_kernel, f32)[:, perm, :]
    # the device keeps rh' = 2*(r*h) in the state rows of x_cat_c (tanh
    # trick), so halve the state rows of the c kernels to compensate
    ck = ck.copy()
    ck[:, 0:U, :] *= 0.5
    kkall = (np.concatenate(
        [np.concatenate([rk[m], uk[m]], axis=1) for m in (1, 2, 3)],
        axis=1) * C_Y).astype(bf)                               # [66, 384]
    kk0 = (np.concatenate([rk[0], uk[0]], axis=1) * G).astype(bf)
    kcall = (np.concatenate([ck[1], ck[2], ck[3]], axis=1) * C_Y).astype(bf)
    kc0 = (ck[0] * G).astype(bf)

    shared = {
        "a1T": np.ascontiguousarray(
            np.asarray(adj1, f32).T * C_ADJ).astype(f8),
        "a2T": np.ascontiguousarray(
            np.asarray(adj2, f32).T * C_ADJ).astype(f8),
        "fsT": np.ascontiguousarray(
            np.concatenate([np.asarray(feat, f32).T, np.asarray(SE, f32).T],
                           axis=0)).astype(bf),
        "wqk": np.concatenate([np.asarray(Wq, f32), np.asarray(Wk, f32)],
                              axis=1).astype(bf),
        "ws1": np.asarray(Ws1, f32).astype(bf),
        "ws2": np.asarray(Ws2, f32).reshape(U, 1).astype(bf),
        "kblob": np.concatenate([kkall, kk0, kcall, kc0], axis=1),
        "bs1v": np.asarray(bs1, f32).reshape(U, 1),
        "bruh": 0.5 * np.concatenate(
            [np.asarray(r_bias, f32).mean(0),
             np.asarray(u_bias, f32).mean(0)]).reshape(-1, 1),
        "bc2": np.tile(np.asarray(c_bias, f32).mean(0), 2).reshape(-1, 1),
        "bs2v": np.asarray(bs2, f32).reshape(1, 1),
    }
    in_maps = []
    for c in range(NCORES):
        bsl = slice(c * BL, (c + 1) * BL)
        m = dict(shared)
        m["xT"] = np.ascontiguousarray(xT[bsl])
        m["hT"] = np.ascontiguousarray(hT[bsl])
        in_maps.append(m)
    return in_maps


def kernel(**inputs):
    os.environ.setdefault("NEURON_RT_RESET_CORES", "1")
    nc = _get_program()
    in_maps = _prep_inputs(**inputs)
    res = None
    err = None
    for _ in range(2):
        try:
            res = run_bass_kernel_spmd(nc, in_maps, list(range(NCORES)))
            break
        except Exception as e:  # e.g. a wedged device; retry once
            err = e
    if res is None:
        raise err
    outs = []
    for c in range(NCORES):
        o = res.results[c]["out"]                     # [BL, U, N] f32
        outs.append(o.transpose(0, 2, 1).reshape(BL, N * U))
    return np.concatenate(outs, axis=0).astype(np.float32)


# revision 79
# speedup vs baseline: 2.2919x; 1.0055x over previous
"""MFGCGRU (graph-conv GRU cell) Trainium2 kernel.

Strategy: data-parallel over batch B=32 across 8 NeuronCores (4 batches
per core). All NxN supports replicated per core and resident in SBUF.

The dominant work — applying the three supports (adj1, adj2, e-attn) to
the per-gate projections Y_m = X @ k_m — runs as fp8(e4m3) matmuls in
DoubleRow perf mode: operands are packed [128, 2, F] so each PE pass
contracts 256 source nodes at half a cycle per output column.

Scaling bookkeeping (so fp8 operands sit in e4m3's sweet spot):
  - adjacencies are sent as 64*S^T            (C_ADJ = 64)
  - support kernels are folded with x16       (C_Y = 16, Y' = 16*X@k)
  - identity kernels are folded with x1024    (G = C_ADJ*C_Y)
  - the e-term normalizer is rdbc = 64/d, applied to the (x16-scaled)
    e-contribution PSUM, matching the x1024 of the adj/identity terms
  - gate activations then use ACT scale = 0.25/1024 (the /M fold)

The attention support is built unnormalized as e^T = exp(K Q^T / 8)
from fp8 Q/K packed [32, 2, N] (u = plane*32 + p), stored fp8 and kept
resident for both the r/u and the c passes; its row-normalizer is
applied to the e-contribution via a second PSUM accumulator.
"""

import contextlib
import os

import numpy as np
import ml_dtypes

import concourse.bass as bass
import concourse.bacc as bacc
import concourse.tile as tile
from concourse import mybir
from concourse.bass_utils import run_bass_kernel_spmd

F32 = mybir.dt.float32
BF16 = mybir.dt.bfloat16
F8 = mybir.dt.float8e4
AF = mybir.ActivationFunctionType
DR = mybir.MatmulPerfMode.DoubleRow

B, N, DIN, U, FD, SD = 32, 2048, 2, 64, 32, 64
NCORES = 8
BL = B // NCORES          # batches per core
NTW = 512                 # n-tile width
NT = N // NTW             # 4 n-tiles
NBW = 128                 # node-block width
NB = N // NBW             # 16 node blocks
NSB = NB // 2             # 8 node super-blocks (256 nodes, DoubleRow)
FROWS = DIN + U           # 66

C_ADJ = 64.0              # host scale on S^T before e4m3 cast
C_Y = 16.0                # host scale folded into support kernels
G = C_ADJ * C_Y           # net scale of the PSUM accumulators
SCL = 0.25 / G            # ACT scale for the gate activations (incl /M)


def _build_program():
    nc = bacc.Bacc("TRN2", debug=False, num_devices=NCORES)

    d = {}

    def din(name, shape, dt):
        d[name] = nc.dram_tensor(name, shape, dt, kind="ExternalInput").ap()

    din("xT", [BL, FROWS, N], BF16)
    din("hT", [BL, U, N], F32)
    din("a1T", [N, N], F8)
    din("a2T", [N, N], F8)
    din("fsT", [FD + SD, N], BF16)
    # kernels are packed into one tensor: each dma_start holds the
    # (single, serialized) HWDGE device ~630ns regardless of size
    din("wqk", [FD, 2 * U], BF16)            # [wq | wk]
    din("ws1", [FD + SD, U], BF16)
    din("ws2", [U, 1], BF16)
    din("kblob", [FROWS, 768], BF16)         # [kkall|kk0|kcall|kc0]
    din("bs1v", [U, 1], F32)
    din("bruh", [2 * U, 1], F32)
    din("bc2", [2 * U, 1], F32)
    din("bs2v", [1, 1], F32)
    out_h = nc.dram_tensor("out", [BL, U, N], F32, kind="ExternalOutput").ap()
    uscr = nc.dram_tensor("uscr", [BL, U, N], F32).ap()

    with tile.TileContext(nc) as tc:
        _emit(tc, d, out_h, uscr)
    nc.compile()
    return nc


def _emit(tc, d, out_h, uscr):
    nc = tc.nc
    ctx = contextlib.ExitStack()
    const = ctx.enter_context(tc.tile_pool(name="const", bufs=1))
    persist = ctx.enter_context(tc.tile_pool(name="persist", bufs=1))
    ypool = ctx.enter_context(tc.tile_pool(name="ypool", bufs=1))
    stage = ctx.enter_context(tc.tile_pool(name="stage", bufs=3))
    # phase-2/3 pipeline tiles: 4 pair-tile tails in flight, so the h/u
    # prefetch and the tanh of tile t+1 never wait on tile t's drain
    p3p = ctx.enter_context(tc.tile_pool(name="p3p", bufs=4))
    ctp = ctx.enter_context(tc.tile_pool(name="ctp", bufs=4))
    dsp = ctx.enter_context(tc.tile_pool(name="dsp", bufs=1))
    psacc = ctx.enter_context(tc.tile_pool(name="psacc", bufs=4, space="PSUM"))
    psscr = ctx.enter_context(tc.tile_pool(name="psscr", bufs=4, space="PSUM"))

    # ---- constants / weights in SBUF ----
    def cload(name, shape=None, dt=None):
        ap = d[name]
        t = const.tile(list(ap.shape) if shape is None else shape,
                       ap.dtype if dt is None else dt, name=f"c_{name}")
        nc.sync.dma_start(out=t, in_=ap)
        return t

    # DMA order matters: the startup critical path is
    #   fsT/wqk (prelude) and xT/kblob (y-gen), then adj slice t=0.
    fsT = cload("fsT")
    wqk = cload("wqk")
    wq, wk = wqk[:, 0:U], wqk[:, U:2 * U]
    kblob = cload("kblob")
    kkall = kblob[:, 0:384]
    kk0 = kblob[:, 384:512]
    kcall = kblob[:, 512:704]
    kc0 = kblob[:, 704:768]

    xTall = persist.tile([FROWS, BL, N], BF16, name="xTall", tag="xTall")
    # two DMAs so y-gen for b=0/1 can start before b=2/3 lands
    nc.sync.dma_start(out=xTall[:, 0:2, :],
                      in_=d["xT"][0:2].rearrange("b f n -> f b n"))
    nc.sync.dma_start(out=xTall[:, 2:4, :],
                      in_=d["xT"][2:4].rearrange("b f n -> f b n"))
    xT = [xTall[:, b, :] for b in range(BL)]

    ws1 = cload("ws1")
    ws2 = cload("ws2")
    bs1v = cload("bs1v")
    bruh = cload("bruh")        # pre-halved host-side for the tanh trick
    bc2 = cload("bc2")
    bs2v = cload("bs2v")

    # ---- resident adjacency slices (fp8, reused by both phases) ----
    a1t = [persist.tile([NBW, NB, NTW], F8, name=f"a1T_{t}", tag=f"a1T_{t}")
           for t in range(NT)]
    a2t = [persist.tile([NBW, NB, NTW], F8, name=f"a2T_{t}", tag=f"a2T_{t}")
           for t in range(NT)]
    for t in range(NT):
        sl = slice(t * NTW, (t + 1) * NTW)
        for name, lst in (("a1T", a1t), ("a2T", a2t)):
            nc.sync.dma_start(
                out=lst[t],
                in_=d[name][:, sl].rearrange("(j p) w -> p j w", p=NBW))

    # k-plane stride of a DoubleRow LDWEIGHTS AP must be 16-byte aligned,
    # so pad the ones column out to 16 bytes per plane
    ones8f = const.tile([NBW, 2, 16], F8, name="ones8")
    nc.vector.memset(ones8f, 1.0)
    ones8 = ones8f[:, :, 0:1]
    ones_row = const.tile([1, NBW], BF16, name="ones_row")
    nc.vector.memset(ones_row, C_ADJ)

    QT = persist.tile([U // 2, 2, N], F8, name="QT", tag="QT")
    KT = persist.tile([U // 2, 2, N], F8, name="KT", tag="KT")
    s_row = persist.tile([1, N], BF16, name="s_row", tag="s_row")
    rdbc = [persist.tile([NBW, NTW], BF16, name=f"rdbc{t}", tag=f"rdbc{t}")
            for t in range(NT)]
    ets = [persist.tile([NBW, NB, NTW], F8, name=f"et{t}", tag=f"et{t}")
           for t in range(NT)]

    # ---- prelude: K^T first (e-gen of t=0 reads all of K), then Q^T
    # (fp8 split-u packing), then s (not needed until the first dchain) ----
    for w, qt in ((wk, KT), (wq, QT)):
        for t in range(NT):
            sl = slice(t * NTW, (t + 1) * NTW)
            for half in range(2):
                pq = psscr.tile([U // 2, NTW], F32, name="pq", tag="scr")
                nc.tensor.matmul(pq, w[:, half * 32:(half + 1) * 32],
                                 fsT[0:FD, sl], start=True, stop=True)
                # alternate engines; both have startup slack
                if half:
                    nc.scalar.activation(qt[:, half, sl], pq, AF.Relu)
                else:
                    nc.vector.tensor_scalar_max(qt[:, half, sl], pq, 0.0)
    for t in range(NT):
        sl = slice(t * NTW, (t + 1) * NTW)
        ps1 = psscr.tile([U, NTW], F32, name="ps1", tag="scr")
        nc.tensor.matmul(ps1, ws1, fsT[:, sl], start=True, stop=True)
        s1t = stage.tile([U, NTW], BF16, name="s1t", tag="s1t")
        nc.scalar.activation(s1t, ps1, AF.Relu, bias=bs1v)
        ps2 = psscr.tile([1, NTW], F32, name="ps2", tag="scr")
        nc.tensor.matmul(ps2, ws2, s1t, start=True, stop=True)
        nc.scalar.activation(s_row[:, sl], ps2, AF.Relu, bias=bs2v)

    # ---- Y tiles: Y[m,b] = C_Y * X_b @ [k_r[m]|k_u[m]], stored fp8
    # [128, NB, 3, 128]: [node%128, node//128, m, u'] ----
    y = [ypool.tile([NBW, NB, 3, 2 * U], F8, name=f"y_{b}", tag=f"y{b}")
         for b in range(BL)]

    _eng = [0]

    def evac(out_ap, in_ap, dve_share=1):
        """PSUM evacuation, rotating DVE / ACT to balance load
        (`dve_share` DVE copies per ACT copy; -1 = DVE only, 0 = ACT
        only)."""
        if dve_share < 0:
            nc.vector.tensor_copy(out_ap, in_ap)
            return
        _eng[0] = (_eng[0] + 1) % (dve_share + 1)
        if _eng[0]:
            nc.vector.tensor_copy(out_ap, in_ap)
        else:
            nc.scalar.activation(out_ap, in_ap, AF.Copy)

    def ygen_thunks(b, dve_share=1):
        def mk(j):
            def f():
                nsl = slice(j * NBW, (j + 1) * NBW)
                py = psscr.tile([NBW, 3 * 2 * U], F32, name="py", tag="scr")
                nc.tensor.matmul(py, xT[b][:, nsl], kkall, start=True,
                                 stop=True)
                evac(y[b][:, j, :, :], py.rearrange("p (m u) -> p m u", m=3),
                     dve_share=dve_share)
            return f
        return [mk(j) for j in range(NB)]

    # y-gen for b=0/1, with tile-0's e^T generation interleaved so et[0]
    # is ready before the t-loop needs it (every later tile's e^T is
    # generated one tile ahead inside agroups, keeping the ACT exp burst
    # off the critical path)
    def e_thunks(t, et):
        """e^T[:, t] = exp(K Q^T / 8) into et (fp8), one node-block per
        call."""
        sl = slice(t * NTW, (t + 1) * NTW)

        def mk(j):
            def f():
                pe = psscr.tile([NBW, NTW], F32, name="pe", tag="scr")
                nc.tensor.matmul(pe, KT[:, :, j * NBW:(j + 1) * NBW],
                                 QT[:, :, sl], start=True, stop=True,
                                 perf_mode=DR)
                nc.scalar.activation(et[:, j, :], pe, AF.Exp, scale=0.125)
            return f
        return [mk(j) for j in range(NB)]

    def emit_dsum(et, pd):
        for j in range(NSB):
            nc.tensor.matmul(pd, ones8, et[:, 2 * j:2 * j + 2, :],
                             start=(j == 0), stop=(j == NSB - 1),
                             perf_mode=DR)

    def interleave(main, extra, ratio=2):
        """Emit `ratio` thunks from main per one from extra."""
        mi = ei = 0
        while mi < len(main) or ei < len(extra):
            for _ in range(ratio):
                if mi < len(main):
                    main[mi](); mi += 1
            if ei < len(extra):
                extra[ei](); ei += 1

    # y-gen for b=0/1, with tile-0's e^T generation interleaved so et[0]
    # is ready before the t-loop needs it (every later tile's e^T is
    # generated one tile ahead inside agroups, keeping the ACT exp burst
    # off the critical path)
    interleave(ygen_thunks(0, dve_share=-1) + ygen_thunks(1, dve_share=-1),
               e_thunks(0, ets[0]), ratio=2)

    # =================== phase 1: r & u gates ===================
    def a_thunks1(b, t, sl, pa):
        th = [lambda: nc.tensor.matmul(pa, kk0, xT[b][:, sl],
                                       start=True, stop=False)]
        for m, asl in ((0, a1t[t]), (1, a2t[t])):
            for j in range(NSB):
                def f(m=m, asl=asl, j=j):
                    nc.tensor.matmul(pa, y[b][:, 2 * j:2 * j + 2, m, :],
                                     asl[:, 2 * j:2 * j + 2, :],
                                     start=False,
                                     stop=(m == 1 and j == NSB - 1),
                                     perf_mode=DR)
                th.append(f)
        return th

    def agroup1(b, t, sl, extra=(), ratio=2):
        pa = psacc.tile([128, NTW], F32, name="pa", tag="acc")
        interleave(a_thunks1(b, t, sl, pa), list(extra), ratio=ratio)
        return pa

    def pbgroup(b, t, et, ytile=None, m=2):
        """The e-support contraction (PE only, so it can be emitted away
        from the DVE/ACT finish chain)."""
        yt = y[b] if ytile is None else ytile
        pb = psacc.tile([128, NTW], F32, name="pb", tag="acc")
        for j in range(NSB):
            nc.tensor.matmul(pb, yt[:, 2 * j:2 * j + 2, m, :],
                             et[:, 2 * j:2 * j + 2, :],
                             start=(j == 0), stop=(j == NSB - 1),
                             perf_mode=DR)
        return pb

    def bfinish1(b, t, sl, pa, pb):
        tmp = stage.tile([128, NTW], F32, name="tmp", tag="tmp")
        nc.vector.tensor_mul(tmp, pb, rdbc[t])
        ssum = stage.tile([128, NTW], F32, name="ssum", tag="ssum")
        nc.vector.tensor_add(ssum, pa, tmp)
        # sigmoid(z) = 0.5*(1 + tanh(z/2)): Tanh lives in the same ACT
        # function table as Exp/Copy, so phase 1 never reloads the table
        # (Sigmoid does not; a reload is 1283ns). sig holds th = 2r-1 /
        # 2u-1; the affine fixups are folded downstream.
        sig = stage.tile([128, NTW], F32, name="sig", tag="sig")
        nc.scalar.activation(sig, ssum, AF.Tanh, scale=SCL / 2, bias=bruh)
        # rh' = (1+th)*h = 2*(r*h) -> rows 0:64 in place as th*h + h
        # (the 0.5 is folded into the state rows of kc0/kcall host-side);
        # two plain tensor-tensor ops on the otherwise idle GpSimd.
        # thu -> DRAM scratch.
        nc.vector.scalar_tensor_tensor(
            xT[b][0:U, sl], sig[0:U, :], 1.0, xT[b][0:U, sl],
            op0=mybir.AluOpType.add, op1=mybir.AluOpType.mult)
        nc.scalar.dma_start(out=uscr[b][:, sl], in_=sig[U:128, :])

    def ycgen_thunks(yct, p, dve_share=1):
        th = []
        for half in range(2):
            b = 2 * p + half
            usl = slice(half * U, (half + 1) * U)
            for j in range(NB):
                def f(b=b, usl=usl, j=j, yct=yct):
                    nsl = slice(j * NBW, (j + 1) * NBW)
                    pyc = psscr.tile([NBW, 3 * U], F32, name="pyc", tag="scr")
                    nc.tensor.matmul(pyc, xT[b][:, nsl], kcall,
                                     start=True, stop=True)
                    evac(yct[:, j, :, usl],
                         pyc.rearrange("p (m u) -> p m u", m=3),
                         dve_share=dve_share)
                th.append(f)
        return th

    yc = [None, None]

    # Per-tile emission order keeps PE fed continuously:
    #   ag0(+e) ag1(+e) ag2 pb0 d8 [dchain] ag3 fin0 pb1 fin1 pb2 fin2
    #   pb3 fin3
    # The PSUM "acc" live set never exceeds 5 (pa0-3 + one pb).
    for t in range(NT):
        sl = slice(t * NTW, (t + 1) * NTW)
        et = ets[t]
        # generate NEXT tile's e^T inside this tile's agroups (this
        # tile's was generated one tile ago)
        eth = e_thunks(t + 1, ets[t + 1]) if t + 1 < NT else []
        if t == 0:
            y2g, y3g = ygen_thunks(2, dve_share=2), ygen_thunks(3, dve_share=2)
            pa0 = agroup1(0, t, sl, eth[0:8], ratio=1)
        else:
            pa0 = agroup1(0, t, sl, eth[0:8], ratio=2)

        # d/rdbc right after ag0 — et[t] was generated one tile ago, so
        # the serial d8 -> dsb -> 1/d -> broadcast chain hides under the
        # remaining agroups instead of stalling the finishes
        pd = psscr.tile([1, NTW], F32, name="pd", tag="scr")
        emit_dsum(et, pd)
        # d[n] = s[n] + colsum(e^T)[n]; rdbc[t][p, n] = C_ADJ / d[n]
        dsb = dsp.tile([1, NTW], F32, name="dsb", tag="dsb")
        nc.vector.tensor_add(dsb, pd, s_row[:, sl])
        rds = dsp.tile([1, NTW], BF16, name="rds", tag="rds")
        with nc.allow_low_precision(reason="1/d feeds an fp8-noise-dominated "
                                    "term; bf16 is plenty"):
            nc.vector.reciprocal(rds, dsb)
        pr = psscr.tile([128, NTW], F32, name="pr", tag="scr")
        nc.tensor.matmul(pr, ones_row, rds, start=True, stop=True)
        nc.vector.tensor_copy(rdbc[t], pr)

        if t == 0:
            pa1 = agroup1(1, t, sl, eth[8:16] + y2g, ratio=1)
            pa2 = agroup1(2, t, sl, y3g, ratio=1)
        else:
            pa1 = agroup1(1, t, sl, eth[8:16], ratio=2)
            pa2 = agroup1(2, t, sl)
        pb0 = pbgroup(0, t, et)
        bfinish1(0, t, sl, pa0, pb0)
        pa3 = agroup1(3, t, sl)
        if t == NT - 1:
            yc[0] = ypool.tile([NBW, NB, 3, 2 * U], F8, name="yc_0", tag="y0")
            pb1 = pbgroup(1, t, et)
            bfinish1(1, t, sl, pa1, pb1)
            # yc[0] gen needs the rh updates of b0/b1 (fin0/fin1) above;
            # interleave it with the remaining e-support contractions
            yc0g = ycgen_thunks(yc[0], 0)
            pb23 = []
            th23 = [lambda: pb23.append(pbgroup(2, t, et)),
                    lambda: pb23.append(pbgroup(3, t, et))]
            interleave(th23, yc0g, ratio=1)
            bfinish1(2, t, sl, pa2, pb23[0])
            bfinish1(3, t, sl, pa3, pb23[1])
        else:
            for b, pa in ((1, pa1), (2, pa2), (3, pa3)):
                pb = pbgroup(b, t, et)
                bfinish1(b, t, sl, pa, pb)

    # =================== phase 2+3: c gate & h_new ===================
    # Yc[pair] = C_Y * [Xc_b0 @ kc[m] | Xc_b1 @ kc[m]], fp8
    # (yc[0] was already generated inside phase-1's final A-group; yc[1]
    # is interleaved into phase-2 t=0's first A-group below, on the
    # mostly idle ACT)
    yc[1] = ypool.tile([NBW, NB, 3, 2 * U], F8, name="yc_1", tag="y1")
    yc1g = ycgen_thunks(yc[1], 1)

    for t in range(NT):
        sl = slice(t * NTW, (t + 1) * NTW)
        et = ets[t]
        a1, a2 = a1t[t], a2t[t]
        # prefetch h and u for the tail chains, one pair-merged DMA each
        hps, ups = [], []
        for p in range(BL // 2):
            hp = p3p.tile([128, NTW], F32, name="hp", tag="hp")
            up = p3p.tile([128, NTW], F32, name="up", tag="up")
            psl = slice(2 * p, 2 * p + 2)
            # the SBUF side stays a plain [128, 512] AP: its (partition,
            # col) iteration order already matches the DRAM side's
            # (b, u, w) — splitting the SBUF partition dim via rearrange
            # produces corrupt flat-offset strides
            nc.sync.dma_start(out=hp, in_=d["hT"][psl, :, sl])
            nc.sync.dma_start(out=up, in_=uscr[psl, :, sl])
            # uscr holds thu = 2u-1; recover u = 0.5*thu + 0.5 in place,
            # off the tail's critical path (the chain below then uses
            # only plain tensor-tensor ops)
            nc.vector.tensor_scalar(up, up, 1.0, 0.5,
                                    op0=mybir.AluOpType.add,
                                    op1=mybir.AluOpType.mult)
            hps.append(hp)
            ups.append(up)
        for p in range(BL // 2):
            b0, b1 = 2 * p, 2 * p + 1
            pa = psacc.tile([128, NTW], F32, name="pa2", tag="acc")
            th = [lambda pa=pa, p=p: nc.tensor.matmul(
                      pa, yc[p][:, 0:2, 0, :], a1[:, 0:2, :],
                      start=True, stop=False, perf_mode=DR),
                  lambda pa=pa, b0=b0: nc.tensor.matmul(
                      pa[0:U, :], kc0, xT[b0][:, sl], start=False, stop=False),
                  lambda pa=pa, b1=b1: nc.tensor.matmul(
                      pa[U:128, :], kc0, xT[b1][:, sl], start=False,
                      stop=False)]
            for m, asl in ((0, a1), (1, a2)):
                for j in range(NSB):
                    if m == 0 and j == 0:
                        continue
                    def f(pa=pa, p=p, m=m, asl=asl, j=j):
                        nc.tensor.matmul(pa, yc[p][:, 2 * j:2 * j + 2, m, :],
                                         asl[:, 2 * j:2 * j + 2, :],
                                         start=False,
                                         stop=(m == 1 and j == NSB - 1),
                                         perf_mode=DR)
                    th.append(f)
            if t == 0 and p == 0:
                interleave(th, yc1g, ratio=1)
            else:
                for f in th:
                    f()
            hp, up = hps[p], ups[p]
            pb = pbgroup(p, t, et, ytile=yc[p])
            tmp = stage.tile([128, NTW], F32, name="tmp2", tag="tmp")
            ct = ctp.tile([128, NTW], F32, name="ct", tag="ct")
            ssum = stage.tile([128, NTW], F32, name="ssum2", tag="ssum")
            # run the gate + h_new chain in column halves so the DVE / ACT
            # / GpSimd / DMA stages pipeline; the elementwise h_new chain
            # (h_new = c + u*(h - c), in place on hp) runs on the otherwise
            # idle GpSimd except on the final tile, where its latency is
            # exposed and DVE (2x f32 SBUF mode) drains faster.
            # the h_new chain stays OFF the DVE queue (in-order: a chain op
            # waiting on tanh would block the next pair's tmp/ssum) except
            # on the final tile, where DVE has nothing left queued and
            # drains faster than GpSimd
            veng = nc.vector if t == NT - 1 else nc.gpsimd
            for c0 in range(0, NTW, NTW // 2):
                cs = slice(c0, c0 + NTW // 2)
                nc.vector.tensor_mul(tmp[:, cs], pb[:, cs], rdbc[t][:, cs])
                nc.vector.tensor_add(ssum[:, cs], pa[:, cs], tmp[:, cs])
                nc.scalar.activation(ct[:, cs], ssum[:, cs], AF.Tanh,
                                     scale=SCL, bias=bc2)
                # h_new = c + u*(h - c), in place on hp
                veng.tensor_sub(hp[:, cs], hp[:, cs], ct[:, cs])
                veng.tensor_mul(hp[:, cs], up[:, cs], hp[:, cs])
                veng.tensor_add(hp[:, cs], hp[:, cs], ct[:, cs])
            # one pair-merged output DMA per (pair, tile)
            nc.scalar.dma_start(out=out_h[b0:b1 + 1, :, sl], in_=hp)

    ctx.close()


_CACHE = {}


def _get_program():
    if "nc" not in _CACHE:
        _CACHE["nc"] = _build_program()
    return _CACHE["nc"]


def _prep_inputs(inputs, h_prev, adj1, adj2, feat, SE, Wq, Wk, Ws1, bs1, Ws2,
                 bs2, r_kernel, r_bias, u_kernel, u_bias, c_kernel, c_bias):
    bf = ml_dtypes.bfloat16
    f8 = ml_dtypes.float8_e4m3
    f32 = np.float32
    perm = list(range(DIN, FROWS)) + list(range(DIN))  # [h(64); inputs(2)]

    h3 = np.asarray(h_prev, f32).reshape(B, N, U)
    hT = np.ascontiguousarray(h3.transpose(0, 2, 1))            # [B, U, N]
    inT = np.asarray(inputs, f32).transpose(0, 2, 1)            # [B, DIN, N]
    xT = np.concatenate([hT, inT], axis=1).astype(bf)           # [B, 66, N]

    rk = np.asarray(r_kernel, f32)[:, perm, :]
    uk = np.asarray(u_kernel, f32)[:, perm, :]
    ck = np.asarray(c_kernel, f32)[:, perm, :]
    # the device keeps rh' = 2*(r*h) in the state rows of x_cat_c (tanh
    # trick), so halve the state rows of the c kernels to compensate
    ck = ck.copy()
    ck[:, 0:U, :] *= 0.5
    kkall = (np.concatenate(
        [np.concatenate([rk[m], uk[m]], axis=1) for m in (1, 2, 3)],
        axis=1) * C_Y).astype(bf)                               # [66, 384]
    kk0 = (np.concatenate([rk[0], uk[0]], axis=1) * G).astype(bf)
    kcall = (np.concatenate([ck[1], ck[2], ck[3]], axis=1) * C_Y).astype(bf)
    kc0 = (ck[0] * G).astype(bf)

    shared = {
        "a1T": np.ascontiguousarray(
            np.asarray(adj1, f32).T * C_ADJ).astype(f8),
        "a2T": np.ascontiguousarray(
            np.asarray(adj2, f32).T * C_ADJ).astype(f8),
        "fsT": np.ascontiguousarray(
            np.concatenate([np.asarray(feat, f32).T, np.asarray(SE, f32).T],
                           axis=0)).astype(bf),
        "wqk": np.concatenate([np.asarray(Wq, f32), np.asarray(Wk, f32)],
                              axis=1).astype(bf),
        "ws1": np.asarray(Ws1, f32).astype(bf),
        "ws2": np.asarray(Ws2, f32).reshape(U, 1).astype(bf),
        "kblob": np.concatenate([kkall, kk0, kcall, kc0], axis=1),
        "bs1v": np.asarray(bs1, f32).reshape(U, 1),
        "bruh": 0.5 * np.concatenate(
            [np.asarray(r_bias, f32).mean(0),
             np.asarray(u_bias, f32).mean(0)]).reshape(-1, 1),
        "bc2": np.tile(np.asarray(c_bias, f32).mean(0), 2).reshape(-1, 1),
        "bs2v": np.asarray(bs2, f32).reshape(1, 1),
    }
    in_maps = []
    for c in range(NCORES):
        bsl = slice(c * BL, (c + 1) * BL)
        m = dict(shared)
        m["xT"] = np.ascontiguousarray(xT[bsl])
        m["hT"] = np.ascontiguousarray(hT[bsl])
        in_maps.append(m)
    return in_maps


def kernel(**inputs):
    os.environ.setdefault("NEURON_RT_RESET_CORES", "1")
    nc = _get_program()
    in_maps = _prep_inputs(**inputs)
    res = None
    err = None
    for _ in range(2):
        try:
            res = run_bass_kernel_spmd(nc, in_maps, list(range(NCORES)))
            break
        except Exception as e:  # e.g. a wedged device; retry once
            err = e
    if res is None:
        raise err
    outs = []
    for c in range(NCORES):
        o = res.results[c]["out"]                     # [BL, U, N] f32
        outs.append(o.transpose(0, 2, 1).reshape(BL, N * U))
    return np.concatenate(outs, axis=0).astype(np.float32)


# revision 94
# speedup vs baseline: 2.3503x; 1.0255x over previous
"""MFGCGRU (graph-conv GRU cell) Trainium2 kernel.

Strategy: data-parallel over batch B=32 across 8 NeuronCores (4 batches
per core). All NxN supports replicated per core and resident in SBUF.

The dominant work — applying the three supports (adj1, adj2, e-attn) to
the per-gate projections Y_m = X @ k_m — runs as fp8(e4m3) matmuls in
DoubleRow perf mode: operands are packed [128, 2, F] so each PE pass
contracts 256 source nodes at half a cycle per output column.

Scaling bookkeeping (so fp8 operands sit in e4m3's sweet spot):
  - adjacencies are sent as 64*S^T            (C_ADJ = 64)
  - support kernels are folded with x16       (C_Y = 16, Y' = 16*X@k)
  - identity kernels are folded with x1024    (G = C_ADJ*C_Y)
  - the e-term normalizer is rdbc = 64/d, applied to the (x16-scaled)
    e-contribution PSUM, matching the x1024 of the adj/identity terms
  - gate activations then use ACT scale = 0.25/1024 (the /M fold)

The attention support is built unnormalized as e^T = exp(K Q^T / 8)
from fp8 Q/K packed [32, 2, N] (u = plane*32 + p), stored fp8 and kept
resident for both the r/u and the c passes; its row-normalizer is
applied to the e-contribution via a second PSUM accumulator.
"""

import contextlib
import os

import numpy as np
import ml_dtypes

import concourse.bass as bass
import concourse.bacc as bacc
import concourse.tile as tile
from concourse import mybir
from concourse.bass_utils import run_bass_kernel_spmd

F32 = mybir.dt.float32
BF16 = mybir.dt.bfloat16
F8 = mybir.dt.float8e4
AF = mybir.ActivationFunctionType
DR = mybir.MatmulPerfMode.DoubleRow

B, N, DIN, U, FD, SD = 32, 2048, 2, 64, 32, 64
NCORES = 8
BL = B // NCORES          # batches per core
NTW = 512                 # n-tile width
NT = N // NTW             # 4 n-tiles
NBW = 128                 # node-block width
NB = N // NBW             # 16 node blocks
NSB = NB // 2             # 8 node super-blocks (256 nodes, DoubleRow)
FROWS = DIN + U           # 66

C_ADJ = 64.0              # host scale on S^T before e4m3 cast
C_Y = 16.0                # host scale folded into support kernels
G = C_ADJ * C_Y           # net scale of the PSUM accumulators
SCL = 0.25 / G            # ACT scale for the gate activations (incl /M)


def _build_program():
    nc = bacc.Bacc("TRN2", debug=False, num_devices=NCORES)

    d = {}

    def din(name, shape, dt):
        d[name] = nc.dram_tensor(name, shape, dt, kind="ExternalInput").ap()

    din("xT", [BL, FROWS, N], BF16)
    din("hT", [BL, U, N], F32)
    din("a1T", [N, N], F8)
    din("a2T", [N, N], F8)
    din("fsT", [FD + SD, N], BF16)
    # kernels are packed into one tensor: each dma_start holds the
    # (single, serialized) HWDGE device ~630ns regardless of size
    din("wqk", [FD, 2 * U], BF16)            # [wq | wk]
    din("ws1", [FD + SD, U], BF16)
    din("ws2", [U, 1], BF16)
    din("kblob", [FROWS, 768], BF16)         # [kkall|kk0|kcall|kc0]
    din("bs1v", [U, 1], F32)
    din("bruh", [2 * U, 1], F32)
    din("bc2", [2 * U, 1], F32)
    din("bs2v", [1, 1], F32)
    out_h = nc.dram_tensor("out", [BL, U, N], F32, kind="ExternalOutput").ap()
    uscr = nc.dram_tensor("uscr", [BL, U, N], F32).ap()

    with tile.TileContext(nc) as tc:
        _emit(tc, d, out_h, uscr)
    nc.compile()
    return nc


def _emit(tc, d, out_h, uscr):
    nc = tc.nc
    ctx = contextlib.ExitStack()
    const = ctx.enter_context(tc.tile_pool(name="const", bufs=1))
    persist = ctx.enter_context(tc.tile_pool(name="persist", bufs=1))
    ypool = ctx.enter_context(tc.tile_pool(name="ypool", bufs=1))
    stage = ctx.enter_context(tc.tile_pool(name="stage", bufs=3))
    # phase-2/3 pipeline tiles: 4 pair-tile tails in flight, so the h/u
    # prefetch and the tanh of tile t+1 never wait on tile t's drain
    p3p = ctx.enter_context(tc.tile_pool(name="p3p", bufs=4))
    ctp = ctx.enter_context(tc.tile_pool(name="ctp", bufs=4))
    dsp = ctx.enter_context(tc.tile_pool(name="dsp", bufs=1))
    psacc = ctx.enter_context(tc.tile_pool(name="psacc", bufs=4, space="PSUM"))
    psscr = ctx.enter_context(tc.tile_pool(name="psscr", bufs=4, space="PSUM"))

    # ---- constants / weights in SBUF ----
    def cload(name, shape=None, dt=None):
        ap = d[name]
        t = const.tile(list(ap.shape) if shape is None else shape,
                       ap.dtype if dt is None else dt, name=f"c_{name}")
        nc.sync.dma_start(out=t, in_=ap)
        return t

    # DMA order matters: the startup critical path is
    #   fsT/wqk (prelude) and xT/kblob (y-gen), then adj slice t=0.
    fsT = cload("fsT")
    wqk = cload("wqk")
    wq, wk = wqk[:, 0:U], wqk[:, U:2 * U]
    kblob = cload("kblob")
    kkall = kblob[:, 0:384]
    kk0 = kblob[:, 384:512]
    kcall = kblob[:, 512:704]
    kc0 = kblob[:, 704:768]

    xTall = persist.tile([FROWS, BL, N], BF16, name="xTall", tag="xTall")
    # two DMAs so y-gen for b=0/1 can start before b=2/3 lands
    nc.sync.dma_start(out=xTall[:, 0:2, :],
                      in_=d["xT"][0:2].rearrange("b f n -> f b n"))
    nc.sync.dma_start(out=xTall[:, 2:4, :],
                      in_=d["xT"][2:4].rearrange("b f n -> f b n"))
    xT = [xTall[:, b, :] for b in range(BL)]

    ws1 = cload("ws1")
    ws2 = cload("ws2")
    bs1v = cload("bs1v")
    bruh = cload("bruh")        # pre-halved host-side for the tanh trick
    bc2 = cload("bc2")
    bs2v = cload("bs2v")

    # ---- resident adjacency slices (fp8, reused by both phases) ----
    a1t = [persist.tile([NBW, NB, NTW], F8, name=f"a1T_{t}", tag=f"a1T_{t}")
           for t in range(NT)]
    a2t = [persist.tile([NBW, NB, NTW], F8, name=f"a2T_{t}", tag=f"a2T_{t}")
           for t in range(NT)]
    for t in range(NT):
        sl = slice(t * NTW, (t + 1) * NTW)
        for name, lst in (("a1T", a1t), ("a2T", a2t)):
            nc.sync.dma_start(
                out=lst[t],
                in_=d[name][:, sl].rearrange("(j p) w -> p j w", p=NBW))

    # k-plane stride of a DoubleRow LDWEIGHTS AP must be 16-byte aligned,
    # so pad the ones column out to 16 bytes per plane
    ones8f = const.tile([NBW, 2, 16], F8, name="ones8")
    nc.vector.memset(ones8f, 1.0)
    ones8 = ones8f[:, :, 0:1]
    ones_row = const.tile([1, NBW], BF16, name="ones_row")
    nc.vector.memset(ones_row, C_ADJ)

    QT = persist.tile([U // 2, 2, N], F8, name="QT", tag="QT")
    KT = persist.tile([U // 2, 2, N], F8, name="KT", tag="KT")
    s_row = persist.tile([1, N], BF16, name="s_row", tag="s_row")
    rdbc = [persist.tile([NBW, NTW], BF16, name=f"rdbc{t}", tag=f"rdbc{t}")
            for t in range(NT)]
    ets = [persist.tile([NBW, NB, NTW], F8, name=f"et{t}", tag=f"et{t}")
           for t in range(NT)]

    # ---- prelude: K^T first (e-gen of t=0 reads all of K), then Q^T
    # (fp8 split-u packing), then s (not needed until the first dchain) ----
    for w, qt in ((wk, KT), (wq, QT)):
        for t in range(NT):
            sl = slice(t * NTW, (t + 1) * NTW)
            for half in range(2):
                pq = psscr.tile([U // 2, NTW], F32, name="pq", tag="scr")
                nc.tensor.matmul(pq, w[:, half * 32:(half + 1) * 32],
                                 fsT[0:FD, sl], start=True, stop=True)
                # alternate engines; both have startup slack
                if half:
                    nc.scalar.activation(qt[:, half, sl], pq, AF.Relu)
                else:
                    nc.vector.tensor_scalar_max(qt[:, half, sl], pq, 0.0)
    for t in range(NT):
        sl = slice(t * NTW, (t + 1) * NTW)
        ps1 = psscr.tile([U, NTW], F32, name="ps1", tag="scr")
        nc.tensor.matmul(ps1, ws1, fsT[:, sl], start=True, stop=True)
        s1t = stage.tile([U, NTW], BF16, name="s1t", tag="s1t")
        nc.scalar.activation(s1t, ps1, AF.Relu, bias=bs1v)
        ps2 = psscr.tile([1, NTW], F32, name="ps2", tag="scr")
        nc.tensor.matmul(ps2, ws2, s1t, start=True, stop=True)
        nc.scalar.activation(s_row[:, sl], ps2, AF.Relu, bias=bs2v)

    # ---- Y tiles: Y[m,b] = C_Y * X_b @ [k_r[m]|k_u[m]], stored fp8
    # [128, NB, 3, 128]: [node%128, node//128, m, u'] ----
    y = [ypool.tile([NBW, NB, 3, 2 * U], F8, name=f"y_{b}", tag=f"y{b}")
         for b in range(BL)]

    _eng = [0]

    def evac(out_ap, in_ap, dve_share=1):
        """PSUM evacuation, rotating DVE / ACT to balance load
        (`dve_share` DVE copies per ACT copy; -1 = DVE only, 0 = ACT
        only)."""
        if dve_share < 0:
            nc.vector.tensor_copy(out_ap, in_ap)
            return
        _eng[0] = (_eng[0] + 1) % (dve_share + 1)
        if _eng[0]:
            nc.vector.tensor_copy(out_ap, in_ap)
        else:
            nc.scalar.activation(out_ap, in_ap, AF.Copy)

    def ygen_thunks(b, dve_share=1):
        def mk(j):
            def f():
                nsl = slice(j * NBW, (j + 1) * NBW)
                py = psscr.tile([NBW, 3 * 2 * U], F32, name="py", tag="scr")
                nc.tensor.matmul(py, xT[b][:, nsl], kkall, start=True,
                                 stop=True)
                evac(y[b][:, j, :, :], py.rearrange("p (m u) -> p m u", m=3),
                     dve_share=dve_share)
            return f
        return [mk(j) for j in range(NB)]

    # y-gen for b=0/1, with tile-0's e^T generation interleaved so et[0]
    # is ready before the t-loop needs it (every later tile's e^T is
    # generated one tile ahead inside agroups, keeping the ACT exp burst
    # off the critical path)
    def e_thunks(t, et):
        """e^T[:, t] = exp(K Q^T / 8) into et (fp8), one node-block per
        call."""
        sl = slice(t * NTW, (t + 1) * NTW)

        def mk(j):
            def f():
                pe = psscr.tile([NBW, NTW], F32, name="pe", tag="scr")
                nc.tensor.matmul(pe, KT[:, :, j * NBW:(j + 1) * NBW],
                                 QT[:, :, sl], start=True, stop=True,
                                 perf_mode=DR)
                nc.scalar.activation(et[:, j, :], pe, AF.Exp, scale=0.125)
            return f
        return [mk(j) for j in range(NB)]

    def emit_dsum(et, pd):
        for j in range(NSB):
            nc.tensor.matmul(pd, ones8, et[:, 2 * j:2 * j + 2, :],
                             start=(j == 0), stop=(j == NSB - 1),
                             perf_mode=DR)

    def interleave(main, extra, ratio=2):
        """Emit `ratio` thunks from main per one from extra."""
        mi = ei = 0
        while mi < len(main) or ei < len(extra):
            for _ in range(ratio):
                if mi < len(main):
                    main[mi](); mi += 1
            if ei < len(extra):
                extra[ei](); ei += 1

    # y-gen for b=0/1, with tile-0's e^T generation interleaved so et[0]
    # is ready before the t-loop needs it (every later tile's e^T is
    # generated one tile ahead inside agroups, keeping the ACT exp burst
    # off the critical path)
    interleave(ygen_thunks(0, dve_share=-1) + ygen_thunks(1, dve_share=-1),
               e_thunks(0, ets[0]), ratio=2)

    # =================== phase 1: r & u gates ===================
    def a_thunks1(b, t, sl, pa):
        th = [lambda: nc.tensor.matmul(pa, kk0, xT[b][:, sl],
                                       start=True, stop=False)]
        for m, asl in ((0, a1t[t]), (1, a2t[t])):
            for j in range(NSB):
                def f(m=m, asl=asl, j=j):
                    nc.tensor.matmul(pa, y[b][:, 2 * j:2 * j + 2, m, :],
                                     asl[:, 2 * j:2 * j + 2, :],
                                     start=False,
                                     stop=(m == 1 and j == NSB - 1),
                                     perf_mode=DR)
                th.append(f)
        return th

    def agroup1(b, t, sl, extra=(), ratio=2):
        pa = psacc.tile([128, NTW], F32, name="pa", tag="acc")
        interleave(a_thunks1(b, t, sl, pa), list(extra), ratio=ratio)
        return pa

    def pbgroup(b, t, et, ytile=None, m=2):
        """The e-support contraction (PE only, so it can be emitted away
        from the DVE/ACT finish chain)."""
        yt = y[b] if ytile is None else ytile
        pb = psacc.tile([128, NTW], F32, name="pb", tag="acc")
        for j in range(NSB):
            nc.tensor.matmul(pb, yt[:, 2 * j:2 * j + 2, m, :],
                             et[:, 2 * j:2 * j + 2, :],
                             start=(j == 0), stop=(j == NSB - 1),
                             perf_mode=DR)
        return pb

    def bfinish1(b, t, sl, pa, pb):
        tmp = stage.tile([128, NTW], F32, name="tmp", tag="tmp")
        nc.vector.tensor_mul(tmp, pb, rdbc[t])
        ssum = stage.tile([128, NTW], F32, name="ssum", tag="ssum")
        nc.vector.tensor_add(ssum, pa, tmp)
        # sigmoid(z) = 0.5*(1 + tanh(z/2)): Tanh lives in the same ACT
        # function table as Exp/Copy, so phase 1 never reloads the table
        # (Sigmoid does not; a reload is 1283ns). sig holds th = 2r-1 /
        # 2u-1; the affine fixups are folded downstream.
        sig = stage.tile([128, NTW], F32, name="sig", tag="sig")
        nc.scalar.activation(sig, ssum, AF.Tanh, scale=SCL / 2, bias=bruh)
        # rh' = (1+th)*h = 2*(r*h) -> rows 0:64 in place as th*h + h
        # (the 0.5 is folded into the state rows of kc0/kcall host-side);
        # two plain tensor-tensor ops on the otherwise idle GpSimd.
        # thu -> DRAM scratch.
        nc.vector.scalar_tensor_tensor(
            xT[b][0:U, sl], sig[0:U, :], 1.0, xT[b][0:U, sl],
            op0=mybir.AluOpType.add, op1=mybir.AluOpType.mult)
        nc.scalar.dma_start(out=uscr[b][:, sl], in_=sig[U:128, :])

    def ycgen_thunks(yct, p, dve_share=1):
        th = []
        for half in range(2):
            b = 2 * p + half
            usl = slice(half * U, (half + 1) * U)
            for j in range(NB):
                def f(b=b, usl=usl, j=j, yct=yct):
                    nsl = slice(j * NBW, (j + 1) * NBW)
                    pyc = psscr.tile([NBW, 3 * U], F32, name="pyc", tag="scr")
                    nc.tensor.matmul(pyc, xT[b][:, nsl], kcall,
                                     start=True, stop=True)
                    evac(yct[:, j, :, usl],
                         pyc.rearrange("p (m u) -> p m u", m=3),
                         dve_share=dve_share)
                th.append(f)
        return th

    yc = [None, None]

    # Per-tile emission order keeps PE fed continuously:
    #   ag0(+e) ag1(+e) ag2 pb0 d8 [dchain] ag3 fin0 pb1 fin1 pb2 fin2
    #   pb3 fin3
    # The PSUM "acc" live set never exceeds 5 (pa0-3 + one pb).
    for t in range(NT):
        sl = slice(t * NTW, (t + 1) * NTW)
        et = ets[t]
        # generate NEXT tile's e^T inside this tile's agroups (this
        # tile's was generated one tile ago)
        eth = e_thunks(t + 1, ets[t + 1]) if t + 1 < NT else []
        if t == 0:
            y2g, y3g = ygen_thunks(2, dve_share=2), ygen_thunks(3, dve_share=2)
            pa0 = agroup1(0, t, sl, eth[0:8], ratio=1)
        else:
            pa0 = agroup1(0, t, sl, eth[0:8], ratio=2)

        # d/rdbc right after ag0 — et[t] was generated one tile ago, so
        # the serial d8 -> dsb -> 1/d -> broadcast chain hides under the
        # remaining agroups instead of stalling the finishes
        pd = psscr.tile([1, NTW], F32, name="pd", tag="scr")
        emit_dsum(et, pd)
        # d[n] = s[n] + colsum(e^T)[n]; rdbc[t][p, n] = C_ADJ / d[n]
        dsb = dsp.tile([1, NTW], F32, name="dsb", tag="dsb")
        nc.vector.tensor_add(dsb, pd, s_row[:, sl])
        rds = dsp.tile([1, NTW], BF16, name="rds", tag="rds")
        with nc.allow_low_precision(reason="1/d feeds an fp8-noise-dominated "
                                    "term; bf16 is plenty"):
            nc.vector.reciprocal(rds, dsb)
        pr = psscr.tile([128, NTW], F32, name="pr", tag="scr")
        nc.tensor.matmul(pr, ones_row, rds, start=True, stop=True)
        nc.vector.tensor_copy(rdbc[t], pr)

        if t == 0:
            pa1 = agroup1(1, t, sl, eth[8:16] + y2g, ratio=1)
        else:
            pa1 = agroup1(1, t, sl, eth[8:16], ratio=2)
        pb0 = pbgroup(0, t, et)
        bfinish1(0, t, sl, pa0, pb0)
        if t == 0:
            pa2 = agroup1(2, t, sl, y3g, ratio=1)
        else:
            pa2 = agroup1(2, t, sl)
        pb1 = pbgroup(1, t, et)
        bfinish1(1, t, sl, pa1, pb1)
        pa3 = agroup1(3, t, sl)
        if t == NT - 1:
            # yc[0] gen needs the rh updates of b0/b1 (fin0/fin1) above;
            # interleave it with the remaining e-support contractions
            yc[0] = ypool.tile([NBW, NB, 3, 2 * U], F8, name="yc_0", tag="y0")
            yc0g = ycgen_thunks(yc[0], 0)
            pb23 = []
            th23 = [lambda: pb23.append(pbgroup(2, t, et)),
                    lambda: pb23.append(pbgroup(3, t, et))]
            interleave(th23, yc0g, ratio=1)
            bfinish1(2, t, sl, pa2, pb23[0])
            bfinish1(3, t, sl, pa3, pb23[1])
        else:
            for b, pa in ((2, pa2), (3, pa3)):
                pb = pbgroup(b, t, et)
                bfinish1(b, t, sl, pa, pb)

    # =================== phase 2+3: c gate & h_new ===================
    # Yc[pair] = C_Y * [Xc_b0 @ kc[m] | Xc_b1 @ kc[m]], fp8
    # (yc[0] was already generated inside phase-1's final A-group; yc[1]
    # is interleaved into phase-2 t=0's first A-group below, on the
    # mostly idle ACT)
    yc[1] = ypool.tile([NBW, NB, 3, 2 * U], F8, name="yc_1", tag="y1")
    yc1g = ycgen_thunks(yc[1], 1)

    for t in range(NT):
        sl = slice(t * NTW, (t + 1) * NTW)
        et = ets[t]
        a1, a2 = a1t[t], a2t[t]
        # prefetch h and u for the tail chains, one pair-merged DMA each
        hps, ups = [], []
        for p in range(BL // 2):
            hp = p3p.tile([128, NTW], F32, name="hp", tag="hp")
            up = p3p.tile([128, NTW], F32, name="up", tag="up")
            psl = slice(2 * p, 2 * p + 2)
            # the SBUF side stays a plain [128, 512] AP: its (partition,
            # col) iteration order already matches the DRAM side's
            # (b, u, w) — splitting the SBUF partition dim via rearrange
            # produces corrupt flat-offset strides
            nc.sync.dma_start(out=hp, in_=d["hT"][psl, :, sl])
            nc.sync.dma_start(out=up, in_=uscr[psl, :, sl])
            # uscr holds thu = 2u-1; recover u = 0.5*thu + 0.5 in place,
            # off the tail's critical path (the chain below then uses
            # only plain tensor-tensor ops)
            nc.vector.tensor_scalar(up, up, 1.0, 0.5,
                                    op0=mybir.AluOpType.add,
                                    op1=mybir.AluOpType.mult)
            hps.append(hp)
            ups.append(up)
        for p in range(BL // 2):
            b0, b1 = 2 * p, 2 * p + 1
            pa = psacc.tile([128, NTW], F32, name="pa2", tag="acc")
            th = [lambda pa=pa, p=p: nc.tensor.matmul(
                      pa, yc[p][:, 0:2, 0, :], a1[:, 0:2, :],
                      start=True, stop=False, perf_mode=DR),
                  lambda pa=pa, b0=b0: nc.tensor.matmul(
                      pa[0:U, :], kc0, xT[b0][:, sl], start=False, stop=False),
                  lambda pa=pa, b1=b1: nc.tensor.matmul(
                      pa[U:128, :], kc0, xT[b1][:, sl], start=False,
                      stop=False)]
            for m, asl in ((0, a1), (1, a2)):
                for j in range(NSB):
                    if m == 0 and j == 0:
                        continue
                    def f(pa=pa, p=p, m=m, asl=asl, j=j):
                        nc.tensor.matmul(pa, yc[p][:, 2 * j:2 * j + 2, m, :],
                                         asl[:, 2 * j:2 * j + 2, :],
                                         start=False,
                                         stop=(m == 1 and j == NSB - 1),
                                         perf_mode=DR)
                    th.append(f)
            hp, up = hps[p], ups[p]
            pb = pbgroup(p, t, et, ytile=yc[p])
            if t == 0 and p == 0:
                interleave(th, yc1g, ratio=1)
            else:
                for f in th:
                    f()
            tmp = stage.tile([128, NTW], F32, name="tmp2", tag="tmp")
            ct = ctp.tile([128, NTW], F32, name="ct", tag="ct")
            ssum = stage.tile([128, NTW], F32, name="ssum2", tag="ssum")
            # run the gate + h_new chain in column halves so the DVE / ACT
            # / GpSimd / DMA stages pipeline; the elementwise h_new chain
            # (h_new = c + u*(h - c), in place on hp) runs on the otherwise
            # idle GpSimd except on the final tile, where its latency is
            # exposed and DVE (2x f32 SBUF mode) drains faster.
            # the h_new chain stays OFF the DVE queue (in-order: a chain op
            # waiting on tanh would block the next pair's tmp/ssum) except
            # on the final tile, where DVE has nothing left queued and
            # drains faster than GpSimd
            veng = nc.vector if t == NT - 1 else nc.gpsimd
            for c0 in range(0, NTW, NTW // 2):
                cs = slice(c0, c0 + NTW // 2)
                nc.vector.tensor_mul(tmp[:, cs], pb[:, cs], rdbc[t][:, cs])
                nc.vector.tensor_add(ssum[:, cs], pa[:, cs], tmp[:, cs])
                nc.scalar.activation(ct[:, cs], ssum[:, cs], AF.Tanh,
                                     scale=SCL, bias=bc2)
                # h_new = c + u*(h - c), in place on hp
                veng.tensor_sub(hp[:, cs], hp[:, cs], ct[:, cs])
                veng.tensor_mul(hp[:, cs], up[:, cs], hp[:, cs])
                veng.tensor_add(hp[:, cs], hp[:, cs], ct[:, cs])
            # one pair-merged output DMA per (pair, tile)
            nc.scalar.dma_start(out=out_h[b0:b1 + 1, :, sl], in_=hp)

    ctx.close()


_CACHE = {}


def _get_program():
    if "nc" not in _CACHE:
        _CACHE["nc"] = _build_program()
    return _CACHE["nc"]


def _prep_inputs(inputs, h_prev, adj1, adj2, feat, SE, Wq, Wk, Ws1, bs1, Ws2,
                 bs2, r_kernel, r_bias, u_kernel, u_bias, c_kernel, c_bias):
    bf = ml_dtypes.bfloat16
    f8 = ml_dtypes.float8_e4m3
    f32 = np.float32
    perm = list(range(DIN, FROWS)) + list(range(DIN))  # [h(64); inputs(2)]

    h3 = np.asarray(h_prev, f32).reshape(B, N, U)
    hT = np.ascontiguousarray(h3.transpose(0, 2, 1))            # [B, U, N]
    inT = np.asarray(inputs, f32).transpose(0, 2, 1)            # [B, DIN, N]
    xT = np.concatenate([hT, inT], axis=1).astype(bf)           # [B, 66, N]

    rk = np.asarray(r_kernel, f32)[:, perm, :]
    uk = np.asarray(u_kernel, f32)[:, perm, :]
    ck = np.asarray(c_kernel, f32)[:, perm, :]
    # the device keeps rh' = 2*(r*h) in the state rows of x_cat_c (tanh
    # trick), so halve the state rows of the c kernels to compensate
    ck = ck.copy()
    ck[:, 0:U, :] *= 0.5
    kkall = (np.concatenate(
        [np.concatenate([rk[m], uk[m]], axis=1) for m in (1, 2, 3)],
        axis=1) * C_Y).astype(bf)                               # [66, 384]
    kk0 = (np.concatenate([rk[0], uk[0]], axis=1) * G).astype(bf)
    kcall = (np.concatenate([ck[1], ck[2], ck[3]], axis=1) * C_Y).astype(bf)
    kc0 = (ck[0] * G).astype(bf)

    shared = {
        "a1T": np.ascontiguousarray(
            np.asarray(adj1, f32).T * C_ADJ).astype(f8),
        "a2T": np.ascontiguousarray(
            np.asarray(adj2, f32).T * C_ADJ).astype(f8),
        "fsT": np.ascontiguousarray(
            np.concatenate([np.asarray(feat, f32).T, np.asarray(SE, f32).T],
                           axis=0)).astype(bf),
        "wqk": np.concatenate([np.asarray(Wq, f32), np.asarray(Wk, f32)],
                              axis=1).astype(bf),
        "ws1": np.asarray(Ws1, f32).astype(bf),
        "ws2": np.asarray(Ws2, f32).reshape(U, 1).astype(bf),
        "kblob": np.concatenate([kkall, kk0, kcall, kc0], axis=1),
        "bs1v": np.asarray(bs1, f32).reshape(U, 1),
        "bruh": 0.5 * np.concatenate(
            [np.asarray(r_bias, f32).mean(0),
             np.asarray(u_bias, f32).mean(0)]).reshape(-1, 1),
        "bc2": np.tile(np.asarray(c_bias, f32).mean(0), 2).reshape(-1, 1),
        "bs2v": np.asarray(bs2, f32).reshape(1, 1),
    }
    in_maps = []
    for c in range(NCORES):
        bsl = slice(c * BL, (c + 1) * BL)
        m = dict(shared)
        m["xT"] = np.ascontiguousarray(xT[bsl])
        m["hT"] = np.ascontiguousarray(hT[bsl])
        in_maps.append(m)
    return in_maps


def kernel(**inputs):
    os.environ.setdefault("NEURON_RT_RESET_CORES", "1")
    nc = _get_program()
    in_maps = _prep_inputs(**inputs)
    res = None
    err = None
    for _ in range(2):
        try:
            res = run_bass_kernel_spmd(nc, in_maps, list(range(NCORES)))
            break
        except Exception as e:  # e.g. a wedged device; retry once
            err = e
    if res is None:
        raise err
    outs = []
    for c in range(NCORES):
        o = res.results[c]["out"]                     # [BL, U, N] f32
        outs.append(o.transpose(0, 2, 1).reshape(BL, N * U))
    return np.concatenate(outs, axis=0).astype(np.float32)


# revision 100
# speedup vs baseline: 2.4015x; 1.0218x over previous
"""MFGCGRU (graph-conv GRU cell) Trainium2 kernel.

Strategy: data-parallel over batch B=32 across 8 NeuronCores (4 batches
per core). All NxN supports replicated per core and resident in SBUF.

The dominant work — applying the three supports (adj1, adj2, e-attn) to
the per-gate projections Y_m = X @ k_m — runs as fp8(e4m3) matmuls in
DoubleRow perf mode: operands are packed [128, 2, F] so each PE pass
contracts 256 source nodes at half a cycle per output column.

Scaling bookkeeping (so fp8 operands sit in e4m3's sweet spot):
  - adjacencies are sent as 64*S^T            (C_ADJ = 64)
  - support kernels are folded with x16       (C_Y = 16, Y' = 16*X@k)
  - identity kernels are folded with x1024    (G = C_ADJ*C_Y)
  - the e-term normalizer is rdbc = 64/d, applied to the (x16-scaled)
    e-contribution PSUM, matching the x1024 of the adj/identity terms
  - gate activations then use ACT scale = 0.25/1024 (the /M fold)

The attention support is built unnormalized as e^T = exp(K Q^T / 8)
from fp8 Q/K packed [32, 2, N] (u = plane*32 + p), stored fp8 and kept
resident for both the r/u and the c passes; its row-normalizer is
applied to the e-contribution via a second PSUM accumulator.
"""

import contextlib
import os

import numpy as np
import ml_dtypes

import concourse.bass as bass
import concourse.bacc as bacc
import concourse.tile as tile
from concourse import mybir
from concourse.bass_utils import run_bass_kernel_spmd

F32 = mybir.dt.float32
BF16 = mybir.dt.bfloat16
F8 = mybir.dt.float8e4
AF = mybir.ActivationFunctionType
DR = mybir.MatmulPerfMode.DoubleRow

B, N, DIN, U, FD, SD = 32, 2048, 2, 64, 32, 64
NCORES = 8
BL = B // NCORES          # batches per core
NTW = 512                 # n-tile width
NT = N // NTW             # 4 n-tiles
NBW = 128                 # node-block width
NB = N // NBW             # 16 node blocks
NSB = NB // 2             # 8 node super-blocks (256 nodes, DoubleRow)
FROWS = DIN + U           # 66

C_ADJ = 64.0              # host scale on S^T before e4m3 cast
C_Y = 16.0                # host scale folded into support kernels
G = C_ADJ * C_Y           # net scale of the PSUM accumulators
SCL = 0.25 / G            # ACT scale for the gate activations (incl /M)


def _build_program():
    nc = bacc.Bacc("TRN2", debug=False, num_devices=NCORES)

    d = {}

    def din(name, shape, dt):
        d[name] = nc.dram_tensor(name, shape, dt, kind="ExternalInput").ap()

    din("xT", [BL, FROWS, N], BF16)
    din("hT", [BL, U, N], F32)
    din("a1T", [N, N], F8)
    din("a2T", [N, N], F8)
    din("fsT", [FD + SD, N], BF16)
    # kernels are packed into one tensor: each dma_start holds the
    # (single, serialized) HWDGE device ~630ns regardless of size
    din("wqk", [FD, 2 * U], BF16)            # [wq | wk]
    din("ws1", [FD + SD, U], BF16)
    din("ws2", [U, 1], BF16)
    din("kblob", [FROWS, 768], BF16)         # [kkall|kk0|kcall|kc0]
    din("bs1v", [U, 1], F32)
    din("bruh", [2 * U, 1], F32)
    din("bc2", [2 * U, 1], F32)
    din("bs2v", [1, 1], F32)
    out_h = nc.dram_tensor("out", [BL, U, N], F32, kind="ExternalOutput").ap()
    uscr = nc.dram_tensor("uscr", [BL, U, N], F32).ap()

    with tile.TileContext(nc) as tc:
        _emit(tc, d, out_h, uscr)
    nc.compile()
    return nc


def _emit(tc, d, out_h, uscr):
    nc = tc.nc
    ctx = contextlib.ExitStack()
    const = ctx.enter_context(tc.tile_pool(name="const", bufs=1))
    persist = ctx.enter_context(tc.tile_pool(name="persist", bufs=1))
    ypool = ctx.enter_context(tc.tile_pool(name="ypool", bufs=1))
    stage = ctx.enter_context(tc.tile_pool(name="stage", bufs=3))
    # phase-2/3 pipeline tiles: 4 pair-tile tails in flight, so the h/u
    # prefetch and the tanh of tile t+1 never wait on tile t's drain
    p3p = ctx.enter_context(tc.tile_pool(name="p3p", bufs=4))
    ctp = ctx.enter_context(tc.tile_pool(name="ctp", bufs=4))
    dsp = ctx.enter_context(tc.tile_pool(name="dsp", bufs=1))
    psacc = ctx.enter_context(tc.tile_pool(name="psacc", bufs=4, space="PSUM"))
    psscr = ctx.enter_context(tc.tile_pool(name="psscr", bufs=4, space="PSUM"))

    # ---- constants / weights in SBUF ----
    def cload(name, shape=None, dt=None):
        ap = d[name]
        t = const.tile(list(ap.shape) if shape is None else shape,
                       ap.dtype if dt is None else dt, name=f"c_{name}")
        nc.sync.dma_start(out=t, in_=ap)
        return t

    # DMA order matters: the startup critical path is
    #   fsT/wqk (prelude) and xT/kblob (y-gen), then adj slice t=0.
    fsT = const.tile([FD + SD, N], BF16, name="c_fsT")
    nc.sync.dma_start(out=fsT[:, 0:NTW], in_=d["fsT"][:, 0:NTW])
    wqk = cload("wqk")
    nc.sync.dma_start(out=fsT[:, NTW:], in_=d["fsT"][:, NTW:])
    wq, wk = wqk[:, 0:U], wqk[:, U:2 * U]
    kblob = cload("kblob")
    kkall = kblob[:, 0:384]
    kk0 = kblob[:, 384:512]
    kcall = kblob[:, 512:704]
    kc0 = kblob[:, 704:768]

    xTall = persist.tile([FROWS, BL, N], BF16, name="xTall", tag="xTall")
    # two DMAs so y-gen for b=0/1 can start before b=2/3 lands
    nc.sync.dma_start(out=xTall[:, 0:2, :],
                      in_=d["xT"][0:2].rearrange("b f n -> f b n"))
    nc.sync.dma_start(out=xTall[:, 2:4, :],
                      in_=d["xT"][2:4].rearrange("b f n -> f b n"))
    xT = [xTall[:, b, :] for b in range(BL)]

    ws1 = cload("ws1")
    ws2 = cload("ws2")
    bs1v = cload("bs1v")
    bruh = cload("bruh")        # pre-halved host-side for the tanh trick
    bc2 = cload("bc2")
    bs2v = cload("bs2v")

    # ---- resident adjacency slices (fp8, reused by both phases) ----
    a1t = [persist.tile([NBW, NB, NTW], F8, name=f"a1T_{t}", tag=f"a1T_{t}")
           for t in range(NT)]
    a2t = [persist.tile([NBW, NB, NTW], F8, name=f"a2T_{t}", tag=f"a2T_{t}")
           for t in range(NT)]
    for t in range(NT):
        sl = slice(t * NTW, (t + 1) * NTW)
        for name, lst in (("a1T", a1t), ("a2T", a2t)):
            nc.sync.dma_start(
                out=lst[t],
                in_=d[name][:, sl].rearrange("(j p) w -> p j w", p=NBW))

    # k-plane stride of a DoubleRow LDWEIGHTS AP must be 16-byte aligned,
    # so pad the ones column out to 16 bytes per plane
    ones8f = const.tile([NBW, 2, 16], F8, name="ones8")
    nc.vector.memset(ones8f, 1.0)
    ones8 = ones8f[:, :, 0:1]
    ones_row = const.tile([1, NBW], BF16, name="ones_row")
    nc.vector.memset(ones_row, C_ADJ)

    QT = persist.tile([U // 2, 2, N], F8, name="QT", tag="QT")
    KT = persist.tile([U // 2, 2, N], F8, name="KT", tag="KT")
    s_row = persist.tile([1, N], BF16, name="s_row", tag="s_row")
    rdbc = [persist.tile([NBW, NTW], BF16, name=f"rdbc{t}", tag=f"rdbc{t}")
            for t in range(NT)]
    ets = [persist.tile([NBW, NB, NTW], F8, name=f"et{t}", tag=f"et{t}")
           for t in range(NT)]

    # ---- prelude: K^T first (e-gen of t=0 reads all of K), then Q^T
    # (fp8 split-u packing), then s (not needed until the first dchain) ----
    for w, qt in ((wk, KT), (wq, QT)):
        for t in range(NT):
            sl = slice(t * NTW, (t + 1) * NTW)
            for half in range(2):
                pq = psscr.tile([U // 2, NTW], F32, name="pq", tag="scr")
                nc.tensor.matmul(pq, w[:, half * 32:(half + 1) * 32],
                                 fsT[0:FD, sl], start=True, stop=True)
                # alternate engines; both have startup slack
                if half:
                    nc.scalar.activation(qt[:, half, sl], pq, AF.Relu)
                else:
                    nc.vector.tensor_scalar_max(qt[:, half, sl], pq, 0.0)
    for t in range(NT):
        sl = slice(t * NTW, (t + 1) * NTW)
        ps1 = psscr.tile([U, NTW], F32, name="ps1", tag="scr")
        nc.tensor.matmul(ps1, ws1, fsT[:, sl], start=True, stop=True)
        s1t = stage.tile([U, NTW], BF16, name="s1t", tag="s1t")
        nc.scalar.activation(s1t, ps1, AF.Relu, bias=bs1v)
        ps2 = psscr.tile([1, NTW], F32, name="ps2", tag="scr")
        nc.tensor.matmul(ps2, ws2, s1t, start=True, stop=True)
        nc.scalar.activation(s_row[:, sl], ps2, AF.Relu, bias=bs2v)

    # ---- Y tiles: Y[m,b] = C_Y * X_b @ [k_r[m]|k_u[m]], stored fp8
    # [128, NB, 3, 128]: [node%128, node//128, m, u'] ----
    y = [ypool.tile([NBW, NB, 3, 2 * U], F8, name=f"y_{b}", tag=f"y{b}")
         for b in range(BL)]

    _eng = [0]

    def evac(out_ap, in_ap, dve_share=1):
        """PSUM evacuation, rotating DVE / ACT to balance load
        (`dve_share` DVE copies per ACT copy; -1 = DVE only, 0 = ACT
        only)."""
        if dve_share < 0:
            nc.vector.tensor_copy(out_ap, in_ap)
            return
        _eng[0] = (_eng[0] + 1) % (dve_share + 1)
        if _eng[0]:
            nc.vector.tensor_copy(out_ap, in_ap)
        else:
            nc.scalar.activation(out_ap, in_ap, AF.Copy)

    def ygen_thunks(b, dve_share=1):
        def mk(j):
            def f():
                nsl = slice(j * NBW, (j + 1) * NBW)
                py = psscr.tile([NBW, 3 * 2 * U], F32, name="py", tag="scr")
                nc.tensor.matmul(py, xT[b][:, nsl], kkall, start=True,
                                 stop=True)
                evac(y[b][:, j, :, :], py.rearrange("p (m u) -> p m u", m=3),
                     dve_share=dve_share)
            return f
        return [mk(j) for j in range(NB)]

    # y-gen for b=0/1, with tile-0's e^T generation interleaved so et[0]
    # is ready before the t-loop needs it (every later tile's e^T is
    # generated one tile ahead inside agroups, keeping the ACT exp burst
    # off the critical path)
    def e_thunks(t, et):
        """e^T[:, t] = exp(K Q^T / 8) into et (fp8), one node-block per
        call."""
        sl = slice(t * NTW, (t + 1) * NTW)

        def mk(j):
            def f():
                pe = psscr.tile([NBW, NTW], F32, name="pe", tag="scr")
                nc.tensor.matmul(pe, KT[:, :, j * NBW:(j + 1) * NBW],
                                 QT[:, :, sl], start=True, stop=True,
                                 perf_mode=DR)
                nc.scalar.activation(et[:, j, :], pe, AF.Exp, scale=0.125)
            return f
        return [mk(j) for j in range(NB)]

    def emit_dsum(et, pd):
        for j in range(NSB):
            nc.tensor.matmul(pd, ones8, et[:, 2 * j:2 * j + 2, :],
                             start=(j == 0), stop=(j == NSB - 1),
                             perf_mode=DR)

    def interleave(main, extra, ratio=2):
        """Emit `ratio` thunks from main per one from extra."""
        mi = ei = 0
        while mi < len(main) or ei < len(extra):
            for _ in range(ratio):
                if mi < len(main):
                    main[mi](); mi += 1
            if ei < len(extra):
                extra[ei](); ei += 1

    # y-gen for b=0/1, with tile-0's e^T generation interleaved so et[0]
    # is ready before the t-loop needs it (every later tile's e^T is
    # generated one tile ahead inside agroups, keeping the ACT exp burst
    # off the critical path)
    interleave(ygen_thunks(0, dve_share=2) + ygen_thunks(1, dve_share=2),
               e_thunks(0, ets[0]), ratio=2)

    # =================== phase 1: r & u gates ===================
    def a_thunks1(b, t, sl, pa):
        th = [lambda: nc.tensor.matmul(pa, kk0, xT[b][:, sl],
                                       start=True, stop=False)]
        for m, asl in ((0, a1t[t]), (1, a2t[t])):
            for j in range(NSB):
                def f(m=m, asl=asl, j=j):
                    nc.tensor.matmul(pa, y[b][:, 2 * j:2 * j + 2, m, :],
                                     asl[:, 2 * j:2 * j + 2, :],
                                     start=False,
                                     stop=(m == 1 and j == NSB - 1),
                                     perf_mode=DR)
                th.append(f)
        return th

    def agroup1(b, t, sl, extra=(), ratio=2):
        pa = psacc.tile([128, NTW], F32, name="pa", tag="acc")
        interleave(a_thunks1(b, t, sl, pa), list(extra), ratio=ratio)
        return pa

    def pbgroup(b, t, et, ytile=None, m=2):
        """The e-support contraction (PE only, so it can be emitted away
        from the DVE/ACT finish chain)."""
        yt = y[b] if ytile is None else ytile
        pb = psacc.tile([128, NTW], F32, name="pb", tag="acc")
        for j in range(NSB):
            nc.tensor.matmul(pb, yt[:, 2 * j:2 * j + 2, m, :],
                             et[:, 2 * j:2 * j + 2, :],
                             start=(j == 0), stop=(j == NSB - 1),
                             perf_mode=DR)
        return pb

    def bfinish1(b, t, sl, pa, pb):
        tmp = stage.tile([128, NTW], F32, name="tmp", tag="tmp")
        nc.vector.tensor_mul(tmp, pb, rdbc[t])
        ssum = stage.tile([128, NTW], F32, name="ssum", tag="ssum")
        nc.vector.tensor_add(ssum, pa, tmp)
        # sigmoid(z) = 0.5*(1 + tanh(z/2)): Tanh lives in the same ACT
        # function table as Exp/Copy, so phase 1 never reloads the table
        # (Sigmoid does not; a reload is 1283ns). sig holds th = 2r-1 /
        # 2u-1; the affine fixups are folded downstream.
        sig = stage.tile([128, NTW], F32, name="sig", tag="sig")
        nc.scalar.activation(sig, ssum, AF.Tanh, scale=SCL / 2, bias=bruh)
        # rh' = (1+th)*h = 2*(r*h) -> rows 0:64 in place as th*h + h
        # (the 0.5 is folded into the state rows of kc0/kcall host-side);
        # two plain tensor-tensor ops on the otherwise idle GpSimd.
        # thu -> DRAM scratch.
        nc.vector.scalar_tensor_tensor(
            xT[b][0:U, sl], sig[0:U, :], 1.0, xT[b][0:U, sl],
            op0=mybir.AluOpType.add, op1=mybir.AluOpType.mult)
        ueng = nc.scalar if b % 2 else nc.sync
        ueng.dma_start(out=uscr[b][:, sl], in_=sig[U:128, :])

    def ycgen_thunks(yct, p, dve_share=1):
        th = []
        for half in range(2):
            b = 2 * p + half
            usl = slice(half * U, (half + 1) * U)
            for j in range(NB):
                def f(b=b, usl=usl, j=j, yct=yct):
                    nsl = slice(j * NBW, (j + 1) * NBW)
                    pyc = psscr.tile([NBW, 3 * U], F32, name="pyc", tag="scr")
                    nc.tensor.matmul(pyc, xT[b][:, nsl], kcall,
                                     start=True, stop=True)
                    evac(yct[:, j, :, usl],
                         pyc.rearrange("p (m u) -> p m u", m=3),
                         dve_share=dve_share)
                th.append(f)
        return th

    yc = [None, None]

    # Per-tile emission order keeps PE fed continuously:
    #   ag0(+e) ag1(+e) ag2 pb0 d8 [dchain] ag3 fin0 pb1 fin1 pb2 fin2
    #   pb3 fin3
    # The PSUM "acc" live set never exceeds 5 (pa0-3 + one pb).
    for t in range(NT):
        sl = slice(t * NTW, (t + 1) * NTW)
        et = ets[t]
        # generate NEXT tile's e^T inside this tile's agroups (this
        # tile's was generated one tile ago)
        eth = e_thunks(t + 1, ets[t + 1]) if t + 1 < NT else []
        if t == 0:
            y2g, y3g = ygen_thunks(2, dve_share=2), ygen_thunks(3, dve_share=2)
            pa0 = agroup1(0, t, sl, eth[0:8], ratio=1)
        else:
            pa0 = agroup1(0, t, sl, eth[0:8], ratio=2)

        # d/rdbc right after ag0 — et[t] was generated one tile ago, so
        # the serial d8 -> dsb -> 1/d -> broadcast chain hides under the
        # remaining agroups instead of stalling the finishes
        pd = psscr.tile([1, NTW], F32, name="pd", tag="scr")
        emit_dsum(et, pd)
        # d[n] = s[n] + colsum(e^T)[n]; rdbc[t][p, n] = C_ADJ / d[n]
        dsb = dsp.tile([1, NTW], F32, name="dsb", tag="dsb")
        nc.vector.tensor_add(dsb, pd, s_row[:, sl])
        rds = dsp.tile([1, NTW], BF16, name="rds", tag="rds")
        with nc.allow_low_precision(reason="1/d feeds an fp8-noise-dominated "
                                    "term; bf16 is plenty"):
            nc.vector.reciprocal(rds, dsb)
        pr = psscr.tile([128, NTW], F32, name="pr", tag="scr")
        nc.tensor.matmul(pr, ones_row, rds, start=True, stop=True)
        nc.vector.tensor_copy(rdbc[t], pr)

        if t == 0:
            pa1 = agroup1(1, t, sl, eth[8:16] + y2g, ratio=1)
        else:
            pa1 = agroup1(1, t, sl, eth[8:16], ratio=2)
        pb0 = pbgroup(0, t, et)
        bfinish1(0, t, sl, pa0, pb0)
        if t == 0:
            pa2 = agroup1(2, t, sl, y3g, ratio=1)
        else:
            pa2 = agroup1(2, t, sl)
        pb1 = pbgroup(1, t, et)
        bfinish1(1, t, sl, pa1, pb1)
        pa3 = agroup1(3, t, sl)
        if t == NT - 1:
            # yc[0] gen needs the rh updates of b0/b1 (fin0/fin1) above;
            # interleave it with the remaining e-support contractions
            yc[0] = ypool.tile([NBW, NB, 3, 2 * U], F8, name="yc_0", tag="y0")
            yc0g = ycgen_thunks(yc[0], 0)
            pb23 = []
            th23 = [lambda: pb23.append(pbgroup(2, t, et)),
                    lambda: pb23.append(pbgroup(3, t, et))]
            interleave(th23, yc0g, ratio=1)
            bfinish1(2, t, sl, pa2, pb23[0])
            bfinish1(3, t, sl, pa3, pb23[1])
        else:
            for b, pa in ((2, pa2), (3, pa3)):
                pb = pbgroup(b, t, et)
                bfinish1(b, t, sl, pa, pb)

    # =================== phase 2+3: c gate & h_new ===================
    # Yc[pair] = C_Y * [Xc_b0 @ kc[m] | Xc_b1 @ kc[m]], fp8
    # (yc[0] was already generated inside phase-1's final A-group; yc[1]
    # is interleaved into phase-2 t=0's first A-group below, on the
    # mostly idle ACT)
    yc[1] = ypool.tile([NBW, NB, 3, 2 * U], F8, name="yc_1", tag="y1")
    yc1g = ycgen_thunks(yc[1], 1)

    for t in range(NT):
        sl = slice(t * NTW, (t + 1) * NTW)
        et = ets[t]
        a1, a2 = a1t[t], a2t[t]
        # prefetch h and u for the tail chains, one pair-merged DMA each
        hps, ups = [], []
        for p in range(BL // 2):
            hp = p3p.tile([128, NTW], F32, name="hp", tag="hp")
            up = p3p.tile([128, NTW], F32, name="up", tag="up")
            psl = slice(2 * p, 2 * p + 2)
            # the SBUF side stays a plain [128, 512] AP: its (partition,
            # col) iteration order already matches the DRAM side's
            # (b, u, w) — splitting the SBUF partition dim via rearrange
            # produces corrupt flat-offset strides
            nc.sync.dma_start(out=hp, in_=d["hT"][psl, :, sl])
            nc.sync.dma_start(out=up, in_=uscr[psl, :, sl])
            # uscr holds thu = 2u-1; recover u = 0.5*thu + 0.5 in place,
            # off the tail's critical path (the chain below then uses
            # only plain tensor-tensor ops)
            nc.vector.tensor_scalar(up, up, 1.0, 0.5,
                                    op0=mybir.AluOpType.add,
                                    op1=mybir.AluOpType.mult)
            hps.append(hp)
            ups.append(up)
        for p in range(BL // 2):
            b0, b1 = 2 * p, 2 * p + 1
            pa = psacc.tile([128, NTW], F32, name="pa2", tag="acc")
            th = [lambda pa=pa, p=p: nc.tensor.matmul(
                      pa, yc[p][:, 0:2, 0, :], a1[:, 0:2, :],
                      start=True, stop=False, perf_mode=DR),
                  lambda pa=pa, b0=b0: nc.tensor.matmul(
                      pa[0:U, :], kc0, xT[b0][:, sl], start=False, stop=False),
                  lambda pa=pa, b1=b1: nc.tensor.matmul(
                      pa[U:128, :], kc0, xT[b1][:, sl], start=False,
                      stop=False)]
            for m, asl in ((0, a1), (1, a2)):
                for j in range(NSB):
                    if m == 0 and j == 0:
                        continue
                    def f(pa=pa, p=p, m=m, asl=asl, j=j):
                        nc.tensor.matmul(pa, yc[p][:, 2 * j:2 * j + 2, m, :],
                                         asl[:, 2 * j:2 * j + 2, :],
                                         start=False,
                                         stop=(m == 1 and j == NSB - 1),
                                         perf_mode=DR)
                    th.append(f)
            hp, up = hps[p], ups[p]
            pb = pbgroup(p, t, et, ytile=yc[p])
            if t == 0 and p == 0:
                interleave(th, yc1g, ratio=1)
            else:
                for f in th:
                    f()
            tmp = stage.tile([128, NTW], F32, name="tmp2", tag="tmp")
            ct = ctp.tile([128, NTW], F32, name="ct", tag="ct")
            ssum = stage.tile([128, NTW], F32, name="ssum2", tag="ssum")
            # run the gate + h_new chain in column halves so the DVE / ACT
            # / GpSimd / DMA stages pipeline; the elementwise h_new chain
            # (h_new = c + u*(h - c), in place on hp) runs on the otherwise
            # idle GpSimd except on the final tile, where its latency is
            # exposed and DVE (2x f32 SBUF mode) drains faster.
            # the h_new chain stays OFF the DVE queue (in-order: a chain op
            # waiting on tanh would block the next pair's tmp/ssum) except
            # on the final tile, where DVE has nothing left queued and
            # drains faster than GpSimd
            veng = nc.vector if t == NT - 1 else nc.gpsimd
            for c0 in range(0, NTW, NTW // 2):
                cs = slice(c0, c0 + NTW // 2)
                nc.vector.tensor_mul(tmp[:, cs], pb[:, cs], rdbc[t][:, cs])
                nc.vector.tensor_add(ssum[:, cs], pa[:, cs], tmp[:, cs])
                nc.scalar.activation(ct[:, cs], ssum[:, cs], AF.Tanh,
                                     scale=SCL, bias=bc2)
                # h_new = c + u*(h - c), in place on hp
                veng.tensor_sub(hp[:, cs], hp[:, cs], ct[:, cs])
                veng.tensor_mul(hp[:, cs], up[:, cs], hp[:, cs])
                veng.tensor_add(hp[:, cs], hp[:, cs], ct[:, cs])
            # one pair-merged output DMA per (pair, tile)
            nc.scalar.dma_start(out=out_h[b0:b1 + 1, :, sl], in_=hp)

    ctx.close()


_CACHE = {}


def _get_program():
    if "nc" not in _CACHE:
        _CACHE["nc"] = _build_program()
    return _CACHE["nc"]


def _prep_inputs(inputs, h_prev, adj1, adj2, feat, SE, Wq, Wk, Ws1, bs1, Ws2,
                 bs2, r_kernel, r_bias, u_kernel, u_bias, c_kernel, c_bias):
    bf = ml_dtypes.bfloat16
    f8 = ml_dtypes.float8_e4m3
    f32 = np.float32
    perm = list(range(DIN, FROWS)) + list(range(DIN))  # [h(64); inputs(2)]

    h3 = np.asarray(h_prev, f32).reshape(B, N, U)
    hT = np.ascontiguousarray(h3.transpose(0, 2, 1))            # [B, U, N]
    inT = np.asarray(inputs, f32).transpose(0, 2, 1)            # [B, DIN, N]
    xT = np.concatenate([hT, inT], axis=1).astype(bf)           # [B, 66, N]

    rk = np.asarray(r_kernel, f32)[:, perm, :]
    uk = np.asarray(u_kernel, f32)[:, perm, :]
    ck = np.asarray(c_kernel, f32)[:, perm, :]
    # the device keeps rh' = 2*(r*h) in the state rows of x_cat_c (tanh
    # trick), so halve the state rows of the c kernels to compensate
    ck = ck.copy()
    ck[:, 0:U, :] *= 0.5
    kkall = (np.concatenate(
        [np.concatenate([rk[m], uk[m]], axis=1) for m in (1, 2, 3)],
        axis=1) * C_Y).astype(bf)                               # [66, 384]
    kk0 = (np.concatenate([rk[0], uk[0]], axis=1) * G).astype(bf)
    kcall = (np.concatenate([ck[1], ck[2], ck[3]], axis=1) * C_Y).astype(bf)
    kc0 = (ck[0] * G).astype(bf)

    shared = {
        "a1T": np.ascontiguousarray(
            np.asarray(adj1, f32).T * C_ADJ).astype(f8),
        "a2T": np.ascontiguousarray(
            np.asarray(adj2, f32).T * C_ADJ).astype(f8),
        "fsT": np.ascontiguousarray(
            np.concatenate([np.asarray(feat, f32).T, np.asarray(SE, f32).T],
                           axis=0)).astype(bf),
        "wqk": np.concatenate([np.asarray(Wq, f32), np.asarray(Wk, f32)],
                              axis=1).astype(bf),
        "ws1": np.asarray(Ws1, f32).astype(bf),
        "ws2": np.asarray(Ws2, f32).reshape(U, 1).astype(bf),
        "kblob": np.concatenate([kkall, kk0, kcall, kc0], axis=1),
        "bs1v": np.asarray(bs1, f32).reshape(U, 1),
        "bruh": 0.5 * np.concatenate(
            [np.asarray(r_bias, f32).mean(0),
             np.asarray(u_bias, f32).mean(0)]).reshape(-1, 1),
        "bc2": np.tile(np.asarray(c_bias, f32).mean(0), 2).reshape(-1, 1),
        "bs2v": np.asarray(bs2, f32).reshape(1, 1),
    }
    in_maps = []
    for c in range(NCORES):
        bsl = slice(c * BL, (c + 1) * BL)
        m = dict(shared)
        m["xT"] = np.ascontiguousarray(xT[bsl])
        m["hT"] = np.ascontiguousarray(hT[bsl])
        in_maps.append(m)
    return in_maps


def kernel(**inputs):
    os.environ.setdefault("NEURON_RT_RESET_CORES", "1")
    nc = _get_program()
    in_maps = _prep_inputs(**inputs)
    res = None
    err = None
    for _ in range(2):
        try:
            res = run_bass_kernel_spmd(nc, in_maps, list(range(NCORES)))
            break
        except Exception as e:  # e.g. a wedged device; retry once
            err = e
    if res is None:
        raise err
    outs = []
    for c in range(NCORES):
        o = res.results[c]["out"]                     # [BL, U, N] f32
        outs.append(o.transpose(0, 2, 1).reshape(BL, N * U))
    return np.concatenate(outs, axis=0).astype(np.float32)


# revision 109
# speedup vs baseline: 2.4241x; 1.0094x over previous
"""MFGCGRU (graph-conv GRU cell) Trainium2 kernel.

Strategy: data-parallel over batch B=32 across 8 NeuronCores (4 batches
per core). All NxN supports replicated per core and resident in SBUF.

The dominant work — applying the three supports (adj1, adj2, e-attn) to
the per-gate projections Y_m = X @ k_m — runs as fp8(e4m3) matmuls in
DoubleRow perf mode: operands are packed [128, 2, F] so each PE pass
contracts 256 source nodes at half a cycle per output column.

Scaling bookkeeping (so fp8 operands sit in e4m3's sweet spot):
  - adjacencies are sent as 64*S^T            (C_ADJ = 64)
  - support kernels are folded with x16       (C_Y = 16, Y' = 16*X@k)
  - identity kernels are folded with x1024    (G = C_ADJ*C_Y)
  - the e-term normalizer is rdbc = 64/d, applied to the (x16-scaled)
    e-contribution PSUM, matching the x1024 of the adj/identity terms
  - gate activations then use ACT scale = 0.25/1024 (the /M fold)

The attention support is built unnormalized as e^T = exp(K Q^T / 8)
from fp8 Q/K packed [32, 2, N] (u = plane*32 + p), stored fp8 and kept
resident for both the r/u and the c passes; its row-normalizer is
applied to the e-contribution via a second PSUM accumulator.
"""

import contextlib
import os

import numpy as np
import ml_dtypes

import concourse.bass as bass
import concourse.bacc as bacc
import concourse.tile as tile
from concourse import mybir
from concourse.bass_utils import run_bass_kernel_spmd

F32 = mybir.dt.float32
BF16 = mybir.dt.bfloat16
F8 = mybir.dt.float8e4
AF = mybir.ActivationFunctionType
DR = mybir.MatmulPerfMode.DoubleRow

B, N, DIN, U, FD, SD = 32, 2048, 2, 64, 32, 64
NCORES = 8
BL = B // NCORES          # batches per core
NTW = 512                 # n-tile width
NT = N // NTW             # 4 n-tiles
NBW = 128                 # node-block width
NB = N // NBW             # 16 node blocks
NSB = NB // 2             # 8 node super-blocks (256 nodes, DoubleRow)
FROWS = DIN + U           # 66

C_ADJ = 64.0              # host scale on S^T before e4m3 cast
C_Y = 16.0                # host scale folded into support kernels
G = C_ADJ * C_Y           # net scale of the PSUM accumulators
SCL = 0.25 / G            # ACT scale for the gate activations (incl /M)


def _build_program():
    nc = bacc.Bacc("TRN2", debug=False, num_devices=NCORES)

    d = {}

    def din(name, shape, dt):
        d[name] = nc.dram_tensor(name, shape, dt, kind="ExternalInput").ap()

    din("xT", [BL, FROWS, N], BF16)
    din("hT", [BL, U, N], F32)
    din("a1T", [N, N], F8)
    din("a2T", [N, N], F8)
    din("fsT", [FD + SD, N], BF16)
    # kernels are packed into one tensor: each dma_start holds the
    # (single, serialized) HWDGE device ~630ns regardless of size
    din("wqk", [FD, 2 * U], BF16)            # [wq | wk]
    din("ws1", [FD + SD, U], BF16)
    din("ws2", [U, 1], BF16)
    din("kblob", [FROWS, 768], BF16)         # [kkall|kk0|kcall|kc0]
    din("bs1v", [U, 1], F32)
    din("bruh", [2 * U, 1], F32)
    din("bc2", [2 * U, 1], F32)
    din("bs2v", [1, 1], F32)
    out_h = nc.dram_tensor("out", [BL, U, N], F32, kind="ExternalOutput").ap()
    uscr = nc.dram_tensor("uscr", [BL, U, N], F32).ap()

    with tile.TileContext(nc) as tc:
        _emit(tc, d, out_h, uscr)
    nc.compile()
    return nc


def _emit(tc, d, out_h, uscr):
    nc = tc.nc
    ctx = contextlib.ExitStack()
    const = ctx.enter_context(tc.tile_pool(name="const", bufs=1))
    persist = ctx.enter_context(tc.tile_pool(name="persist", bufs=1))
    ypool = ctx.enter_context(tc.tile_pool(name="ypool", bufs=1))
    stage = ctx.enter_context(tc.tile_pool(name="stage", bufs=3))
    # phase-2/3 pipeline tiles: 4 pair-tile tails in flight, so the h/u
    # prefetch and the tanh of tile t+1 never wait on tile t's drain
    p3p = ctx.enter_context(tc.tile_pool(name="p3p", bufs=4))
    ctp = ctx.enter_context(tc.tile_pool(name="ctp", bufs=4))
    dsp = ctx.enter_context(tc.tile_pool(name="dsp", bufs=1))
    psacc = ctx.enter_context(tc.tile_pool(name="psacc", bufs=4, space="PSUM"))
    psscr = ctx.enter_context(tc.tile_pool(name="psscr", bufs=4, space="PSUM"))

    # ---- constants / weights in SBUF ----
    def cload(name, shape=None, dt=None):
        ap = d[name]
        t = const.tile(list(ap.shape) if shape is None else shape,
                       ap.dtype if dt is None else dt, name=f"c_{name}")
        nc.sync.dma_start(out=t, in_=ap)
        return t

    # DMA order matters: the startup critical path is
    #   fsT/wqk (prelude) and xT/kblob (y-gen), then adj slice t=0.
    fsT = const.tile([FD + SD, N], BF16, name="c_fsT")
    nc.sync.dma_start(out=fsT[:, 0:NTW], in_=d["fsT"][:, 0:NTW])
    wqk = cload("wqk")
    nc.sync.dma_start(out=fsT[:, NTW:], in_=d["fsT"][:, NTW:])
    wq, wk = wqk[:, 0:U], wqk[:, U:2 * U]
    kblob = cload("kblob")
    kkall = kblob[:, 0:384]
    kk0 = kblob[:, 384:512]
    kcall = kblob[:, 512:704]
    kc0 = kblob[:, 704:768]

    xTall = persist.tile([FROWS, BL, N], BF16, name="xTall", tag="xTall")
    # two DMAs so y-gen for b=0/1 can start before b=2/3 lands
    nc.sync.dma_start(out=xTall[:, 0:2, :],
                      in_=d["xT"][0:2].rearrange("b f n -> f b n"))
    nc.sync.dma_start(out=xTall[:, 2:4, :],
                      in_=d["xT"][2:4].rearrange("b f n -> f b n"))
    xT = [xTall[:, b, :] for b in range(BL)]

    ws1 = cload("ws1")
    ws2 = cload("ws2")
    bs1v = cload("bs1v")
    bruh = cload("bruh")        # pre-halved host-side for the tanh trick
    bc2 = cload("bc2")
    bs2v = cload("bs2v")

    # ---- resident adjacency slices (fp8, reused by both phases) ----
    a1t = [persist.tile([NBW, NB, NTW], F8, name=f"a1T_{t}", tag=f"a1T_{t}")
           for t in range(NT)]
    a2t = [persist.tile([NBW, NB, NTW], F8, name=f"a2T_{t}", tag=f"a2T_{t}")
           for t in range(NT)]
    for t in range(NT):
        sl = slice(t * NTW, (t + 1) * NTW)
        for name, lst in (("a1T", a1t), ("a2T", a2t)):
            nc.sync.dma_start(
                out=lst[t],
                in_=d[name][:, sl].rearrange("(j p) w -> p j w", p=NBW))

    # k-plane stride of a DoubleRow LDWEIGHTS AP must be 16-byte aligned,
    # so pad the ones column out to 16 bytes per plane
    ones8f = const.tile([NBW, 2, 16], F8, name="ones8")
    nc.vector.memset(ones8f, 1.0)
    ones8 = ones8f[:, :, 0:1]
    ones_row = const.tile([1, NBW], BF16, name="ones_row")
    nc.vector.memset(ones_row, C_ADJ)

    QT = persist.tile([U // 2, 2, N], F8, name="QT", tag="QT")
    KT = persist.tile([U // 2, 2, N], F8, name="KT", tag="KT")
    s_row = persist.tile([1, N], BF16, name="s_row", tag="s_row")
    rdbc = [persist.tile([NBW, NTW], BF16, name=f"rdbc{t}", tag=f"rdbc{t}")
            for t in range(NT)]
    ets = [persist.tile([NBW, NB, NTW], F8, name=f"et{t}", tag=f"et{t}")
           for t in range(NT)]

    # ---- prelude: K^T first (e-gen of t=0 reads all of K), then Q^T
    # (fp8 split-u packing), then s (not needed until the first dchain) ----
    for w, qt in ((wk, KT), (wq, QT)):
        for t in range(NT):
            sl = slice(t * NTW, (t + 1) * NTW)
            for half in range(2):
                pq = psscr.tile([U // 2, NTW], F32, name="pq", tag="scr")
                nc.tensor.matmul(pq, w[:, half * 32:(half + 1) * 32],
                                 fsT[0:FD, sl], start=True, stop=True)
                # alternate engines; both have startup slack
                if half:
                    nc.scalar.activation(qt[:, half, sl], pq, AF.Relu)
                else:
                    nc.vector.tensor_scalar_max(qt[:, half, sl], pq, 0.0)
    for t in range(NT):
        sl = slice(t * NTW, (t + 1) * NTW)
        ps1 = psscr.tile([U, NTW], F32, name="ps1", tag="scr")
        nc.tensor.matmul(ps1, ws1, fsT[:, sl], start=True, stop=True)
        s1t = stage.tile([U, NTW], BF16, name="s1t", tag="s1t")
        nc.scalar.activation(s1t, ps1, AF.Relu, bias=bs1v)
        ps2 = psscr.tile([1, NTW], F32, name="ps2", tag="scr")
        nc.tensor.matmul(ps2, ws2, s1t, start=True, stop=True)
        nc.scalar.activation(s_row[:, sl], ps2, AF.Relu, bias=bs2v)

    # ---- Y tiles: Y[m,b] = C_Y * X_b @ [k_r[m]|k_u[m]], stored fp8
    # [128, NB, 3, 128]: [node%128, node//128, m, u'] ----
    y = [ypool.tile([NBW, NB, 3, 2 * U], F8, name=f"y_{b}", tag=f"y{b}")
         for b in range(BL)]

    _eng = [0]

    def evac(out_ap, in_ap, dve_share=1):
        """PSUM evacuation, rotating DVE / ACT to balance load
        (`dve_share` DVE copies per ACT copy; -1 = DVE only, 0 = ACT
        only)."""
        if dve_share < 0:
            nc.vector.tensor_copy(out_ap, in_ap)
            return
        _eng[0] = (_eng[0] + 1) % (dve_share + 1)
        if _eng[0]:
            nc.vector.tensor_copy(out_ap, in_ap)
        else:
            nc.scalar.activation(out_ap, in_ap, AF.Copy)

    def ygen_thunks(b, dve_share=1):
        def mk(j):
            def f():
                nsl = slice(j * NBW, (j + 1) * NBW)
                py = psscr.tile([NBW, 3 * 2 * U], F32, name="py", tag="scr")
                nc.tensor.matmul(py, xT[b][:, nsl], kkall, start=True,
                                 stop=True)
                evac(y[b][:, j, :, :], py.rearrange("p (m u) -> p m u", m=3),
                     dve_share=dve_share)
            return f
        return [mk(j) for j in range(NB)]

    # y-gen for b=0/1, with tile-0's e^T generation interleaved so et[0]
    # is ready before the t-loop needs it (every later tile's e^T is
    # generated one tile ahead inside agroups, keeping the ACT exp burst
    # off the critical path)
    def e_thunks(t, et):
        """e^T[:, t] = exp(K Q^T / 8) into et (fp8), one node-block per
        call."""
        sl = slice(t * NTW, (t + 1) * NTW)

        def mk(j):
            def f():
                pe = psscr.tile([NBW, NTW], F32, name="pe", tag="scr")
                nc.tensor.matmul(pe, KT[:, :, j * NBW:(j + 1) * NBW],
                                 QT[:, :, sl], start=True, stop=True,
                                 perf_mode=DR)
                nc.scalar.activation(et[:, j, :], pe, AF.Exp, scale=0.125)
            return f
        return [mk(j) for j in range(NB)]

    def emit_dsum(et, pd):
        for j in range(NSB):
            nc.tensor.matmul(pd, ones8, et[:, 2 * j:2 * j + 2, :],
                             start=(j == 0), stop=(j == NSB - 1),
                             perf_mode=DR)

    def interleave(main, extra, ratio=2):
        """Emit `ratio` thunks from main per one from extra."""
        mi = ei = 0
        while mi < len(main) or ei < len(extra):
            for _ in range(ratio):
                if mi < len(main):
                    main[mi](); mi += 1
            if ei < len(extra):
                extra[ei](); ei += 1

    # y-gen for b=0/1, with tile-0's e^T generation interleaved so et[0]
    # is ready before the t-loop needs it (every later tile's e^T is
    # generated one tile ahead inside agroups, keeping the ACT exp burst
    # off the critical path)
    interleave(ygen_thunks(0, dve_share=2) + ygen_thunks(1, dve_share=2),
               e_thunks(0, ets[0]), ratio=2)

    # =================== phase 1: r & u gates ===================
    def a_thunks1(b, t, sl, pa):
        th = [lambda: nc.tensor.matmul(pa, kk0, xT[b][:, sl],
                                       start=True, stop=False)]
        for m, asl in ((0, a1t[t]), (1, a2t[t])):
            for j in range(NSB):
                def f(m=m, asl=asl, j=j):
                    nc.tensor.matmul(pa, y[b][:, 2 * j:2 * j + 2, m, :],
                                     asl[:, 2 * j:2 * j + 2, :],
                                     start=False,
                                     stop=(m == 1 and j == NSB - 1),
                                     perf_mode=DR)
                th.append(f)
        return th

    def agroup1(b, t, sl, extra=(), ratio=2):
        pa = psacc.tile([128, NTW], F32, name="pa", tag="acc")
        interleave(a_thunks1(b, t, sl, pa), list(extra), ratio=ratio)
        return pa

    def pbgroup(b, t, et, ytile=None, m=2):
        """The e-support contraction (PE only, so it can be emitted away
        from the DVE/ACT finish chain)."""
        yt = y[b] if ytile is None else ytile
        pb = psacc.tile([128, NTW], F32, name="pb", tag="acc")
        for j in range(NSB):
            nc.tensor.matmul(pb, yt[:, 2 * j:2 * j + 2, m, :],
                             et[:, 2 * j:2 * j + 2, :],
                             start=(j == 0), stop=(j == NSB - 1),
                             perf_mode=DR)
        return pb

    def bfinish1(b, t, sl, pa, pb):
        tmp = stage.tile([128, NTW], F32, name="tmp", tag="tmp")
        nc.vector.tensor_mul(tmp, pb, rdbc[t])
        ssum = stage.tile([128, NTW], F32, name="ssum", tag="ssum")
        nc.vector.tensor_add(ssum, pa, tmp)
        # sigmoid(z) = 0.5*(1 + tanh(z/2)): Tanh lives in the same ACT
        # function table as Exp/Copy, so phase 1 never reloads the table
        # (Sigmoid does not; a reload is 1283ns). sig holds th = 2r-1 /
        # 2u-1; the affine fixups are folded downstream.
        sig = stage.tile([128, NTW], F32, name="sig", tag="sig")
        nc.scalar.activation(sig, ssum, AF.Tanh, scale=SCL / 2, bias=bruh)
        # rh' = (1+th)*h = 2*(r*h) -> rows 0:64 in place as th*h + h
        # (the 0.5 is folded into the state rows of kc0/kcall host-side);
        # two plain tensor-tensor ops on the otherwise idle GpSimd.
        # thu -> DRAM scratch.
        rhs = stage.tile([U, NTW], BF16, name="rhs", tag="s1t")
        nc.gpsimd.tensor_mul(rhs, sig[0:U, :], xT[b][0:U, sl])
        nc.gpsimd.tensor_add(xT[b][0:U, sl], rhs, xT[b][0:U, sl])
        ueng = nc.scalar if b % 2 else nc.sync
        ueng.dma_start(out=uscr[b][:, sl], in_=sig[U:128, :])

    def ycgen_thunks(yct, p, dve_share=1):
        th = []
        for half in range(2):
            b = 2 * p + half
            usl = slice(half * U, (half + 1) * U)
            for j in range(NB):
                def f(b=b, usl=usl, j=j, yct=yct):
                    nsl = slice(j * NBW, (j + 1) * NBW)
                    pyc = psscr.tile([NBW, 3 * U], F32, name="pyc", tag="scr")
                    nc.tensor.matmul(pyc, xT[b][:, nsl], kcall,
                                     start=True, stop=True)
                    evac(yct[:, j, :, usl],
                         pyc.rearrange("p (m u) -> p m u", m=3),
                         dve_share=dve_share)
                th.append(f)
        return th

    yc = [None, None]

    # Per-tile emission order keeps PE fed continuously:
    #   ag0(+e) ag1(+e) ag2 pb0 d8 [dchain] ag3 fin0 pb1 fin1 pb2 fin2
    #   pb3 fin3
    # The PSUM "acc" live set never exceeds 5 (pa0-3 + one pb).
    for t in range(NT):
        sl = slice(t * NTW, (t + 1) * NTW)
        et = ets[t]
        # generate NEXT tile's e^T inside this tile's agroups (this
        # tile's was generated one tile ago)
        eth = e_thunks(t + 1, ets[t + 1]) if t + 1 < NT else []
        if t == 0:
            y2g, y3g = ygen_thunks(2, dve_share=2), ygen_thunks(3, dve_share=2)
            pa0 = agroup1(0, t, sl, eth[0:8], ratio=1)
        else:
            pa0 = agroup1(0, t, sl, eth[0:8], ratio=2)

        # d/rdbc right after ag0 — et[t] was generated one tile ago, so
        # the serial d8 -> dsb -> 1/d -> broadcast chain hides under the
        # remaining agroups instead of stalling the finishes
        pd = psscr.tile([1, NTW], F32, name="pd", tag="scr")
        emit_dsum(et, pd)
        # d[n] = s[n] + colsum(e^T)[n]; rdbc[t][p, n] = C_ADJ / d[n]
        dsb = dsp.tile([1, NTW], F32, name="dsb", tag="dsb")
        nc.vector.tensor_add(dsb, pd, s_row[:, sl])
        rds = dsp.tile([1, NTW], BF16, name="rds", tag="rds")
        with nc.allow_low_precision(reason="1/d feeds an fp8-noise-dominated "
                                    "term; bf16 is plenty"):
            nc.vector.reciprocal(rds, dsb)
        pr = psscr.tile([128, NTW], F32, name="pr", tag="scr")
        nc.tensor.matmul(pr, ones_row, rds, start=True, stop=True)
        nc.vector.tensor_copy(rdbc[t], pr)

        if t == 0:
            pa1 = agroup1(1, t, sl, eth[8:16] + y2g, ratio=1)
        else:
            pa1 = agroup1(1, t, sl, eth[8:16], ratio=2)
        pb0 = pbgroup(0, t, et)
        bfinish1(0, t, sl, pa0, pb0)
        if t == 0:
            pa2 = agroup1(2, t, sl, y3g, ratio=1)
        else:
            pa2 = agroup1(2, t, sl)
        pb1 = pbgroup(1, t, et)
        bfinish1(1, t, sl, pa1, pb1)
        pa3 = agroup1(3, t, sl)
        if t == NT - 1:
            # yc[0] gen needs the rh updates of b0/b1 (fin0/fin1) above;
            # interleave it with the remaining e-support contractions
            yc[0] = ypool.tile([NBW, NB, 3, 2 * U], F8, name="yc_0", tag="y0")
            yc0g = ycgen_thunks(yc[0], 0)
            pb23 = []
            th23 = [lambda: pb23.append(pbgroup(2, t, et)),
                    lambda: pb23.append(pbgroup(3, t, et))]
            interleave(th23, yc0g, ratio=1)
            bfinish1(2, t, sl, pa2, pb23[0])
            bfinish1(3, t, sl, pa3, pb23[1])
        else:
            for b, pa in ((2, pa2), (3, pa3)):
                pb = pbgroup(b, t, et)
                bfinish1(b, t, sl, pa, pb)

    # =================== phase 2+3: c gate & h_new ===================
    # Yc[pair] = C_Y * [Xc_b0 @ kc[m] | Xc_b1 @ kc[m]], fp8
    # (yc[0] was already generated inside phase-1's final A-group; yc[1]
    # is interleaved into phase-2 t=0's first A-group below, on the
    # mostly idle ACT)
    yc[1] = ypool.tile([NBW, NB, 3, 2 * U], F8, name="yc_1", tag="y1")
    yc1g = ycgen_thunks(yc[1], 1)

    for t in range(NT):
        sl = slice(t * NTW, (t + 1) * NTW)
        et = ets[t]
        a1, a2 = a1t[t], a2t[t]
        # prefetch h and u for the tail chains, one pair-merged DMA each
        hps, ups = [], []
        for p in range(BL // 2):
            hp = p3p.tile([128, NTW], F32, name="hp", tag="hp")
            up = p3p.tile([128, NTW], F32, name="up", tag="up")
            psl = slice(2 * p, 2 * p + 2)
            # the SBUF side stays a plain [128, 512] AP: its (partition,
            # col) iteration order already matches the DRAM side's
            # (b, u, w) — splitting the SBUF partition dim via rearrange
            # produces corrupt flat-offset strides
            nc.sync.dma_start(out=hp, in_=d["hT"][psl, :, sl])
            nc.sync.dma_start(out=up, in_=uscr[psl, :, sl])
            # uscr holds thu = 2u-1; recover u = 0.5*thu + 0.5 in place,
            # off the tail's critical path (the chain below then uses
            # only plain tensor-tensor ops)
            nc.vector.tensor_scalar(up, up, 1.0, 0.5,
                                    op0=mybir.AluOpType.add,
                                    op1=mybir.AluOpType.mult)
            hps.append(hp)
            ups.append(up)
        for p in range(BL // 2):
            b0, b1 = 2 * p, 2 * p + 1
            pa = psacc.tile([128, NTW], F32, name="pa2", tag="acc")
            th = [lambda pa=pa, p=p: nc.tensor.matmul(
                      pa, yc[p][:, 0:2, 0, :], a1[:, 0:2, :],
                      start=True, stop=False, perf_mode=DR),
                  lambda pa=pa, b0=b0: nc.tensor.matmul(
                      pa[0:U, :], kc0, xT[b0][:, sl], start=False, stop=False),
                  lambda pa=pa, b1=b1: nc.tensor.matmul(
                      pa[U:128, :], kc0, xT[b1][:, sl], start=False,
                      stop=False)]
            for m, asl in ((0, a1), (1, a2)):
                for j in range(NSB):
                    if m == 0 and j == 0:
                        continue
                    def f(pa=pa, p=p, m=m, asl=asl, j=j):
                        nc.tensor.matmul(pa, yc[p][:, 2 * j:2 * j + 2, m, :],
                                         asl[:, 2 * j:2 * j + 2, :],
                                         start=False,
                                         stop=(m == 1 and j == NSB - 1),
                                         perf_mode=DR)
                    th.append(f)
            hp, up = hps[p], ups[p]
            pb = pbgroup(p, t, et, ytile=yc[p])
            if t == 0 and p == 0:
                interleave(th, yc1g, ratio=1)
            else:
                for f in th:
                    f()
            tmp = stage.tile([128, NTW], F32, name="tmp2", tag="tmp")
            ct = ctp.tile([128, NTW], F32, name="ct", tag="ct")
            ssum = stage.tile([128, NTW], F32, name="ssum2", tag="ssum")
            # run the gate + h_new chain in column halves so the DVE / ACT
            # / GpSimd / DMA stages pipeline; the elementwise h_new chain
            # (h_new = c + u*(h - c), in place on hp) runs on the otherwise
            # idle GpSimd except on the final tile, where its latency is
            # exposed and DVE (2x f32 SBUF mode) drains faster.
            # the h_new chain stays OFF the DVE queue (in-order: a chain op
            # waiting on tanh would block the next pair's tmp/ssum) except
            # on the final tile, where DVE has nothing left queued and
            # drains faster than GpSimd
            veng = nc.vector if t == NT - 1 else nc.gpsimd
            for c0 in range(0, NTW, NTW // 2):
                cs = slice(c0, c0 + NTW // 2)
                nc.vector.tensor_mul(tmp[:, cs], pb[:, cs], rdbc[t][:, cs])
                nc.vector.tensor_add(ssum[:, cs], pa[:, cs], tmp[:, cs])
                nc.scalar.activation(ct[:, cs], ssum[:, cs], AF.Tanh,
                                     scale=SCL, bias=bc2)
                # h_new = c + u*(h - c), in place on hp
                veng.tensor_sub(hp[:, cs], hp[:, cs], ct[:, cs])
                veng.tensor_mul(hp[:, cs], up[:, cs], hp[:, cs])
                veng.tensor_add(hp[:, cs], hp[:, cs], ct[:, cs])
            # one pair-merged output DMA per (pair, tile)
            nc.scalar.dma_start(out=out_h[b0:b1 + 1, :, sl], in_=hp)

    ctx.close()


_CACHE = {}


def _get_program():
    if "nc" not in _CACHE:
        _CACHE["nc"] = _build_program()
    return _CACHE["nc"]


def _prep_inputs(inputs, h_prev, adj1, adj2, feat, SE, Wq, Wk, Ws1, bs1, Ws2,
                 bs2, r_kernel, r_bias, u_kernel, u_bias, c_kernel, c_bias):
    bf = ml_dtypes.bfloat16
    f8 = ml_dtypes.float8_e4m3
    f32 = np.float32
    perm = list(range(DIN, FROWS)) + list(range(DIN))  # [h(64); inputs(2)]

    h3 = np.asarray(h_prev, f32).reshape(B, N, U)
    hT = np.ascontiguousarray(h3.transpose(0, 2, 1))            # [B, U, N]
    inT = np.asarray(inputs, f32).transpose(0, 2, 1)            # [B, DIN, N]
    xT = np.concatenate([hT, inT], axis=1).astype(bf)           # [B, 66, N]

    rk = np.asarray(r_kernel, f32)[:, perm, :]
    uk = np.asarray(u_kernel, f32)[:, perm, :]
    ck = np.asarray(c_kernel, f32)[:, perm, :]
    # the device keeps rh' = 2*(r*h) in the state rows of x_cat_c (tanh
    # trick), so halve the state rows of the c kernels to compensate
    ck = ck.copy()
    ck[:, 0:U, :] *= 0.5
    kkall = (np.concatenate(
        [np.concatenate([rk[m], uk[m]], axis=1) for m in (1, 2, 3)],
        axis=1) * C_Y).astype(bf)                               # [66, 384]
    kk0 = (np.concatenate([rk[0], uk[0]], axis=1) * G).astype(bf)
    kcall = (np.concatenate([ck[1], ck[2], ck[3]], axis=1) * C_Y).astype(bf)
    kc0 = (ck[0] * G).astype(bf)

    shared = {
        "a1T": np.ascontiguousarray(
            np.asarray(adj1, f32).T * C_ADJ).astype(f8),
        "a2T": np.ascontiguousarray(
            np.asarray(adj2, f32).T * C_ADJ).astype(f8),
        "fsT": np.ascontiguousarray(
            np.concatenate([np.asarray(feat, f32).T, np.asarray(SE, f32).T],
                           axis=0)).astype(bf),
        "wqk": np.concatenate([np.asarray(Wq, f32), np.asarray(Wk, f32)],
                              axis=1).astype(bf),
        "ws1": np.asarray(Ws1, f32).astype(bf),
        "ws2": np.asarray(Ws2, f32).reshape(U, 1).astype(bf),
        "kblob": np.concatenate([kkall, kk0, kcall, kc0], axis=1),
        "bs1v": np.asarray(bs1, f32).reshape(U, 1),
        "bruh": 0.5 * np.concatenate(
            [np.asarray(r_bias, f32).mean(0),
             np.asarray(u_bias, f32).mean(0)]).reshape(-1, 1),
        "bc2": np.tile(np.asarray(c_bias, f32).mean(0), 2).reshape(-1, 1),
        "bs2v": np.asarray(bs2, f32).reshape(1, 1),
    }
    in_maps = []
    for c in range(NCORES):
        bsl = slice(c * BL, (c + 1) * BL)
        m = dict(shared)
        m["xT"] = np.ascontiguousarray(xT[bsl])
        m["hT"] = np.ascontiguousarray(hT[bsl])
        in_maps.append(m)
    return in_maps


def kernel(**inputs):
    os.environ.setdefault("NEURON_RT_RESET_CORES", "1")
    nc = _get_program()
    in_maps = _prep_inputs(**inputs)
    res = None
    err = None
    for _ in range(2):
        try:
            res = run_bass_kernel_spmd(nc, in_maps, list(range(NCORES)))
            break
        except Exception as e:  # e.g. a wedged device; retry once
            err = e
    if res is None:
        raise err
    outs = []
    for c in range(NCORES):
        o = res.results[c]["out"]                     # [BL, U, N] f32
        outs.append(o.transpose(0, 2, 1).reshape(BL, N * U))
    return np.concatenate(outs, axis=0).astype(np.float32)


# revision 117
# speedup vs baseline: 2.4412x; 1.0071x over previous
"""MFGCGRU (graph-conv GRU cell) Trainium2 kernel.

Strategy: data-parallel over batch B=32 across 8 NeuronCores (4 batches
per core). All NxN supports replicated per core and resident in SBUF.

The dominant work — applying the three supports (adj1, adj2, e-attn) to
the per-gate projections Y_m = X @ k_m — runs as fp8(e4m3) matmuls in
DoubleRow perf mode: operands are packed [128, 2, F] so each PE pass
contracts 256 source nodes at half a cycle per output column.

Scaling bookkeeping (so fp8 operands sit in e4m3's sweet spot):
  - adjacencies are sent as 64*S^T            (C_ADJ = 64)
  - support kernels are folded with x16       (C_Y = 16, Y' = 16*X@k)
  - identity kernels are folded with x1024    (G = C_ADJ*C_Y)
  - the e-term normalizer is rdbc = 64/d, applied to the (x16-scaled)
    e-contribution PSUM, matching the x1024 of the adj/identity terms
  - gate activations then use ACT scale = 0.25/1024 (the /M fold)

The attention support is built unnormalized as e^T = exp(K Q^T / 8)
from fp8 Q/K packed [32, 2, N] (u = plane*32 + p), stored fp8 and kept
resident for both the r/u and the c passes; its row-normalizer is
applied to the e-contribution via a second PSUM accumulator.
"""

import contextlib
import os

import numpy as np
import ml_dtypes

import concourse.bass as bass
import concourse.bacc as bacc
import concourse.tile as tile
from concourse import mybir
from concourse.bass_utils import run_bass_kernel_spmd

F32 = mybir.dt.float32
BF16 = mybir.dt.bfloat16
F8 = mybir.dt.float8e4
AF = mybir.ActivationFunctionType
DR = mybir.MatmulPerfMode.DoubleRow

B, N, DIN, U, FD, SD = 32, 2048, 2, 64, 32, 64
NCORES = 8
BL = B // NCORES          # batches per core
NTW = 512                 # n-tile width
NT = N // NTW             # 4 n-tiles
NBW = 128                 # node-block width
NB = N // NBW             # 16 node blocks
NSB = NB // 2             # 8 node super-blocks (256 nodes, DoubleRow)
FROWS = DIN + U           # 66

C_ADJ = 64.0              # host scale on S^T before e4m3 cast
C_Y = 16.0                # host scale folded into support kernels
G = C_ADJ * C_Y           # net scale of the PSUM accumulators
SCL = 0.25 / G            # ACT scale for the gate activations (incl /M)


def _build_program():
    nc = bacc.Bacc("TRN2", debug=False, num_devices=NCORES)

    d = {}

    def din(name, shape, dt):
        d[name] = nc.dram_tensor(name, shape, dt, kind="ExternalInput").ap()

    din("xT", [BL, FROWS, N], BF16)
    din("hT", [BL, U, N], F32)
    din("a1T", [N, N], F8)
    din("a2T", [N, N], F8)
    din("fsT", [FD + SD, N], BF16)
    # kernels are packed into one tensor: each dma_start holds the
    # (single, serialized) HWDGE device ~630ns regardless of size
    din("wqk", [FD, 2 * U], BF16)            # [wq | wk]
    din("ws1", [FD + SD, U], BF16)
    din("ws2", [U, 1], BF16)
    din("kblob", [FROWS, 768], BF16)         # [kkall|kk0|kcall|kc0]
    din("bs1v", [U, 1], F32)
    din("bruh", [2 * U, 1], F32)
    din("bc2", [2 * U, 1], F32)
    din("bs2v", [1, 1], F32)
    out_h = nc.dram_tensor("out", [BL, U, N], F32, kind="ExternalOutput").ap()
    uscr = nc.dram_tensor("uscr", [BL, U, N], F32).ap()

    with tile.TileContext(nc) as tc:
        _emit(tc, d, out_h, uscr)
    nc.compile()
    return nc


def _emit(tc, d, out_h, uscr):
    nc = tc.nc
    ctx = contextlib.ExitStack()
    const = ctx.enter_context(tc.tile_pool(name="const", bufs=1))
    persist = ctx.enter_context(tc.tile_pool(name="persist", bufs=1))
    ypool = ctx.enter_context(tc.tile_pool(name="ypool", bufs=1))
    stage = ctx.enter_context(tc.tile_pool(name="stage", bufs=3))
    # phase-2/3 pipeline tiles: 4 pair-tile tails in flight, so the h/u
    # prefetch and the tanh of tile t+1 never wait on tile t's drain
    p3p = ctx.enter_context(tc.tile_pool(name="p3p", bufs=4))
    ctp = ctx.enter_context(tc.tile_pool(name="ctp", bufs=4))
    dsp = ctx.enter_context(tc.tile_pool(name="dsp", bufs=1))
    psacc = ctx.enter_context(tc.tile_pool(name="psacc", bufs=4, space="PSUM"))
    psscr = ctx.enter_context(tc.tile_pool(name="psscr", bufs=4, space="PSUM"))

    # ---- constants / weights in SBUF ----
    def cload(name, shape=None, dt=None):
        ap = d[name]
        t = const.tile(list(ap.shape) if shape is None else shape,
                       ap.dtype if dt is None else dt, name=f"c_{name}")
        nc.sync.dma_start(out=t, in_=ap)
        return t

    # DMA order matters: the startup critical path is
    #   fsT/wqk (prelude) and xT/kblob (y-gen), then adj slice t=0.
    fsT = const.tile([FD + SD, N], BF16, name="c_fsT")
    nc.sync.dma_start(out=fsT[:, 0:NTW], in_=d["fsT"][:, 0:NTW])
    wqk = cload("wqk")
    nc.sync.dma_start(out=fsT[:, NTW:], in_=d["fsT"][:, NTW:])
    wq, wk = wqk[:, 0:U], wqk[:, U:2 * U]
    kblob = cload("kblob")
    kkall = kblob[:, 0:384]
    kk0 = kblob[:, 384:512]
    kcall = kblob[:, 512:704]
    kc0 = kblob[:, 704:768]

    xTall = persist.tile([FROWS, BL, N], BF16, name="xTall", tag="xTall")
    # two DMAs so y-gen for b=0/1 can start before b=2/3 lands
    nc.sync.dma_start(out=xTall[:, 0:2, :],
                      in_=d["xT"][0:2].rearrange("b f n -> f b n"))
    nc.sync.dma_start(out=xTall[:, 2:4, :],
                      in_=d["xT"][2:4].rearrange("b f n -> f b n"))
    xT = [xTall[:, b, :] for b in range(BL)]

    ws1 = cload("ws1")
    ws2 = cload("ws2")
    bs1v = cload("bs1v")
    bruh = cload("bruh")        # pre-halved host-side for the tanh trick
    bc2 = cload("bc2")
    bs2v = cload("bs2v")

    # ---- resident adjacency slices (fp8, reused by both phases) ----
    a1t = [persist.tile([NBW, NB, NTW], F8, name=f"a1T_{t}", tag=f"a1T_{t}")
           for t in range(NT)]
    a2t = [persist.tile([NBW, NB, NTW], F8, name=f"a2T_{t}", tag=f"a2T_{t}")
           for t in range(NT)]
    for t in range(NT):
        sl = slice(t * NTW, (t + 1) * NTW)
        for name, lst in (("a1T", a1t), ("a2T", a2t)):
            nc.sync.dma_start(
                out=lst[t],
                in_=d[name][:, sl].rearrange("(j p) w -> p j w", p=NBW))

    # k-plane stride of a DoubleRow LDWEIGHTS AP must be 16-byte aligned,
    # so pad the ones column out to 16 bytes per plane
    ones8f = const.tile([NBW, 2, 16], F8, name="ones8")
    nc.vector.memset(ones8f, 1.0)
    ones8 = ones8f[:, :, 0:1]
    ones_row = const.tile([1, NBW], BF16, name="ones_row")
    nc.vector.memset(ones_row, C_ADJ)

    # PE p-state warm-up: the model throttles matmuls until 3us of
    # continuous execution. Run const-data fillers during the initial
    # weight-DMA wait so the array is at full clock when real work lands.
    fillm = const.tile([1, NTW], BF16, name="fillm")
    nc.vector.memset(fillm, 1.0)
    for _ in range(7):
        pf = psscr.tile([NBW, NTW], F32, name="pf", tag="scr")
        nc.tensor.matmul(pf, ones_row, fillm, start=True, stop=True)

    QT = persist.tile([U // 2, 2, N], F8, name="QT", tag="QT")
    KT = persist.tile([U // 2, 2, N], F8, name="KT", tag="KT")
    s_row = persist.tile([1, N], BF16, name="s_row", tag="s_row")
    rdbc = [persist.tile([NBW, NTW], BF16, name=f"rdbc{t}", tag=f"rdbc{t}")
            for t in range(NT)]
    ets = [persist.tile([NBW, NB, NTW], F8, name=f"et{t}", tag=f"et{t}")
           for t in range(NT)]

    # ---- prelude: K^T first (e-gen of t=0 reads all of K), then Q^T
    # (fp8 split-u packing), then s (not needed until the first dchain) ----
    for w, qt in ((wk, KT), (wq, QT)):
        for t in range(NT):
            sl = slice(t * NTW, (t + 1) * NTW)
            for half in range(2):
                pq = psscr.tile([U // 2, NTW], F32, name="pq", tag="scr")
                nc.tensor.matmul(pq, w[:, half * 32:(half + 1) * 32],
                                 fsT[0:FD, sl], start=True, stop=True)
                # alternate engines; both have startup slack
                if half:
                    nc.scalar.activation(qt[:, half, sl], pq, AF.Relu)
                else:
                    nc.vector.tensor_scalar_max(qt[:, half, sl], pq, 0.0)
    for t in range(NT):
        sl = slice(t * NTW, (t + 1) * NTW)
        ps1 = psscr.tile([U, NTW], F32, name="ps1", tag="scr")
        nc.tensor.matmul(ps1, ws1, fsT[:, sl], start=True, stop=True)
        s1t = stage.tile([U, NTW], BF16, name="s1t", tag="s1t")
        nc.scalar.activation(s1t, ps1, AF.Relu, bias=bs1v)
        ps2 = psscr.tile([1, NTW], F32, name="ps2", tag="scr")
        nc.tensor.matmul(ps2, ws2, s1t, start=True, stop=True)
        nc.scalar.activation(s_row[:, sl], ps2, AF.Relu, bias=bs2v)

    # ---- Y tiles: Y[m,b] = C_Y * X_b @ [k_r[m]|k_u[m]], stored fp8
    # [128, NB, 3, 128]: [node%128, node//128, m, u'] ----
    y = [ypool.tile([NBW, NB, 3, 2 * U], F8, name=f"y_{b}", tag=f"y{b}")
         for b in range(BL)]

    _eng = [0]

    def evac(out_ap, in_ap, dve_share=1):
        """PSUM evacuation, rotating DVE / ACT to balance load
        (`dve_share` DVE copies per ACT copy; -1 = DVE only, 0 = ACT
        only)."""
        if dve_share < 0:
            nc.vector.tensor_copy(out_ap, in_ap)
            return
        _eng[0] = (_eng[0] + 1) % (dve_share + 1)
        if _eng[0]:
            nc.vector.tensor_copy(out_ap, in_ap)
        else:
            nc.scalar.activation(out_ap, in_ap, AF.Copy)

    def ygen_thunks(b, dve_share=1):
        def mk(j):
            def f():
                nsl = slice(j * NBW, (j + 1) * NBW)
                py = psscr.tile([NBW, 3 * 2 * U], F32, name="py", tag="scr")
                nc.tensor.matmul(py, xT[b][:, nsl], kkall, start=True,
                                 stop=True)
                evac(y[b][:, j, :, :], py.rearrange("p (m u) -> p m u", m=3),
                     dve_share=dve_share)
            return f
        return [mk(j) for j in range(NB)]

    # y-gen for b=0/1, with tile-0's e^T generation interleaved so et[0]
    # is ready before the t-loop needs it (every later tile's e^T is
    # generated one tile ahead inside agroups, keeping the ACT exp burst
    # off the critical path)
    def e_thunks(t, et):
        """e^T[:, t] = exp(K Q^T / 8) into et (fp8), one node-block per
        call."""
        sl = slice(t * NTW, (t + 1) * NTW)

        def mk(j):
            def f():
                pe = psscr.tile([NBW, NTW], F32, name="pe", tag="scr")
                nc.tensor.matmul(pe, KT[:, :, j * NBW:(j + 1) * NBW],
                                 QT[:, :, sl], start=True, stop=True,
                                 perf_mode=DR)
                nc.scalar.activation(et[:, j, :], pe, AF.Exp, scale=0.125)
            return f
        return [mk(j) for j in range(NB)]

    def emit_dsum(et, pd):
        for j in range(NSB):
            nc.tensor.matmul(pd, ones8, et[:, 2 * j:2 * j + 2, :],
                             start=(j == 0), stop=(j == NSB - 1),
                             perf_mode=DR)

    def interleave(main, extra, ratio=2):
        """Emit `ratio` thunks from main per one from extra."""
        mi = ei = 0
        while mi < len(main) or ei < len(extra):
            for _ in range(ratio):
                if mi < len(main):
                    main[mi](); mi += 1
            if ei < len(extra):
                extra[ei](); ei += 1

    # y-gen for b=0/1, with tile-0's e^T generation interleaved so et[0]
    # is ready before the t-loop needs it (every later tile's e^T is
    # generated one tile ahead inside agroups, keeping the ACT exp burst
    # off the critical path)
    interleave(ygen_thunks(0, dve_share=2) + ygen_thunks(1, dve_share=2),
               e_thunks(0, ets[0]), ratio=2)

    # =================== phase 1: r & u gates ===================
    def a_thunks1(b, t, sl, pa):
        th = [lambda: nc.tensor.matmul(pa, kk0, xT[b][:, sl],
                                       start=True, stop=False)]
        for m, asl in ((0, a1t[t]), (1, a2t[t])):
            for j in range(NSB):
                def f(m=m, asl=asl, j=j):
                    nc.tensor.matmul(pa, y[b][:, 2 * j:2 * j + 2, m, :],
                                     asl[:, 2 * j:2 * j + 2, :],
                                     start=False,
                                     stop=(m == 1 and j == NSB - 1),
                                     perf_mode=DR)
                th.append(f)
        return th

    def agroup1(b, t, sl, extra=(), ratio=2):
        pa = psacc.tile([128, NTW], F32, name="pa", tag="acc")
        interleave(a_thunks1(b, t, sl, pa), list(extra), ratio=ratio)
        return pa

    def pbgroup(b, t, et, ytile=None, m=2):
        """The e-support contraction (PE only, so it can be emitted away
        from the DVE/ACT finish chain)."""
        yt = y[b] if ytile is None else ytile
        pb = psacc.tile([128, NTW], F32, name="pb", tag="acc")
        for j in range(NSB):
            nc.tensor.matmul(pb, yt[:, 2 * j:2 * j + 2, m, :],
                             et[:, 2 * j:2 * j + 2, :],
                             start=(j == 0), stop=(j == NSB - 1),
                             perf_mode=DR)
        return pb

    def bfinish1(b, t, sl, pa, pb):
        tmp = stage.tile([128, NTW], F32, name="tmp", tag="tmp")
        nc.vector.tensor_mul(tmp, pb, rdbc[t])
        ssum = stage.tile([128, NTW], F32, name="ssum", tag="ssum")
        nc.vector.tensor_add(ssum, pa, tmp)
        # sigmoid(z) = 0.5*(1 + tanh(z/2)): Tanh lives in the same ACT
        # function table as Exp/Copy, so phase 1 never reloads the table
        # (Sigmoid does not; a reload is 1283ns). sig holds th = 2r-1 /
        # 2u-1; the affine fixups are folded downstream.
        sig = stage.tile([128, NTW], F32, name="sig", tag="sig")
        nc.scalar.activation(sig, ssum, AF.Tanh, scale=SCL / 2, bias=bruh)
        # rh' = (1+th)*h = 2*(r*h) -> rows 0:64 in place as th*h + h
        # (the 0.5 is folded into the state rows of kc0/kcall host-side);
        # two plain tensor-tensor ops on the otherwise idle GpSimd.
        # thu -> DRAM scratch.
        rhs = stage.tile([U, NTW], BF16, name="rhs", tag="s1t")
        nc.gpsimd.tensor_mul(rhs, sig[0:U, :], xT[b][0:U, sl])
        nc.gpsimd.tensor_add(xT[b][0:U, sl], rhs, xT[b][0:U, sl])
        ueng = nc.scalar if b % 2 else nc.sync
        ueng.dma_start(out=uscr[b][:, sl], in_=sig[U:128, :])

    def ycgen_thunks(yct, p, dve_share=1):
        th = []
        for half in range(2):
            b = 2 * p + half
            usl = slice(half * U, (half + 1) * U)
            for j in range(NB):
                def f(b=b, usl=usl, j=j, yct=yct):
                    nsl = slice(j * NBW, (j + 1) * NBW)
                    pyc = psscr.tile([NBW, 3 * U], F32, name="pyc", tag="scr")
                    nc.tensor.matmul(pyc, xT[b][:, nsl], kcall,
                                     start=True, stop=True)
                    evac(yct[:, j, :, usl],
                         pyc.rearrange("p (m u) -> p m u", m=3),
                         dve_share=dve_share)
                th.append(f)
        return th

    yc = [None, None]

    # Per-tile emission order keeps PE fed continuously:
    #   ag0(+e) ag1(+e) ag2 pb0 d8 [dchain] ag3 fin0 pb1 fin1 pb2 fin2
    #   pb3 fin3
    # The PSUM "acc" live set never exceeds 5 (pa0-3 + one pb).
    for t in range(NT):
        sl = slice(t * NTW, (t + 1) * NTW)
        et = ets[t]
        # generate NEXT tile's e^T inside this tile's agroups (this
        # tile's was generated one tile ago)
        eth = e_thunks(t + 1, ets[t + 1]) if t + 1 < NT else []
        if t == 0:
            y2g, y3g = ygen_thunks(2, dve_share=2), ygen_thunks(3, dve_share=2)
            pa0 = agroup1(0, t, sl, eth[0:8], ratio=1)
        else:
            pa0 = agroup1(0, t, sl, eth[0:8], ratio=2)

        # d/rdbc right after ag0 — et[t] was generated one tile ago, so
        # the serial d8 -> dsb -> 1/d -> broadcast chain hides under the
        # remaining agroups instead of stalling the finishes
        pd = psscr.tile([1, NTW], F32, name="pd", tag="scr")
        emit_dsum(et, pd)
        # d[n] = s[n] + colsum(e^T)[n]; rdbc[t][p, n] = C_ADJ / d[n]
        dsb = dsp.tile([1, NTW], F32, name="dsb", tag="dsb")
        nc.vector.tensor_add(dsb, pd, s_row[:, sl])
        rds = dsp.tile([1, NTW], BF16, name="rds", tag="rds")
        with nc.allow_low_precision(reason="1/d feeds an fp8-noise-dominated "
                                    "term; bf16 is plenty"):
            nc.vector.reciprocal(rds, dsb)
        pr = psscr.tile([128, NTW], F32, name="pr", tag="scr")
        nc.tensor.matmul(pr, ones_row, rds, start=True, stop=True)
        nc.vector.tensor_copy(rdbc[t], pr)

        if t == 0:
            pa1 = agroup1(1, t, sl, eth[8:16] + y2g, ratio=1)
        else:
            pa1 = agroup1(1, t, sl, eth[8:16], ratio=2)
        pb0 = pbgroup(0, t, et)
        bfinish1(0, t, sl, pa0, pb0)
        if t == 0:
            pa2 = agroup1(2, t, sl, y3g, ratio=1)
        else:
            pa2 = agroup1(2, t, sl)
        pb1 = pbgroup(1, t, et)
        bfinish1(1, t, sl, pa1, pb1)
        pa3 = agroup1(3, t, sl)
        if t == NT - 1:
            # yc[0] gen needs the rh updates of b0/b1 (fin0/fin1) above;
            # interleave it with the remaining e-support contractions
            yc[0] = ypool.tile([NBW, NB, 3, 2 * U], F8, name="yc_0", tag="y0")
            yc0g = ycgen_thunks(yc[0], 0)
            pb23 = []
            th23 = [lambda: pb23.append(pbgroup(2, t, et)),
                    lambda: pb23.append(pbgroup(3, t, et))]
            interleave(th23, yc0g, ratio=1)
            bfinish1(2, t, sl, pa2, pb23[0])
            bfinish1(3, t, sl, pa3, pb23[1])
        else:
            for b, pa in ((2, pa2), (3, pa3)):
                pb = pbgroup(b, t, et)
                bfinish1(b, t, sl, pa, pb)

    # =================== phase 2+3: c gate & h_new ===================
    # Yc[pair] = C_Y * [Xc_b0 @ kc[m] | Xc_b1 @ kc[m]], fp8
    # (yc[0] was already generated inside phase-1's final A-group; yc[1]
    # is interleaved into phase-2 t=0's first A-group below, on the
    # mostly idle ACT)
    yc[1] = ypool.tile([NBW, NB, 3, 2 * U], F8, name="yc_1", tag="y1")
    yc1g = ycgen_thunks(yc[1], 1)

    for t in range(NT):
        sl = slice(t * NTW, (t + 1) * NTW)
        et = ets[t]
        a1, a2 = a1t[t], a2t[t]
        # prefetch h and u for the tail chains, one pair-merged DMA each
        hps, ups = [], []
        for p in range(BL // 2):
            hp = p3p.tile([128, NTW], F32, name="hp", tag="hp")
            up = p3p.tile([128, NTW], F32, name="up", tag="up")
            psl = slice(2 * p, 2 * p + 2)
            # the SBUF side stays a plain [128, 512] AP: its (partition,
            # col) iteration order already matches the DRAM side's
            # (b, u, w) — splitting the SBUF partition dim via rearrange
            # produces corrupt flat-offset strides
            nc.sync.dma_start(out=hp, in_=d["hT"][psl, :, sl])
            nc.sync.dma_start(out=up, in_=uscr[psl, :, sl])
            # uscr holds thu = 2u-1; recover u = 0.5*thu + 0.5 in place,
            # off the tail's critical path (the chain below then uses
            # only plain tensor-tensor ops)
            nc.vector.tensor_scalar(up, up, 1.0, 0.5,
                                    op0=mybir.AluOpType.add,
                                    op1=mybir.AluOpType.mult)
            hps.append(hp)
            ups.append(up)
        for p in range(BL // 2):
            b0, b1 = 2 * p, 2 * p + 1
            pa = psacc.tile([128, NTW], F32, name="pa2", tag="acc")
            th = [lambda pa=pa, p=p: nc.tensor.matmul(
                      pa, yc[p][:, 0:2, 0, :], a1[:, 0:2, :],
                      start=True, stop=False, perf_mode=DR),
                  lambda pa=pa, b0=b0: nc.tensor.matmul(
                      pa[0:U, :], kc0, xT[b0][:, sl], start=False, stop=False),
                  lambda pa=pa, b1=b1: nc.tensor.matmul(
                      pa[U:128, :], kc0, xT[b1][:, sl], start=False,
                      stop=False)]
            for m, asl in ((0, a1), (1, a2)):
                for j in range(NSB):
                    if m == 0 and j == 0:
                        continue
                    def f(pa=pa, p=p, m=m, asl=asl, j=j):
                        nc.tensor.matmul(pa, yc[p][:, 2 * j:2 * j + 2, m, :],
                                         asl[:, 2 * j:2 * j + 2, :],
                                         start=False,
                                         stop=(m == 1 and j == NSB - 1),
                                         perf_mode=DR)
                    th.append(f)
            hp, up = hps[p], ups[p]
            pb = pbgroup(p, t, et, ytile=yc[p])
            if t == 0 and p == 0:
                interleave(th, yc1g, ratio=1)
            else:
                for f in th:
                    f()
            tmp = stage.tile([128, NTW], F32, name="tmp2", tag="tmp")
            ct = ctp.tile([128, NTW], F32, name="ct", tag="ct")
            ssum = stage.tile([128, NTW], F32, name="ssum2", tag="ssum")
            # run the gate + h_new chain in column halves so the DVE / ACT
            # / GpSimd / DMA stages pipeline; the elementwise h_new chain
            # (h_new = c + u*(h - c), in place on hp) runs on the otherwise
            # idle GpSimd except on the final tile, where its latency is
            # exposed and DVE (2x f32 SBUF mode) drains faster.
            # the h_new chain stays OFF the DVE queue (in-order: a chain op
            # waiting on tanh would block the next pair's tmp/ssum) except
            # on the final tile, where DVE has nothing left queued and
            # drains faster than GpSimd
            veng = nc.vector if t == NT - 1 else nc.gpsimd
            for c0 in range(0, NTW, NTW // 2):
                cs = slice(c0, c0 + NTW // 2)
                nc.vector.tensor_mul(tmp[:, cs], pb[:, cs], rdbc[t][:, cs])
                nc.vector.tensor_add(ssum[:, cs], pa[:, cs], tmp[:, cs])
                nc.scalar.activation(ct[:, cs], ssum[:, cs], AF.Tanh,
                                     scale=SCL, bias=bc2)
                # h_new = c + u*(h - c), in place on hp
                veng.tensor_sub(hp[:, cs], hp[:, cs], ct[:, cs])
                veng.tensor_mul(hp[:, cs], up[:, cs], hp[:, cs])
                veng.tensor_add(hp[:, cs], hp[:, cs], ct[:, cs])
            # one pair-merged output DMA per (pair, tile)
            nc.scalar.dma_start(out=out_h[b0:b1 + 1, :, sl], in_=hp)

    ctx.close()


_CACHE = {}


def _get_program():
    if "nc" not in _CACHE:
        _CACHE["nc"] = _build_program()
    return _CACHE["nc"]


def _prep_inputs(inputs, h_prev, adj1, adj2, feat, SE, Wq, Wk, Ws1, bs1, Ws2,
                 bs2, r_kernel, r_bias, u_kernel, u_bias, c_kernel, c_bias):
    bf = ml_dtypes.bfloat16
    f8 = ml_dtypes.float8_e4m3
    f32 = np.float32
    perm = list(range(DIN, FROWS)) + list(range(DIN))  # [h(64); inputs(2)]

    h3 = np.asarray(h_prev, f32).reshape(B, N, U)
    hT = np.ascontiguousarray(h3.transpose(0, 2, 1))            # [B, U, N]
    inT = np.asarray(inputs, f32).transpose(0, 2, 1)            # [B, DIN, N]
    xT = np.concatenate([hT, inT], axis=1).astype(bf)           # [B, 66, N]

    rk = np.asarray(r_kernel, f32)[:, perm, :]
    uk = np.asarray(u_kernel, f32)[:, perm, :]
    ck = np.asarray(c_kernel, f32)[:, perm, :]
    # the device keeps rh' = 2*(r*h) in the state rows of x_cat_c (tanh
    # trick), so halve the state rows of the c kernels to compensate
    ck = ck.copy()
    ck[:, 0:U, :] *= 0.5
    kkall = (np.concatenate(
        [np.concatenate([rk[m], uk[m]], axis=1) for m in (1, 2, 3)],
        axis=1) * C_Y).astype(bf)                               # [66, 384]
    kk0 = (np.concatenate([rk[0], uk[0]], axis=1) * G).astype(bf)
    kcall = (np.concatenate([ck[1], ck[2], ck[3]], axis=1) * C_Y).astype(bf)
    kc0 = (ck[0] * G).astype(bf)

    shared = {
        "a1T": np.ascontiguousarray(
            np.asarray(adj1, f32).T * C_ADJ).astype(f8),
        "a2T": np.ascontiguousarray(
            np.asarray(adj2, f32).T * C_ADJ).astype(f8),
        "fsT": np.ascontiguousarray(
            np.concatenate([np.asarray(feat, f32).T, np.asarray(SE, f32).T],
                           axis=0)).astype(bf),
        "wqk": np.concatenate([np.asarray(Wq, f32), np.asarray(Wk, f32)],
                              axis=1).astype(bf),
        "ws1": np.asarray(Ws1, f32).astype(bf),
        "ws2": np.asarray(Ws2, f32).reshape(U, 1).astype(bf),
        "kblob": np.concatenate([kkall, kk0, kcall, kc0], axis=1),
        "bs1v": np.asarray(bs1, f32).reshape(U, 1),
        "bruh": 0.5 * np.concatenate(
            [np.asarray(r_bias, f32).mean(0),
             np.asarray(u_bias, f32).mean(0)]).reshape(-1, 1),
        "bc2": np.tile(np.asarray(c_bias, f32).mean(0), 2).reshape(-1, 1),
        "bs2v": np.asarray(bs2, f32).reshape(1, 1),
    }
    in_maps = []
    for c in range(NCORES):
        bsl = slice(c * BL, (c + 1) * BL)
        m = dict(shared)
        m["xT"] = np.ascontiguousarray(xT[bsl])
        m["hT"] = np.ascontiguousarray(hT[bsl])
        in_maps.append(m)
    return in_maps


def kernel(**inputs):
    os.environ.setdefault("NEURON_RT_RESET_CORES", "1")
    nc = _get_program()
    in_maps = _prep_inputs(**inputs)
    res = None
    err = None
    for _ in range(2):
        try:
            res = run_bass_kernel_spmd(nc, in_maps, list(range(NCORES)))
            break
        except Exception as e:  # e.g. a wedged device; retry once
            err = e
    if res is None:
        raise err
    outs = []
    for c in range(NCORES):
        o = res.results[c]["out"]                     # [BL, U, N] f32
        outs.append(o.transpose(0, 2, 1).reshape(BL, N * U))
    return np.concatenate(outs, axis=0).astype(np.float32)


# revision 119
# speedup vs baseline: 2.4455x; 1.0018x over previous
"""MFGCGRU (graph-conv GRU cell) Trainium2 kernel.

Strategy: data-parallel over batch B=32 across 8 NeuronCores (4 batches
per core). All NxN supports replicated per core and resident in SBUF.

The dominant work — applying the three supports (adj1, adj2, e-attn) to
the per-gate projections Y_m = X @ k_m — runs as fp8(e4m3) matmuls in
DoubleRow perf mode: operands are packed [128, 2, F] so each PE pass
contracts 256 source nodes at half a cycle per output column.

Scaling bookkeeping (so fp8 operands sit in e4m3's sweet spot):
  - adjacencies are sent as 64*S^T            (C_ADJ = 64)
  - support kernels are folded with x16       (C_Y = 16, Y' = 16*X@k)
  - identity kernels are folded with x1024    (G = C_ADJ*C_Y)
  - the e-term normalizer is rdbc = 64/d, applied to the (x16-scaled)
    e-contribution PSUM, matching the x1024 of the adj/identity terms
  - gate activations then use ACT scale = 0.25/1024 (the /M fold)

The attention support is built unnormalized as e^T = exp(K Q^T / 8)
from fp8 Q/K packed [32, 2, N] (u = plane*32 + p), stored fp8 and kept
resident for both the r/u and the c passes; its row-normalizer is
applied to the e-contribution via a second PSUM accumulator.
"""

import contextlib
import os

import numpy as np
import ml_dtypes

import concourse.bass as bass
import concourse.bacc as bacc
import concourse.tile as tile
from concourse import mybir
from concourse.bass_utils import run_bass_kernel_spmd

F32 = mybir.dt.float32
BF16 = mybir.dt.bfloat16
F8 = mybir.dt.float8e4
AF = mybir.ActivationFunctionType
DR = mybir.MatmulPerfMode.DoubleRow

B, N, DIN, U, FD, SD = 32, 2048, 2, 64, 32, 64
NCORES = 8
BL = B // NCORES          # batches per core
NTW = 512                 # n-tile width
NT = N // NTW             # 4 n-tiles
NBW = 128                 # node-block width
NB = N // NBW             # 16 node blocks
NSB = NB // 2             # 8 node super-blocks (256 nodes, DoubleRow)
FROWS = DIN + U           # 66

C_ADJ = 64.0              # host scale on S^T before e4m3 cast
C_Y = 16.0                # host scale folded into support kernels
G = C_ADJ * C_Y           # net scale of the PSUM accumulators
SCL = 0.25 / G            # ACT scale for the gate activations (incl /M)


def _build_program():
    nc = bacc.Bacc("TRN2", debug=False, num_devices=NCORES)

    d = {}

    def din(name, shape, dt):
        d[name] = nc.dram_tensor(name, shape, dt, kind="ExternalInput").ap()

    din("xT", [BL, FROWS, N], BF16)
    din("hT", [BL, U, N], F32)
    din("a1T", [N, N], F8)
    din("a2T", [N, N], F8)
    din("fsT", [FD + SD, N], BF16)
    # kernels are packed into one tensor: each dma_start holds the
    # (single, serialized) HWDGE device ~630ns regardless of size
    din("wqk", [FD, 2 * U], BF16)            # [wq | wk]
    din("ws1", [FD + SD, U], BF16)
    din("ws2", [U, 1], BF16)
    din("kblob", [FROWS, 768], BF16)         # [kkall|kk0|kcall|kc0]
    din("bs1v", [U, 1], F32)
    din("bruh", [2 * U, 1], F32)
    din("bc2", [2 * U, 1], F32)
    din("bs2v", [1, 1], F32)
    out_h = nc.dram_tensor("out", [BL, U, N], F32, kind="ExternalOutput").ap()
    uscr = nc.dram_tensor("uscr", [BL, U, N], F32).ap()

    with tile.TileContext(nc) as tc:
        _emit(tc, d, out_h, uscr)
    nc.compile()
    return nc


def _emit(tc, d, out_h, uscr):
    nc = tc.nc
    ctx = contextlib.ExitStack()
    const = ctx.enter_context(tc.tile_pool(name="const", bufs=1))
    persist = ctx.enter_context(tc.tile_pool(name="persist", bufs=1))
    ypool = ctx.enter_context(tc.tile_pool(name="ypool", bufs=1))
    stage = ctx.enter_context(tc.tile_pool(name="stage", bufs=3))
    # phase-2/3 pipeline tiles: 4 pair-tile tails in flight, so the h/u
    # prefetch and the tanh of tile t+1 never wait on tile t's drain
    p3p = ctx.enter_context(tc.tile_pool(name="p3p", bufs=4))
    ctp = ctx.enter_context(tc.tile_pool(name="ctp", bufs=4))
    dsp = ctx.enter_context(tc.tile_pool(name="dsp", bufs=1))
    psacc = ctx.enter_context(tc.tile_pool(name="psacc", bufs=4, space="PSUM"))
    psscr = ctx.enter_context(tc.tile_pool(name="psscr", bufs=4, space="PSUM"))

    # ---- constants / weights in SBUF ----
    def cload(name, shape=None, dt=None):
        ap = d[name]
        t = const.tile(list(ap.shape) if shape is None else shape,
                       ap.dtype if dt is None else dt, name=f"c_{name}")
        nc.sync.dma_start(out=t, in_=ap)
        return t

    # DMA order matters: the startup critical path is
    #   fsT/wqk (prelude) and xT/kblob (y-gen), then adj slice t=0.
    fsT = const.tile([FD + SD, N], BF16, name="c_fsT")
    nc.sync.dma_start(out=fsT[:, 0:NTW], in_=d["fsT"][:, 0:NTW])
    wqk = cload("wqk")
    nc.sync.dma_start(out=fsT[:, NTW:], in_=d["fsT"][:, NTW:])
    wq, wk = wqk[:, 0:U], wqk[:, U:2 * U]
    kblob = cload("kblob")
    kkall = kblob[:, 0:384]
    kk0 = kblob[:, 384:512]
    kcall = kblob[:, 512:704]
    kc0 = kblob[:, 704:768]

    xTall = persist.tile([FROWS, BL, N], BF16, name="xTall", tag="xTall")
    # two DMAs so y-gen for b=0/1 can start before b=2/3 lands
    nc.sync.dma_start(out=xTall[:, 0:2, :],
                      in_=d["xT"][0:2].rearrange("b f n -> f b n"))
    nc.sync.dma_start(out=xTall[:, 2:4, :],
                      in_=d["xT"][2:4].rearrange("b f n -> f b n"))
    xT = [xTall[:, b, :] for b in range(BL)]

    ws1 = cload("ws1")
    ws2 = cload("ws2")
    bs1v = cload("bs1v")
    bruh = cload("bruh")        # pre-halved host-side for the tanh trick
    bc2 = cload("bc2")
    bs2v = cload("bs2v")

    # ---- resident adjacency slices (fp8, reused by both phases) ----
    a1t = [persist.tile([NBW, NB, NTW], F8, name=f"a1T_{t}", tag=f"a1T_{t}")
           for t in range(NT)]
    a2t = [persist.tile([NBW, NB, NTW], F8, name=f"a2T_{t}", tag=f"a2T_{t}")
           for t in range(NT)]
    for t in range(NT):
        sl = slice(t * NTW, (t + 1) * NTW)
        for name, lst in (("a1T", a1t), ("a2T", a2t)):
            nc.sync.dma_start(
                out=lst[t],
                in_=d[name][:, sl].rearrange("(j p) w -> p j w", p=NBW))

    # k-plane stride of a DoubleRow LDWEIGHTS AP must be 16-byte aligned,
    # so pad the ones column out to 16 bytes per plane
    ones8f = const.tile([NBW, 2, 16], F8, name="ones8")
    nc.vector.memset(ones8f, 1.0)
    ones8 = ones8f[:, :, 0:1]
    ones_row = const.tile([1, NBW], BF16, name="ones_row")
    nc.vector.memset(ones_row, C_ADJ)

    # PE p-state warm-up: the model throttles matmuls until 3us of
    # continuous execution. Run const-data fillers during the initial
    # weight-DMA wait so the array is at full clock when real work lands.
    fillm = const.tile([1, NTW], BF16, name="fillm")
    nc.vector.memset(fillm, 1.0)
    for _ in range(5):
        pf = psscr.tile([NBW, NTW], F32, name="pf", tag="scr")
        nc.tensor.matmul(pf, ones_row, fillm, start=True, stop=True)

    QT = persist.tile([U // 2, 2, N], F8, name="QT", tag="QT")
    KT = persist.tile([U // 2, 2, N], F8, name="KT", tag="KT")
    s_row = persist.tile([1, N], BF16, name="s_row", tag="s_row")
    rdbc = [persist.tile([NBW, NTW], BF16, name=f"rdbc{t}", tag=f"rdbc{t}")
            for t in range(NT)]
    ets = [persist.tile([NBW, NB, NTW], F8, name=f"et{t}", tag=f"et{t}")
           for t in range(NT)]

    # ---- prelude: K^T first (e-gen of t=0 reads all of K), then Q^T
    # (fp8 split-u packing), then s (not needed until the first dchain) ----
    for w, qt in ((wk, KT), (wq, QT)):
        for t in range(NT):
            sl = slice(t * NTW, (t + 1) * NTW)
            for half in range(2):
                pq = psscr.tile([U // 2, NTW], F32, name="pq", tag="scr")
                nc.tensor.matmul(pq, w[:, half * 32:(half + 1) * 32],
                                 fsT[0:FD, sl], start=True, stop=True)
                # alternate engines; both have startup slack
                if half:
                    nc.scalar.activation(qt[:, half, sl], pq, AF.Relu)
                else:
                    nc.vector.tensor_scalar_max(qt[:, half, sl], pq, 0.0)
    for t in range(NT):
        sl = slice(t * NTW, (t + 1) * NTW)
        ps1 = psscr.tile([U, NTW], F32, name="ps1", tag="scr")
        nc.tensor.matmul(ps1, ws1, fsT[:, sl], start=True, stop=True)
        s1t = stage.tile([U, NTW], BF16, name="s1t", tag="s1t")
        nc.scalar.activation(s1t, ps1, AF.Relu, bias=bs1v)
        ps2 = psscr.tile([1, NTW], F32, name="ps2", tag="scr")
        nc.tensor.matmul(ps2, ws2, s1t, start=True, stop=True)
        nc.scalar.activation(s_row[:, sl], ps2, AF.Relu, bias=bs2v)

    # ---- Y tiles: Y[m,b] = C_Y * X_b @ [k_r[m]|k_u[m]], stored fp8
    # [128, NB, 3, 128]: [node%128, node//128, m, u'] ----
    y = [ypool.tile([NBW, NB, 3, 2 * U], F8, name=f"y_{b}", tag=f"y{b}")
         for b in range(BL)]

    _eng = [0]

    def evac(out_ap, in_ap, dve_share=1):
        """PSUM evacuation, rotating DVE / ACT to balance load
        (`dve_share` DVE copies per ACT copy; -1 = DVE only, 0 = ACT
        only)."""
        if dve_share < 0:
            nc.vector.tensor_copy(out_ap, in_ap)
            return
        _eng[0] = (_eng[0] + 1) % (dve_share + 1)
        if _eng[0]:
            nc.vector.tensor_copy(out_ap, in_ap)
        else:
            nc.scalar.activation(out_ap, in_ap, AF.Copy)

    def ygen_thunks(b, dve_share=1):
        def mk(j):
            def f():
                nsl = slice(j * NBW, (j + 1) * NBW)
                py = psscr.tile([NBW, 3 * 2 * U], F32, name="py", tag="scr")
                nc.tensor.matmul(py, xT[b][:, nsl], kkall, start=True,
                                 stop=True)
                evac(y[b][:, j, :, :], py.rearrange("p (m u) -> p m u", m=3),
                     dve_share=dve_share)
            return f
        return [mk(j) for j in range(NB)]

    # y-gen for b=0/1, with tile-0's e^T generation interleaved so et[0]
    # is ready before the t-loop needs it (every later tile's e^T is
    # generated one tile ahead inside agroups, keeping the ACT exp burst
    # off the critical path)
    def e_thunks(t, et):
        """e^T[:, t] = exp(K Q^T / 8) into et (fp8), one node-block per
        call."""
        sl = slice(t * NTW, (t + 1) * NTW)

        def mk(j):
            def f():
                pe = psscr.tile([NBW, NTW], F32, name="pe", tag="scr")
                nc.tensor.matmul(pe, KT[:, :, j * NBW:(j + 1) * NBW],
                                 QT[:, :, sl], start=True, stop=True,
                                 perf_mode=DR)
                nc.scalar.activation(et[:, j, :], pe, AF.Exp, scale=0.125)
            return f
        return [mk(j) for j in range(NB)]

    def emit_dsum(et, pd):
        for j in range(NSB):
            nc.tensor.matmul(pd, ones8, et[:, 2 * j:2 * j + 2, :],
                             start=(j == 0), stop=(j == NSB - 1),
                             perf_mode=DR)

    def interleave(main, extra, ratio=2):
        """Emit `ratio` thunks from main per one from extra."""
        mi = ei = 0
        while mi < len(main) or ei < len(extra):
            for _ in range(ratio):
                if mi < len(main):
                    main[mi](); mi += 1
            if ei < len(extra):
                extra[ei](); ei += 1

    # y-gen for b=0/1, with tile-0's e^T generation interleaved so et[0]
    # is ready before the t-loop needs it (every later tile's e^T is
    # generated one tile ahead inside agroups, keeping the ACT exp burst
    # off the critical path)
    interleave(ygen_thunks(0, dve_share=2) + ygen_thunks(1, dve_share=2),
               e_thunks(0, ets[0]), ratio=2)

    # =================== phase 1: r & u gates ===================
    def a_thunks1(b, t, sl, pa):
        th = [lambda: nc.tensor.matmul(pa, kk0, xT[b][:, sl],
                                       start=True, stop=False)]
        for m, asl in ((0, a1t[t]), (1, a2t[t])):
            for j in range(NSB):
                def f(m=m, asl=asl, j=j):
                    nc.tensor.matmul(pa, y[b][:, 2 * j:2 * j + 2, m, :],
                                     asl[:, 2 * j:2 * j + 2, :],
                                     start=False,
                                     stop=(m == 1 and j == NSB - 1),
                                     perf_mode=DR)
                th.append(f)
        return th

    def agroup1(b, t, sl, extra=(), ratio=2):
        pa = psacc.tile([128, NTW], F32, name="pa", tag="acc")
        interleave(a_thunks1(b, t, sl, pa), list(extra), ratio=ratio)
        return pa

    def pbgroup(b, t, et, ytile=None, m=2):
        """The e-support contraction (PE only, so it can be emitted away
        from the DVE/ACT finish chain)."""
        yt = y[b] if ytile is None else ytile
        pb = psacc.tile([128, NTW], F32, name="pb", tag="acc")
        for j in range(NSB):
            nc.tensor.matmul(pb, yt[:, 2 * j:2 * j + 2, m, :],
                             et[:, 2 * j:2 * j + 2, :],
                             start=(j == 0), stop=(j == NSB - 1),
                             perf_mode=DR)
        return pb

    def bfinish1(b, t, sl, pa, pb):
        tmp = stage.tile([128, NTW], F32, name="tmp", tag="tmp")
        nc.vector.tensor_mul(tmp, pb, rdbc[t])
        ssum = stage.tile([128, NTW], F32, name="ssum", tag="ssum")
        nc.vector.tensor_add(ssum, pa, tmp)
        # sigmoid(z) = 0.5*(1 + tanh(z/2)): Tanh lives in the same ACT
        # function table as Exp/Copy, so phase 1 never reloads the table
        # (Sigmoid does not; a reload is 1283ns). sig holds th = 2r-1 /
        # 2u-1; the affine fixups are folded downstream.
        sig = stage.tile([128, NTW], F32, name="sig", tag="sig")
        nc.scalar.activation(sig, ssum, AF.Tanh, scale=SCL / 2, bias=bruh)
        # rh' = (1+th)*h = 2*(r*h) -> rows 0:64 in place as th*h + h
        # (the 0.5 is folded into the state rows of kc0/kcall host-side);
        # two plain tensor-tensor ops on the otherwise idle GpSimd.
        # thu -> DRAM scratch.
        rhs = stage.tile([U, NTW], BF16, name="rhs", tag="s1t")
        nc.gpsimd.tensor_mul(rhs, sig[0:U, :], xT[b][0:U, sl])
        nc.gpsimd.tensor_add(xT[b][0:U, sl], rhs, xT[b][0:U, sl])
        ueng = nc.scalar if b % 2 else nc.sync
        ueng.dma_start(out=uscr[b][:, sl], in_=sig[U:128, :])

    def ycgen_thunks(yct, p, dve_share=1):
        th = []
        for half in range(2):
            b = 2 * p + half
            usl = slice(half * U, (half + 1) * U)
            for j in range(NB):
                def f(b=b, usl=usl, j=j, yct=yct):
                    nsl = slice(j * NBW, (j + 1) * NBW)
                    pyc = psscr.tile([NBW, 3 * U], F32, name="pyc", tag="scr")
                    nc.tensor.matmul(pyc, xT[b][:, nsl], kcall,
                                     start=True, stop=True)
                    evac(yct[:, j, :, usl],
                         pyc.rearrange("p (m u) -> p m u", m=3),
                         dve_share=dve_share)
                th.append(f)
        return th

    yc = [None, None]

    # Per-tile emission order keeps PE fed continuously:
    #   ag0(+e) ag1(+e) ag2 pb0 d8 [dchain] ag3 fin0 pb1 fin1 pb2 fin2
    #   pb3 fin3
    # The PSUM "acc" live set never exceeds 5 (pa0-3 + one pb).
    for t in range(NT):
        sl = slice(t * NTW, (t + 1) * NTW)
        et = ets[t]
        # generate NEXT tile's e^T inside this tile's agroups (this
        # tile's was generated one tile ago)
        eth = e_thunks(t + 1, ets[t + 1]) if t + 1 < NT else []
        if t == 0:
            y2g, y3g = ygen_thunks(2, dve_share=2), ygen_thunks(3, dve_share=2)
            pa0 = agroup1(0, t, sl, eth[0:8], ratio=1)
        else:
            pa0 = agroup1(0, t, sl, eth[0:8], ratio=2)

        # d/rdbc right after ag0 — et[t] was generated one tile ago, so
        # the serial d8 -> dsb -> 1/d -> broadcast chain hides under the
        # remaining agroups instead of stalling the finishes
        pd = psscr.tile([1, NTW], F32, name="pd", tag="scr")
        emit_dsum(et, pd)
        # d[n] = s[n] + colsum(e^T)[n]; rdbc[t][p, n] = C_ADJ / d[n]
        dsb = dsp.tile([1, NTW], F32, name="dsb", tag="dsb")
        nc.vector.tensor_add(dsb, pd, s_row[:, sl])
        rds = dsp.tile([1, NTW], BF16, name="rds", tag="rds")
        with nc.allow_low_precision(reason="1/d feeds an fp8-noise-dominated "
                                    "term; bf16 is plenty"):
            nc.vector.reciprocal(rds, dsb)
        pr = psscr.tile([128, NTW], F32, name="pr", tag="scr")
        nc.tensor.matmul(pr, ones_row, rds, start=True, stop=True)
        nc.vector.tensor_copy(rdbc[t], pr)

        if t == 0:
            pa1 = agroup1(1, t, sl, eth[8:16] + y2g, ratio=1)
        else:
            pa1 = agroup1(1, t, sl, eth[8:16], ratio=2)
        pb0 = pbgroup(0, t, et)
        bfinish1(0, t, sl, pa0, pb0)
        if t == 0:
            pa2 = agroup1(2, t, sl, y3g, ratio=1)
        else:
            pa2 = agroup1(2, t, sl)
        pb1 = pbgroup(1, t, et)
        bfinish1(1, t, sl, pa1, pb1)
        pa3 = agroup1(3, t, sl)
        if t == NT - 1:
            # yc[0] gen needs the rh updates of b0/b1 (fin0/fin1) above;
            # interleave it with the remaining e-support contractions
            yc[0] = ypool.tile([NBW, NB, 3, 2 * U], F8, name="yc_0", tag="y0")
            yc0g = ycgen_thunks(yc[0], 0)
            pb23 = []
            th23 = [lambda: pb23.append(pbgroup(2, t, et)),
                    lambda: pb23.append(pbgroup(3, t, et))]
            interleave(th23, yc0g, ratio=1)
            bfinish1(2, t, sl, pa2, pb23[0])
            bfinish1(3, t, sl, pa3, pb23[1])
        else:
            for b, pa in ((2, pa2), (3, pa3)):
                pb = pbgroup(b, t, et)
                bfinish1(b, t, sl, pa, pb)

    # =================== phase 2+3: c gate & h_new ===================
    # Yc[pair] = C_Y * [Xc_b0 @ kc[m] | Xc_b1 @ kc[m]], fp8
    # (yc[0] was already generated inside phase-1's final A-group; yc[1]
    # is interleaved into phase-2 t=0's first A-group below, on the
    # mostly idle ACT)
    yc[1] = ypool.tile([NBW, NB, 3, 2 * U], F8, name="yc_1", tag="y1")
    yc1g = ycgen_thunks(yc[1], 1)

    for t in range(NT):
        sl = slice(t * NTW, (t + 1) * NTW)
        et = ets[t]
        a1, a2 = a1t[t], a2t[t]
        # prefetch h and u for the tail chains, one pair-merged DMA each
        hps, ups = [], []
        for p in range(BL // 2):
            hp = p3p.tile([128, NTW], F32, name="hp", tag="hp")
            up = p3p.tile([128, NTW], F32, name="up", tag="up")
            psl = slice(2 * p, 2 * p + 2)
            # the SBUF side stays a plain [128, 512] AP: its (partition,
            # col) iteration order already matches the DRAM side's
            # (b, u, w) — splitting the SBUF partition dim via rearrange
            # produces corrupt flat-offset strides
            nc.sync.dma_start(out=hp, in_=d["hT"][psl, :, sl])
            nc.sync.dma_start(out=up, in_=uscr[psl, :, sl])
            # uscr holds thu = 2u-1; recover u = 0.5*thu + 0.5 in place,
            # off the tail's critical path (the chain below then uses
            # only plain tensor-tensor ops)
            nc.vector.tensor_scalar(up, up, 1.0, 0.5,
                                    op0=mybir.AluOpType.add,
                                    op1=mybir.AluOpType.mult)
            hps.append(hp)
            ups.append(up)
        for p in range(BL // 2):
            b0, b1 = 2 * p, 2 * p + 1
            pa = psacc.tile([128, NTW], F32, name="pa2", tag="acc")
            th = [lambda pa=pa, p=p: nc.tensor.matmul(
                      pa, yc[p][:, 0:2, 0, :], a1[:, 0:2, :],
                      start=True, stop=False, perf_mode=DR),
                  lambda pa=pa, b0=b0: nc.tensor.matmul(
                      pa[0:U, :], kc0, xT[b0][:, sl], start=False, stop=False),
                  lambda pa=pa, b1=b1: nc.tensor.matmul(
                      pa[U:128, :], kc0, xT[b1][:, sl], start=False,
                      stop=False)]
            for m, asl in ((0, a1), (1, a2)):
                for j in range(NSB):
                    if m == 0 and j == 0:
                        continue
                    def f(pa=pa, p=p, m=m, asl=asl, j=j):
                        nc.tensor.matmul(pa, yc[p][:, 2 * j:2 * j + 2, m, :],
                                         asl[:, 2 * j:2 * j + 2, :],
                                         start=False,
                                         stop=(m == 1 and j == NSB - 1),
                                         perf_mode=DR)
                    th.append(f)
            hp, up = hps[p], ups[p]
            pb = pbgroup(p, t, et, ytile=yc[p])
            if t == 0 and p == 0:
                interleave(th, yc1g, ratio=1)
            else:
                for f in th:
                    f()
            tmp = stage.tile([128, NTW], F32, name="tmp2", tag="tmp")
            ct = ctp.tile([128, NTW], F32, name="ct", tag="ct")
            ssum = stage.tile([128, NTW], F32, name="ssum2", tag="ssum")
            # run the gate + h_new chain in column halves so the DVE / ACT
            # / GpSimd / DMA stages pipeline; the elementwise h_new chain
            # (h_new = c + u*(h - c), in place on hp) runs on the otherwise
            # idle GpSimd except on the final tile, where its latency is
            # exposed and DVE (2x f32 SBUF mode) drains faster.
            # the h_new chain stays OFF the DVE queue (in-order: a chain op
            # waiting on tanh would block the next pair's tmp/ssum) except
            # on the final tile, where DVE has nothing left queued and
            # drains faster than GpSimd
            veng = nc.vector if t == NT - 1 else nc.gpsimd
            for c0 in range(0, NTW, NTW // 2):
                cs = slice(c0, c0 + NTW // 2)
                nc.vector.tensor_mul(tmp[:, cs], pb[:, cs], rdbc[t][:, cs])
                nc.vector.tensor_add(ssum[:, cs], pa[:, cs], tmp[:, cs])
                nc.scalar.activation(ct[:, cs], ssum[:, cs], AF.Tanh,
                                     scale=SCL, bias=bc2)
                # h_new = c + u*(h - c), in place on hp
                veng.tensor_sub(hp[:, cs], hp[:, cs], ct[:, cs])
                veng.tensor_mul(hp[:, cs], up[:, cs], hp[:, cs])
                veng.tensor_add(hp[:, cs], hp[:, cs], ct[:, cs])
            # one pair-merged output DMA per (pair, tile)
            nc.scalar.dma_start(out=out_h[b0:b1 + 1, :, sl], in_=hp)

    ctx.close()


_CACHE = {}


def _get_program():
    if "nc" not in _CACHE:
        _CACHE["nc"] = _build_program()
    return _CACHE["nc"]


def _prep_inputs(inputs, h_prev, adj1, adj2, feat, SE, Wq, Wk, Ws1, bs1, Ws2,
                 bs2, r_kernel, r_bias, u_kernel, u_bias, c_kernel, c_bias):
    bf = ml_dtypes.bfloat16
    f8 = ml_dtypes.float8_e4m3
    f32 = np.float32
    perm = list(range(DIN, FROWS)) + list(range(DIN))  # [h(64); inputs(2)]

    h3 = np.asarray(h_prev, f32).reshape(B, N, U)
    hT = np.ascontiguousarray(h3.transpose(0, 2, 1))            # [B, U, N]
    inT = np.asarray(inputs, f32).transpose(0, 2, 1)            # [B, DIN, N]
    xT = np.concatenate([hT, inT], axis=1).astype(bf)           # [B, 66, N]

    rk = np.asarray(r_kernel, f32)[:, perm, :]
    uk = np.asarray(u_kernel, f32)[:, perm, :]
    ck = np.asarray(c_kernel, f32)[:, perm, :]
    # the device keeps rh' = 2*(r*h) in the state rows of x_cat_c (tanh
    # trick), so halve the state rows of the c kernels to compensate
    ck = ck.copy()
    ck[:, 0:U, :] *= 0.5
    kkall = (np.concatenate(
        [np.concatenate([rk[m], uk[m]], axis=1) for m in (1, 2, 3)],
        axis=1) * C_Y).astype(bf)                               # [66, 384]
    kk0 = (np.concatenate([rk[0], uk[0]], axis=1) * G).astype(bf)
    kcall = (np.concatenate([ck[1], ck[2], ck[3]], axis=1) * C_Y).astype(bf)
    kc0 = (ck[0] * G).astype(bf)

    shared = {
        "a1T": np.ascontiguousarray(
            np.asarray(adj1, f32).T * C_ADJ).astype(f8),
        "a2T": np.ascontiguousarray(
            np.asarray(adj2, f32).T * C_ADJ).astype(f8),
        "fsT": np.ascontiguousarray(
            np.concatenate([np.asarray(feat, f32).T, np.asarray(SE, f32).T],
                           axis=0)).astype(bf),
        "wqk": np.concatenate([np.asarray(Wq, f32), np.asarray(Wk, f32)],
                              axis=1).astype(bf),
        "ws1": np.asarray(Ws1, f32).astype(bf),
        "ws2": np.asarray(Ws2, f32).reshape(U, 1).astype(bf),
        "kblob": np.concatenate([kkall, kk0, kcall, kc0], axis=1),
        "bs1v": np.asarray(bs1, f32).reshape(U, 1),
        "bruh": 0.5 * np.concatenate(
            [np.asarray(r_bias, f32).mean(0),
             np.asarray(u_bias, f32).mean(0)]).reshape(-1, 1),
        "bc2": np.tile(np.asarray(c_bias, f32).mean(0), 2).reshape(-1, 1),
        "bs2v": np.asarray(bs2, f32).reshape(1, 1),
    }
    in_maps = []
    for c in range(NCORES):
        bsl = slice(c * BL, (c + 1) * BL)
        m = dict(shared)
        m["xT"] = np.ascontiguousarray(xT[bsl])
        m["hT"] = np.ascontiguousarray(hT[bsl])
        in_maps.append(m)
    return in_maps


def kernel(**inputs):
    os.environ.setdefault("NEURON_RT_RESET_CORES", "1")
    nc = _get_program()
    in_maps = _prep_inputs(**inputs)
    res = None
    err = None
    for _ in range(2):
        try:
            res = run_bass_kernel_spmd(nc, in_maps, list(range(NCORES)))
            break
        except Exception as e:  # e.g. a wedged device; retry once
            err = e
    if res is None:
        raise err
    outs = []
    for c in range(NCORES):
        o = res.results[c]["out"]                     # [BL, U, N] f32
        outs.append(o.transpose(0, 2, 1).reshape(BL, N * U))
    return np.concatenate(outs, axis=0).astype(np.float32)
